# revision 27
# baseline (speedup 1.0000x reference)
"""BertCRF forward (BERT-base encoder + CRF NLL) on 8 Trainium2 NeuronCores.

Strategy: data-parallel over the batch (32 examples -> 4 per core), params
replicated.  Each core runs the full 12-layer encoder on its 1024 tokens with
fp8 (DoubleRow) matmuls for the dense projections, bf16 attention, fp32
layernorm, and a max-free softmax whose normalizer is computed by an extra
ones-matmul on the PE and inverted with a fast DVE reciprocal.  The CRF
numerator's label-only terms (start/transition/end) are computed on the host;
the device computes the gold-emission dot product and the exact linear-space
forward scan with a fixed per-step shift (cancels exactly between numerator
and denominator).  The host shards inputs, pre-arranges weight layouts, and
sums the 8 per-core partial NLLs plus the host-side label constant.
"""

import contextlib
import os

import numpy as np
import ml_dtypes

import concourse.bass as bass  # noqa: F401
import concourse.mybir as mybir
import concourse.tile as tile
from concourse import bacc
from concourse.bass import IndirectOffsetOnAxis
from concourse.bass_utils import run_bass_kernel_spmd
from concourse.masks import make_identity

# ---- problem constants (hardcoded per the task spec) ----
L, H, NH, DH, FF, V, K = 12, 768, 12, 64, 3072, 30522, 9
B, S = 32, 256
CORES = 8
BC = B // CORES          # 4 examples per core
T = BC * S               # 1024 tokens per core
P = 128
NT = T // P              # 8 token tiles
HC = H // P              # 6 hidden chunks
FC = FF // P             # 24 ff chunks
C_SHIFT = 2.35           # per-step CRF shift (cancels exactly in num-denom)

F32 = mybir.dt.float32
BF16 = mybir.dt.bfloat16
FP8 = mybir.dt.float8e4
I32 = mybir.dt.int32
AX = mybir.AxisListType
OP = mybir.AluOpType
AF = mybir.ActivationFunctionType
DR = mybir.MatmulPerfMode.DoubleRow

BF = ml_dtypes.bfloat16

# fp8 quantization scales (exact powers of two)
WS = 1024.0              # weight scale into fp8e4
XS = 32.0                # activation scale into fp8e4
DQ_WX = 2.0 ** -15       # dequant for w*x products
DQ_W = 2.0 ** -10        # dequant when only the weight was scaled


def _bf(x):
    return np.ascontiguousarray(np.asarray(x, dtype=np.float32)).astype(BF)


def _f8(x, scale):
    return np.ascontiguousarray(np.clip(
        np.asarray(x, dtype=np.float32) * scale, -240.0, 240.0)
    ).astype(ml_dtypes.float8_e4m3)


def _f32(x):
    return np.ascontiguousarray(np.asarray(x, dtype=np.float32))


# ---------------------------------------------------------------------------
# device program
# ---------------------------------------------------------------------------

def _layernorm(nc, tmp, out_bf, xf, gb, s1=None, sq_act=True):
    """LN over the free dim of xf [P, H] f32 -> out_bf (bf16).

    s1, if given, is a [P, 1] tile already holding sum(xf) (computed for free
    via accum_out on the op that produced xf).  sq_act picks the engine for
    the sum-of-squares pass: ACT (Square is in every table set) when ACT has
    headroom, DVE when ACT is the busier engine (attention phase).
    """
    if s1 is None:
        s1 = tmp.tile([P, 1], F32, tag="s1", name="s1")
        nc.vector.tensor_reduce(out=s1[:], in_=xf[:], axis=AX.X, op=OP.add)
    sq = tmp.tile([P, H], F32, tag="sq", name="sq")
    s2 = tmp.tile([P, 1], F32, tag="s2", name="s2")
    if sq_act:
        nc.scalar.activation(sq[:], xf[:], AF.Square, accum_out=s2[:])
    else:
        nc.vector.scalar_tensor_tensor(out=sq[:], in0=xf[:], scalar=1.0,
                                       in1=xf[:], op0=OP.mult, op1=OP.mult,
                                       accum_out=s2[:])
    m = tmp.tile([P, 1], F32, tag="m", name="m")
    nc.vector.tensor_scalar(out=m[:], in0=s1[:], scalar1=1.0 / H, scalar2=None,
                            op0=OP.mult)
    msq = tmp.tile([P, 1], F32, tag="msq", name="msq")
    nc.vector.tensor_tensor(out=msq[:], in0=m[:], in1=m[:], op=OP.mult)
    var = tmp.tile([P, 1], F32, tag="var", name="var")
    nc.vector.tensor_scalar(out=var[:], in0=s2[:], scalar1=1.0 / H,
                            scalar2=msq[:, :1], op0=OP.mult, op1=OP.subtract)
    # eps=1e-12 is below f32 resolution for var~O(1); bias=0.0 is identical
    sd = tmp.tile([P, 1], F32, tag="sd", name="sd")
    nc.scalar.activation(sd[:], var[:], AF.Sqrt, bias=0.0)
    rs = tmp.tile([P, 1], F32, tag="rs", name="rs")
    nc.vector.reciprocal_approx_fast(rs[:], sd[:])
    if gb is None:
        # out = rs*x - m*rs, one half on DVE, one half on ACT (in parallel)
        nmrs = tmp.tile([P, 1], F32, tag="nmrs", name="nmrs")
        nc.vector.tensor_scalar(out=nmrs[:], in0=m[:], scalar1=-1.0,
                                scalar2=rs[:, :1], op0=OP.mult, op1=OP.mult)
        HH = H // 2
        nc.vector.tensor_scalar(out=out_bf[:, 0:HH], in0=xf[:, 0:HH],
                                scalar1=m[:, :1], scalar2=rs[:, :1],
                                op0=OP.subtract, op1=OP.mult)
        nc.scalar.activation(out_bf[:, HH:H], xf[:, HH:H], AF.Identity,
                             bias=nmrs[:, :1], scale=rs[:, :1])
    else:
        G, Bb = gb
        t2 = tmp.tile([P, H], F32, tag="t2", name="t2")
        nc.vector.tensor_scalar(out=t2[:], in0=xf[:], scalar1=m[:, :1],
                                scalar2=rs[:, :1], op0=OP.subtract, op1=OP.mult)
        t3 = tmp.tile([P, H], F32, tag="t3", name="t3")
        nc.vector.tensor_tensor(out=t3[:], in0=t2[:], in1=G[:], op=OP.mult)
        nc.vector.tensor_tensor(out=out_bf, in0=t3[:], in1=Bb[:], op=OP.add)


def _transpose_tiles(nc, ps, xT, x_sb, ident, tiles):
    """x_sb [P, NT, H] token-major -> xT [P, HC, T] feature-major, per tile.

    All six 128x128 transposes of a tile go into ONE bf16 PSUM bank (768 of
    1024 cols) so a single ACT copy drains them: half the ACT-FIFO entries
    and half the PSUM-ring allocations of the two-group variant."""
    for g in tiles:
        pt_ = ps.tile([P, 1024], BF16, tag="ps", name="ptp", space="PSUM")
        for c in range(HC):
            nc.tensor.matmul(
                pt_[:, c * P:(c + 1) * P],
                lhsT=x_sb[:, g, c * P:(c + 1) * P], rhs=ident[:],
                start=True, stop=True, is_transpose=True)
        nc.scalar.activation(
            xT[:, :, g * P:(g + 1) * P],
            pt_[:, :768], AF.Identity, scale=XS)


def _bcast_row(nc, ps, tmp, dst, row_dram, ones_bf):
    """dst [P, H] bf16 = broadcast of a [1, H] bf16 dram row across partitions."""
    row = tmp.tile([1, H], BF16, tag="brow", name="brow")
    nc.sync.dma_start(row[:], row_dram)
    for fh in range(2):
        pb = ps.tile([P, 512], F32, tag="ps", name="pbc", space="PSUM")
        nc.tensor.matmul(pb[:, :384], lhsT=ones_bf[:, :P],
                         rhs=row[:, fh * 384:(fh + 1) * 384],
                         start=True, stop=True)
        nc.scalar.activation(dst[:, fh * 384:(fh + 1) * 384], pb[:, :384],
                             AF.Identity)


def build_nc(n_layers=L, mask_ones=True, zero_bias=True, unit_ln=True,
             debug=False):
    nc = bacc.Bacc("TRN2", target_bir_lowering=False, debug=False)

    dd = {}

    def din(name, shape, dtype):
        dd[name] = nc.dram_tensor(name, list(shape), dtype, kind="ExternalInput")
        return dd[name]

    def dout(name, shape, dtype):
        dd[name] = nc.dram_tensor(name, list(shape), dtype, kind="ExternalOutput")
        return dd[name]

    din("word", [V, H], BF16)
    din("ids", [NT, P, 1], I32)
    din("pt", [P, 2, H], BF16)
    din("wq", [L, P, 3, 2, H], FP8)
    din("wk", [L, P, 3, 2, H], FP8)
    din("wv", [L, P, 3, 2, H], FP8)
    din("wo", [L, P, 3, 2, H], FP8)
    din("w1", [L, FC, P, 3, 2, P], FP8)   # [l, j, ki, c2, ko, m]
    din("w2", [L, FC // 2, P, 2, H], FP8)  # [l, c2, ki, ko, n]
    din("clsw", [P, 3, 2, 16], FP8)      # K padded to 16
    din("clsb", [K, 1], F32)             # already shifted by -C_SHIFT
    din("mexp", [K, K], F32)             # exp(crf_trans)
    din("mexpt", [K, K], F32)            # exp(crf_trans).T
    din("mrep", [K, BC, 16, K], F32)     # exp(crf_trans) replicated 64x
    din("expstart", [K, 1], F32)         # exp(crf_start)
    din("expend", [K, 1], F32)           # exp(crf_end)
    din("oh9w", [K, T], F32)             # one-hot(labels) * emission weight
    if not mask_ones:
        din("maskrow", [1, T], I32)
        din("maskcols", [P, 2, BC], I32)
    if not zero_bias:
        din("bq", [L, P, HC], F32)
        din("bk", [L, P, HC], F32)
        din("b1", [L, P, FC], F32)
        din("bvrow", [L, 1, H], BF16)
        din("borow", [L, 1, H], BF16)
        din("b2row", [L, 1, H], BF16)
    if not unit_ln:
        din("lng", [L, 2, 1, H], BF16)
        din("lnb", [L, 2, 1, H], BF16)
        din("elng", [2, 1, H], BF16)

    dout("out", [1, 4], F32)
    if debug:
        dout("dbg_x0", [P, NT, H], BF16)
        dout("dbg_x", [P, NT, H], BF16)
        dout("dbg_em", [K, T], F32)

    _build_body(nc, dd, n_layers, mask_ones, zero_bias, unit_ln, debug)
    nc.compile()
    return nc


def _build_body(nc, dd, n_layers, mask_ones, zero_bias, unit_ln, debug):
    (word, ids, pt, wq_d, wk_d, wv_d, wo_d, w1_d, w2_d, clsw_d, clsb_d,
     mexp_d, expstart_d, expend_d, oh9w_d, out_d) = (
        dd["word"], dd["ids"], dd["pt"], dd["wq"], dd["wk"], dd["wv"],
        dd["wo"], dd["w1"], dd["w2"], dd["clsw"], dd["clsb"], dd["mexp"],
        dd["expstart"], dd["expend"], dd["oh9w"], dd["out"])
    if not mask_ones:
        maskrow_d = dd["maskrow"]
        maskcols_d = dd["maskcols"]
    if not zero_bias:
        bq_d, bk_d, b1_d = dd["bq"], dd["bk"], dd["b1"]
        bvrow_d, borow_d, b2row_d = dd["bvrow"], dd["borow"], dd["b2row"]
    if not unit_ln:
        lng_d, lnb_d, elng_d = dd["lng"], dd["lnb"], dd["elng"]
    with tile.TileContext(nc) as tc, contextlib.ExitStack() as octx:
        cst = octx.enter_context(tc.tile_pool(name="cst", bufs=1))
        act = octx.enter_context(tc.tile_pool(name="act", bufs=1))
        ps = octx.enter_context(tc.tile_pool(name="ps", bufs=8, space="PSUM"))

        # ---- persistent activation buffers ----
        x_sb = act.tile([P, NT, H], BF16, tag="x_sb", name="x_sb")
        xT = act.tile([P, HC, T], FP8, tag="xT", name="xT")
        qT = act.tile([P, HC, T], BF16, tag="qT", name="qT")
        kT = act.tile([P, HC, T], BF16, tag="kT", name="kT")
        vS = act.tile([P, NT, NH, DH], BF16, tag="vS", name="vS")
        cT = act.tile([P, HC, T], FP8, tag="cT", name="cT")
        hT = act.tile([P, FC, T // 2], FP8, tag="hT", name="hT")
        emT = act.tile([K, T], F32, tag="emT", name="emT")
        expEm = act.tile([K, T], F32, tag="expEm", name="expEm")

        # ---- constants ----
        ident = cst.tile([P, P], BF16, tag="ident", name="ident")
        make_identity(nc, ident[:])
        ones_bf = cst.tile([1, P], BF16, tag="ones_bf", name="ones_bf")
        nc.vector.memset(ones_bf[:], 1.0)
        # ones64: [128, 64] all-ones lhsT; sum over keys of exp(scores) into
        # one 64-partition half of the normalizer PSUM tile per head
        ones64 = cst.tile([P, DH], BF16, tag="ones64", name="ones64")
        nc.vector.memset(ones64[:], 1.0)
        onescol_f = cst.tile([P, 1], F32, tag="onescol_f", name="onescol_f")
        nc.vector.memset(onescol_f[:], 1.0)

        pt_sb = cst.tile([P, 2, H], BF16, tag="pt_sb", name="pt_sb")
        nc.sync.dma_start(pt_sb[:], pt[:])
        if not mask_ones:
            mcol = cst.tile([P, 2, BC], F32, tag="mcol", name="mcol")
        if not unit_ln:
            elnG = cst.tile([P, H], BF16, tag="elnG", name="elnG")
            elnB = cst.tile([P, H], BF16, tag="elnB", name="elnB")

        with contextlib.ExitStack() as ictx:
            wts = ictx.enter_context(tc.tile_pool(name="wts", bufs=1))
            tmp = ictx.enter_context(tc.tile_pool(name="tmp", bufs=3))

            if not unit_ln:
                _bcast_row(nc, ps, tmp, elnG, elng_d[0], ones_bf)
                _bcast_row(nc, ps, tmp, elnB, elng_d[1], ones_bf)

            # =========== embeddings ===========
            for g in range(NT):
                idx = tmp.tile([P, 1], I32, tag="idx", name="idx")
                nc.sync.dma_start(idx[:], ids[g])
                emb = tmp.tile([P, H], BF16, tag="emb", name="emb")
                nc.gpsimd.indirect_dma_start(
                    out=emb[:], out_offset=None, in_=word[:],
                    in_offset=IndirectOffsetOnAxis(ap=idx[:, :1], axis=0),
                )
                xf = tmp.tile([P, H], F32, tag="xf", name="xf")
                s1e = tmp.tile([P, 1], F32, tag="s1e", name="s1e")
                nc.vector.scalar_tensor_tensor(
                    out=xf[:], in0=emb[:], scalar=0.0,
                    in1=pt_sb[:, g % 2, :], op0=OP.add, op1=OP.add,
                    accum_out=s1e[:])
                _layernorm(nc, tmp, x_sb[:, g, :], xf,
                           None if unit_ln else (elnG, elnB), s1=s1e,
                           sq_act=(g < 4))
            if debug:
                nc.sync.dma_start(dd["dbg_x0"][:], x_sb[:])
            # prime xT tiles 0-3 for layer 0's QK proj t2=0
            _transpose_tiles(nc, ps, xT, x_sb, ident, range(4))

            if not mask_ones:
                mi = tmp.tile([P, 2, BC], I32, tag="mi", name="mi")
                nc.sync.dma_start(mi[:], maskcols_d[:])
                nc.vector.tensor_scalar(out=mcol[:], in0=mi[:], scalar1=1.0,
                                        scalar2=10000.0, op0=OP.subtract,
                                        op1=OP.mult)

            # =========== encoder layers ===========
            for l in range(n_layers):
                wq = wts.tile([P, 3, 2, H], FP8, tag="wq", name="wq")
                wk = wts.tile([P, 3, 2, H], FP8, tag="wk", name="wk")
                wv = wts.tile([P, 3, 2, H], FP8, tag="wv", name="wv")
                wo = wts.tile([P, 3, 2, H], FP8, tag="wo", name="wo")
                nc.sync.dma_start(wq[:], wq_d[l])
                nc.sync.dma_start(wk[:], wk_d[l])
                nc.sync.dma_start(wv[:], wv_d[l])
                nc.sync.dma_start(wo[:], wo_d[l])

                if not zero_bias:
                    bq_sb = wts.tile([P, HC], F32, tag="bq", name="bq")
                    bk_sb = wts.tile([P, HC], F32, tag="bk", name="bk")
                    b1_sb = wts.tile([P, FC], F32, tag="b1", name="b1")
                    nc.sync.dma_start(bq_sb[:], bq_d[l])
                    nc.sync.dma_start(bk_sb[:], bk_d[l])
                    nc.sync.dma_start(b1_sb[:], b1_d[l])
                    bvrow = wts.tile([1, H], BF16, tag="bvrow", name="bvrow")
                    borow = wts.tile([1, H], BF16, tag="borow", name="borow")
                    b2row = wts.tile([1, H], BF16, tag="b2row", name="b2row")
                    nc.sync.dma_start(bvrow[:], bvrow_d[l])
                    nc.sync.dma_start(borow[:], borow_d[l])
                    nc.sync.dma_start(b2row[:], b2row_d[l])
                if not unit_ln:
                    G1 = wts.tile([P, H], BF16, tag="G1", name="G1")
                    B1t = wts.tile([P, H], BF16, tag="B1t", name="B1t")
                    G2 = wts.tile([P, H], BF16, tag="G2", name="G2")
                    B2t = wts.tile([P, H], BF16, tag="B2t", name="B2t")
                    _bcast_row(nc, ps, tmp, G1, lng_d[l, 0], ones_bf)
                    _bcast_row(nc, ps, tmp, B1t, lnb_d[l, 0], ones_bf)
                    _bcast_row(nc, ps, tmp, G2, lng_d[l, 1], ones_bf)
                    _bcast_row(nc, ps, tmp, B2t, lnb_d[l, 1], ones_bf)

                # ---- qT/kT projections, token-half pipelined.  xT tiles 0-3
                #      were already transposed in the previous layer's FFN
                #      tail (or right after the embeddings for layer 0), so
                #      QK proj t2=0 can start while the previous layer's
                #      second-half LN2 chain is still draining; tiles 4-7 are
                #      transposed here in between. ----
                for t2 in range(2):
                    if t2 == 1:
                        _transpose_tiles(nc, ps, xT, x_sb, ident, range(4, 8))
                    for wmat, bname, dst in ((wq, "bq", qT), (wk, "bk", kT)):
                        for f in range(HC):
                            pm = ps.tile([P, 512], F32, tag="ps", name="pqk",
                                         space="PSUM")
                            for c2 in range(3):
                                nc.tensor.matmul(
                                    pm[:],
                                    lhsT=wmat[:, c2, :, f * P:(f + 1) * P],
                                    rhs=xT[:, 2 * c2:2 * c2 + 2,
                                           t2 * 512:(t2 + 1) * 512],
                                    start=(c2 == 0), stop=(c2 == 2),
                                    perf_mode=DR)
                            if zero_bias:
                                nc.vector.tensor_scalar(
                                    out=dst[:, f, t2 * 512:(t2 + 1) * 512],
                                    in0=pm[:], scalar1=DQ_WX, scalar2=None,
                                    op0=OP.mult)
                            else:
                                bias = (bq_sb if bname == "bq"
                                        else bk_sb)[:, f:f + 1]
                                nc.scalar.activation(
                                    dst[:, f, t2 * 512:(t2 + 1) * 512], pm[:],
                                    AF.Identity, bias=bias, scale=DQ_WX)

                # ---- V projection (token-major into vS) ----
                for g in range(NT):
                    for fh in range(2):
                        pm = ps.tile([P, 512], F32, tag="ps", name="pv",
                                     space="PSUM")
                        for c2 in range(3):
                            nc.tensor.matmul(
                                pm[:, :384],
                                lhsT=xT[:, 2 * c2:2 * c2 + 2,
                                        g * P:(g + 1) * P],
                                rhs=wv[:, c2, :, fh * 384:(fh + 1) * 384],
                                start=(c2 == 0),
                                stop=(c2 == 2 and zero_bias),
                                perf_mode=DR)
                        if not zero_bias:
                            nc.tensor.matmul(
                                pm[:, :384], lhsT=ones_bf[:, :P],
                                rhs=bvrow[:, fh * 384:(fh + 1) * 384],
                                start=False, stop=True)
                        nc.vector.tensor_scalar(
                            out=vS[:, g, 6 * fh:6 * fh + 6, :],
                            in0=pm[:, :384], scalar1=DQ_WX, scalar2=None,
                            op0=OP.mult)

                # ---- output proj + residual + LN1 (interleaved
                #      into the attention loop, per example) ----
                def _oproj_ln1(g, sq_act=True):
                    xf = tmp.tile([P, H], F32, tag="xf", name="xf")
                    s1a = tmp.tile([P, 1], F32, tag="s1a", name="s1a")
                    s1b = tmp.tile([P, 1], F32, tag="s1b", name="s1b")
                    for fh in range(2):
                        pm = ps.tile([P, 512], F32, tag="ps", name="po",
                                     space="PSUM")
                        for c2 in range(3):
                            nc.tensor.matmul(
                                pm[:, :384],
                                lhsT=cT[:, 2 * c2:2 * c2 + 2,
                                        g * P:(g + 1) * P],
                                rhs=wo[:, c2, :, fh * 384:(fh + 1) * 384],
                                start=(c2 == 0),
                                stop=(c2 == 2 and zero_bias),
                                perf_mode=DR)
                        if not zero_bias:
                            nc.tensor.matmul(
                                pm[:, :384], lhsT=ones_bf[:, :P],
                                rhs=borow[:, fh * 384:(fh + 1) * 384],
                                start=False, stop=True)
                        nc.vector.scalar_tensor_tensor(
                            out=xf[:, fh * 384:(fh + 1) * 384],
                            in0=pm[:, :384], scalar=DQ_WX,
                            in1=x_sb[:, g, fh * 384:(fh + 1) * 384],
                            op0=OP.mult, op1=OP.add,
                            accum_out=(s1a[:] if fh == 0 else s1b[:]))
                    s1g = tmp.tile([P, 1], F32, tag="s1g", name="s1g",
                                   bufs=4)
                    nc.vector.tensor_tensor(out=s1g[:], in0=s1a[:],
                                            in1=s1b[:], op=OP.add)
                    _layernorm(nc, tmp, x_sb[:, g, :], xf,
                               None if unit_ln else (G1, B1t), s1=s1g,
                               sq_act=sq_act)

                # ---- attention.  Per head-pair: QK^T (2 heads in separate
                #      PE row-groups), fused exp on ACT, then per head both
                #      the AxV matmul and a ones-matmul normalizer sum (z)
                #      on PE.  The normalize tail (fast reciprocal of z +
                #      multiply) is deferred one pair so it overlaps the
                #      next pair's matmul/exp front.  PSUM: 4 banks/pair ->
                #      two pairs in flight. ----
                def _attn_tail(st):
                    e, ch, prbz, pcx = st
                    rb2 = tmp.tile([P, S], F32, tag="rb", name="rb")
                    nc.vector.reciprocal_approx_fast(rb2[:], prbz[:, :S])
                    for hh in range(2):
                        r0 = hh * DH
                        nc.vector.tensor_tensor(
                            out=cT[r0:r0 + DH, ch, e * S:(e + 1) * S],
                            in0=pcx[:DH, hh * S:(hh + 1) * S],
                            in1=rb2[r0:r0 + DH, :], op=OP.mult)

                # QK^T contracts only 64 partitions; heads A/B live in
                # PE row-groups {0,1}/{2,3} (lhsT base 0/64), so
                # alternating their matmuls runs them concurrently.
                # The QK matmuls of pair i+1 are EMITTED before the AV/z
                # matmuls of pair i: PE executes in order, so this gives it
                # work to do while pair i's exp runs on ACT.
                def _emit_qk(e, ch):
                    pscs = [ps.tile([P, 512], F32, tag="ps", name="psc",
                                    space="PSUM") for _ in range(2)]
                    for kt in range(2):
                        for hh in range(2):
                            r0 = hh * DH
                            nc.tensor.matmul(
                                pscs[hh][:, kt * S:(kt + 1) * S],
                                lhsT=kT[r0:r0 + DH, ch,
                                        e * S + kt * P:
                                        e * S + (kt + 1) * P],
                                rhs=qT[r0:r0 + DH, ch, e * S:(e + 1) * S],
                                start=True, stop=True)
                    return pscs

                pairs = [(e, ch) for e in range(BC) for ch in range(NH // 2)]
                prev_st = None
                pscs = _emit_qk(*pairs[0])
                for i, (e, ch) in enumerate(pairs):
                    next_pscs = (_emit_qk(*pairs[i + 1])
                                 if i + 1 < len(pairs) else None)
                    ET2 = tmp.tile([P, 2, 2, S], BF16, tag="ET", name="ET")
                    pcx = ps.tile([P, 512], F32, tag="ps", name="pcx",
                                  space="PSUM")
                    prbz = ps.tile([P, 512], F32, tag="ps", name="prb",
                                   space="PSUM")
                    for hh in range(2):
                        r0 = hh * DH
                        psc = pscs[hh]
                        if mask_ones:
                            # one fused exp over both key tiles
                            nc.scalar.activation(
                                ET2[:, hh].rearrange("p k s -> p (k s)"),
                                psc[:], AF.Exp, bias=0.0, scale=0.125)
                        else:
                            for kt in range(2):
                                nc.scalar.activation(
                                    ET2[:, hh, kt, :],
                                    psc[:, kt * S:(kt + 1) * S],
                                    AF.Exp, bias=mcol[:, kt, e:e + 1],
                                    scale=0.125)
                        h = 2 * ch + hh
                        for kt in range(2):
                            nc.tensor.matmul(
                                pcx[:DH, hh * S:(hh + 1) * S],
                                lhsT=vS[:, 2 * e + kt, h, :],
                                rhs=ET2[:, hh, kt, :],
                                start=(kt == 0), stop=(kt == 1))
                        for kt in range(2):
                            nc.tensor.matmul(
                                prbz[r0:r0 + DH, :S],
                                lhsT=ones64[:],
                                rhs=ET2[:, hh, kt, :],
                                start=(kt == 0), stop=(kt == 1))
                    if prev_st is not None:
                        _attn_tail(prev_st)
                    prev_st = (e, ch, prbz, pcx)
                    pscs = next_pscs
                _attn_tail(prev_st)
                # tiles 4-7's LN1 square runs on DVE: the post-LN1 transpose
                # copies of tiles 0-3 (emitted at the FFN top) queue on the
                # ACT FIFO behind the remaining LN1 ACT ops, and FFN1-th0
                # can't start until those copies land — shrinking the late
                # tiles' ACT work moves FFN1-th0's start earlier
                for g in range(NT):
                    _oproj_ln1(g, sq_act=(g < 4))

                # ---- FFN (two token-half passes) ----
                for th in range(2):
                    tiles = list(range(4 * th, 4 * th + 4))
                    _transpose_tiles(nc, ps, xT, x_sb, ident, tiles)
                    for j in range(FC):
                        if th == 1 and j == FC - 1:
                            # post-LN2 re-transpose of tiles 0-3 for the next
                            # layer (or classifier), emitted here so its
                            # PSUM->xT copies drain on ACT during the FFN2
                            # window instead of behind the LN2 chain
                            _transpose_tiles(nc, ps, xT, x_sb, ident, range(4))
                        w1j = wts.tile([P, 3, 2, P], FP8, tag="w1j",
                                       name="w1j", bufs=4)
                        nc.sync.dma_start(w1j[:], w1_d[l, j])
                        pm = ps.tile([P, 512], F32, tag="ps", name="ph",
                                     space="PSUM")
                        for c2 in range(3):
                            nc.tensor.matmul(
                                pm[:], lhsT=w1j[:, c2],
                                rhs=xT[:, 2 * c2:2 * c2 + 2,
                                       th * 512:(th + 1) * 512],
                                start=(c2 == 0), stop=(c2 == 2),
                                perf_mode=DR)
                        bias = 0.0 if zero_bias else b1_sb[:, j:j + 1]
                        nc.scalar.activation(hT[:, j, :], pm[:], AF.Gelu,
                                             bias=bias, scale=DQ_WX)
                    # FFN2: f-half outer so W2 streams once per (th, fh)
                    xfs = [tmp.tile([P, H], F32, tag="xff", name="xff", bufs=4)
                           for _ in range(4)]
                    s1as = [tmp.tile([P, 1], F32, tag="s1fa", name="s1fa",
                                     bufs=4) for _ in range(4)]
                    s1bs = [tmp.tile([P, 1], F32, tag="s1fb", name="s1fb",
                                     bufs=4) for _ in range(4)]
                    for fh in range(2):
                        pms = [ps.tile([P, 512], F32, tag="ps", name="pf2",
                                       space="PSUM") for _ in range(4)]
                        for c2 in range(FC // 2):
                            w2c = wts.tile([P, 2, 384], FP8, tag="w2c",
                                           name="w2c", bufs=6)
                            nc.sync.dma_start(
                                w2c[:],
                                w2_d[l, c2, :, :, fh * 384:(fh + 1) * 384])
                            for gi in range(4):
                                nc.tensor.matmul(
                                    pms[gi][:, :384],
                                    lhsT=hT[:, 2 * c2:2 * c2 + 2,
                                            gi * P:(gi + 1) * P],
                                    rhs=w2c[:],
                                    start=(c2 == 0),
                                    stop=(c2 == FC // 2 - 1 and zero_bias),
                                    perf_mode=DR)
                        if not zero_bias:
                            for gi in range(4):
                                nc.tensor.matmul(
                                    pms[gi][:, :384], lhsT=ones_bf[:, :P],
                                    rhs=b2row[:, fh * 384:(fh + 1) * 384],
                                    start=False, stop=True)
                        for gi in range(4):
                            g = tiles[gi]
                            nc.vector.scalar_tensor_tensor(
                                out=xfs[gi][:, fh * 384:(fh + 1) * 384],
                                in0=pms[gi][:, :384], scalar=DQ_W,
                                in1=x_sb[:, g, fh * 384:(fh + 1) * 384],
                                op0=OP.mult, op1=OP.add,
                                accum_out=(s1as[gi][:] if fh == 0
                                           else s1bs[gi][:]))
                    for gi in range(4):
                        s1g = tmp.tile([P, 1], F32, tag="s1g", name="s1g",
                                       bufs=4)
                        nc.vector.tensor_tensor(out=s1g[:], in0=s1as[gi][:],
                                                in1=s1bs[gi][:], op=OP.add)
                        _layernorm(nc, tmp, x_sb[:, tiles[gi], :], xfs[gi],
                                   None if unit_ln else (G2, B2t), s1=s1g)

            if debug:
                nc.sync.dma_start(dd["dbg_x"][:], x_sb[:])

            # =========== classifier ===========
            clsw = cst.tile([P, 3, 2, 16], FP8, tag="clsw", name="clsw")
            nc.sync.dma_start(clsw[:], clsw_d[:])
            clsb = cst.tile([K, 1], F32, tag="clsb", name="clsb")
            nc.sync.dma_start(clsb[:], clsb_d[:])
            # tiles 0-3 already re-transposed in the last layer's FFN tail
            _transpose_tiles(nc, ps, xT, x_sb, ident, range(4, NT))
            for t2 in range(2):
                pm = ps.tile([P, 512], F32, tag="ps", name="pcls", space="PSUM")
                for c2 in range(3):
                    nc.tensor.matmul(
                        pm[:K, :], lhsT=clsw[:, c2, :, 0:K],
                        rhs=xT[:, 2 * c2:2 * c2 + 2, t2 * 512:(t2 + 1) * 512],
                        start=(c2 == 0), stop=(c2 == 2), perf_mode=DR)
                nc.scalar.activation(emT[:, t2 * 512:(t2 + 1) * 512],
                                     pm[:K, :], AF.Identity, bias=clsb[:, :1],
                                     scale=DQ_WX)
            nc.scalar.activation(expEm[:], emT[:], AF.Exp)
            if debug:
                nc.sync.dma_start(dd["dbg_em"][:], emT[:])

        # =========== CRF (weights/tmp pools closed; SBUF freed) ===========
        with contextlib.ExitStack() as cctx:
            crf = cctx.enter_context(tc.tile_pool(name="crf", bufs=1))
            ctmp = cctx.enter_context(tc.tile_pool(name="ctmp", bufs=4))

            def ct(name, shape, dtype=F32):
                return crf.tile(shape, dtype, tag=name, name=name)

            Mexp = ct("Mexp", [K, K])
            nc.sync.dma_start(Mexp[:], mexp_d[:])
            expStart = ct("expStart", [K, 1])
            expEnd = ct("expEnd", [K, 1])
            nc.sync.dma_start(expStart[:], expstart_d[:])
            nc.sync.dma_start(expEnd[:], expend_d[:])
            oh9w = ct("oh9w", [K, T])
            nc.sync.dma_start(oh9w[:], oh9w_d[:])

            # gold-emission dot product: num_dev = sum(emT * oh9w)
            sink9 = ct("sink9", [K, T])
            accK = ct("accK", [K, 1])
            nc.vector.scalar_tensor_tensor(
                out=sink9[:], in0=emT[:], scalar=1.0, in1=oh9w[:],
                op0=OP.mult, op1=OP.mult, accum_out=accK[:])

            # ---- linear-space scan ----
            expEm4 = expEm[:].rearrange("k (b s) -> k b s", b=BC)
            if mask_ones:
                # Chunked scan: alpha_255 = D_255 G_15...G_0 (M^T alpha_0)
                # with B_t = M^T D_t and G_c = B_{16c+16}...B_{16c+1}
                # (G_15 ends at B_254).  The 16 chunk factors G_c^T are built
                # simultaneously, 15 batched rounds of one matmul + one
                # row-scale over all (example, chunk) blocks:
                #   Pt <- D_t (M @ Pt),  t descending within each chunk.
                # The sequential fold is then only 16 steps deep per example.
                CH, CL = 16, 16
                # em_rep[k, t, j] = expEm[k, t]  (j-broadcast via 9 copies,
                # split ACT/DVE; both engines' copies avoid table reloads)
                em_rep = ct("em_rep", [K, T, K])
                srcEm = expEm[:].rearrange("k (t o) -> k t o", o=1)
                for j in range(K):
                    if j % 2 == 0:
                        nc.vector.tensor_copy(em_rep[:, :, j:j + 1], srcEm)
                    else:
                        nc.scalar.copy(em_rep[:, :, j:j + 1], srcEm)
                emr = em_rep[:].rearrange("k (b c s) j -> k b c s j",
                                          b=BC, c=CH)
                mexptS = ct("mexptS", [K, K])
                nc.sync.dma_start(mexptS[:], dd["mexpt"][:])
                mrepS = ct("mrepS", [K, BC, CH, K])
                nc.sync.dma_start(mrepS[:], dd["mrep"][:])
                PtS = ct("PtS", [K, BC, CH, K])
                # init chunks 0..14 at t=16(c+1); chunk 15 at t=254
                nc.vector.tensor_tensor(
                    out=PtS[:, :, 0:CH - 1, :], in0=mrepS[:, :, 0:CH - 1, :],
                    in1=emr[:, :, 1:CH, 0, :], op=OP.mult)
                nc.vector.tensor_tensor(
                    out=PtS[:, :, CH - 1, :], in0=mrepS[:, :, CH - 1, :],
                    in1=emr[:, :, CH - 1, 14, :], op=OP.mult)
                for r in range(1, CL):
                    cmax = CH - 1 if r <= 2 else CH
                    for h in range(2):
                        pu = ps.tile([P, 512], F32, tag="ps", name="pu",
                                     space="PSUM")
                        nc.tensor.matmul(
                            pu[:K, :2 * cmax * K], lhsT=mexptS[:],
                            rhs=PtS[:, 2 * h:2 * h + 2, 0:cmax, :],
                            start=True, stop=True)
                        nc.vector.tensor_tensor(
                            out=PtS[:, 2 * h:2 * h + 2, 0:cmax, :],
                            in0=pu[:K, :2 * cmax * K].rearrange(
                                "k (b c j) -> k b c j", b=2, c=cmax),
                            in1=emr[:, 2 * h:2 * h + 2, 0:cmax, CL - r, :],
                            op=OP.mult)
                # fold: beta0 = M^T @ (expStart * em_0), then 16 steps/example
                a0 = ctmp.tile([K, BC], F32, tag="a0", name="a0")
                nc.vector.tensor_scalar(
                    out=a0[:], in0=expEm4[:, :, 0],
                    scalar1=expStart[:, :1], scalar2=None, op0=OP.mult)
                endem = ctmp.tile([K, BC], F32, tag="endem", name="endem")
                nc.vector.tensor_scalar(
                    out=endem[:], in0=expEm4[:, :, S - 1],
                    scalar1=expEnd[:, :1], scalar2=None, op0=OP.mult)
                pb0 = ps.tile([P, 512], F32, tag="ps", name="pb0",
                              space="PSUM")
                nc.tensor.matmul(pb0[:K, :BC], lhsT=Mexp[:], rhs=a0[:],
                                 start=True, stop=True)
                gams = []
                for b in range(BC):
                    g0 = ctmp.tile([K, 1], F32, tag=f"g{b}", name=f"g0_{b}")
                    nc.vector.tensor_copy(g0[:], pb0[:K, b:b + 1])
                    gams.append(g0)
                F_ = ctmp.tile([K, BC], F32, tag="F", name="F_")
                for c in range(CH):
                    for b in range(BC):
                        pg = ps.tile([P, 512], F32, tag="ps", name="pg",
                                     space="PSUM")
                        nc.tensor.matmul(pg[:K, :1], lhsT=PtS[:, b, c, :],
                                         rhs=gams[b][:], start=True,
                                         stop=True)
                        if c == CH - 1:
                            nc.vector.tensor_tensor(
                                out=F_[:, b:b + 1], in0=pg[:K, :1],
                                in1=endem[:, b:b + 1], op=OP.mult)
                        else:
                            gn = ctmp.tile([K, 1], F32, tag=f"g{b}",
                                           name=f"g{c}_{b}")
                            nc.vector.tensor_copy(gn[:], pg[:K, :1])
                            gams[b] = gn
            else:
                mrow_i = ct("mrow_i", [1, T], I32)
                nc.sync.dma_start(mrow_i[:], dd["maskrow"][:])
                mrow = ct("mrow", [1, T])
                nc.vector.tensor_copy(mrow[:], mrow_i[:])
                inv9 = ct("inv9", [K, T])
                mb9 = ct("mb9", [K, T])
                for i in range(2):
                    pb = ps.tile([P, 512], F32, tag="ps", name="pmb",
                                 space="PSUM")
                    nc.tensor.matmul(pb[:K, :], lhsT=onescol_f[:1, :K],
                                     rhs=mrow[:, i * 512:(i + 1) * 512],
                                     start=True, stop=True)
                    nc.scalar.activation(mb9[:, i * 512:(i + 1) * 512],
                                         pb[:K, :], AF.Identity)
                nc.vector.tensor_scalar(out=inv9[:], in0=mb9[:], scalar1=0.0,
                                        scalar2=None, op0=OP.is_equal)
                inv4 = inv9[:].rearrange("k (b s) -> k b s", b=BC)
                # two independent 2-example chains
                Ecurs = []
                for hf in range(2):
                    Ec = ctmp.tile([K, 2], F32, tag=f"E{hf}", name=f"E0_{hf}")
                    nc.vector.tensor_scalar(
                        out=Ec[:], in0=expEm4[:, 2 * hf:2 * hf + 2, 0],
                        scalar1=expStart[:, :1], scalar2=None, op0=OP.mult)
                    Ecurs.append(Ec)
                for t in range(1, S):
                    for hf in range(2):
                        psn = ps.tile([P, 512], F32, tag="ps", name="pcrf",
                                      space="PSUM")
                        nc.tensor.matmul(psn[:K, :2], lhsT=Mexp[:],
                                         rhs=Ecurs[hf][:],
                                         start=True, stop=True)
                        Enew = ctmp.tile([K, 2], F32, tag=f"E{hf}",
                                         name=f"E{t}_{hf}")
                        nc.vector.tensor_tensor(
                            out=Enew[:], in0=psn[:K, :2],
                            in1=expEm4[:, 2 * hf:2 * hf + 2, t], op=OP.mult)
                        nc.vector.copy_predicated(
                            Enew[:], inv4[:, 2 * hf:2 * hf + 2, t],
                            Ecurs[hf][:])
                        Ecurs[hf] = Enew

                F_ = ctmp.tile([K, BC], F32, tag="F", name="F_")
                for hf in range(2):
                    nc.vector.tensor_scalar(
                        out=F_[:, 2 * hf:2 * hf + 2], in0=Ecurs[hf][:],
                        scalar1=expEnd[:, :1], scalar2=None, op0=OP.mult)
            psd = ps.tile([P, 512], F32, tag="ps", name="psd", space="PSUM")
            nc.tensor.matmul(psd[:1, :BC], lhsT=onescol_f[:K, :], rhs=F_[:],
                             start=True, stop=True)
            denomv = ctmp.tile([1, BC], F32, tag="denomv", name="denomv")
            denom_tot = ct("denom_tot", [1, 1])
            nc.scalar.activation(denomv[:], psd[:1, :BC], AF.Ln,
                                 accum_out=denom_tot[:])

            psn2 = ps.tile([P, 512], F32, tag="ps", name="psn2", space="PSUM")
            nc.tensor.matmul(psn2[:1, :1], lhsT=onescol_f[:K, :],
                             rhs=accK[:], start=True, stop=True)
            num_tot = ct("num_tot", [1, 1])
            nc.vector.tensor_copy(num_tot[:], psn2[:1, :1])
            out_sb = ct("out_sb", [1, 4])
            nc.vector.memset(out_sb[:], 0.0)
            nc.vector.tensor_tensor(out=out_sb[:, 0:1], in0=denom_tot[:],
                                    in1=num_tot[:], op=OP.subtract)
            nc.vector.tensor_copy(out_sb[:, 1:2], num_tot[:])
            nc.vector.tensor_copy(out_sb[:, 2:3], denom_tot[:])
            nc.sync.dma_start(out_d[:], out_sb[:])


# ---------------------------------------------------------------------------
# host wrapper
# ---------------------------------------------------------------------------

_NC_CACHE = {}


def _get_nc(key):
    if key not in _NC_CACHE:
        _NC_CACHE[key] = build_nc(*key)
    return _NC_CACHE[key]


def prepare_maps(inputs, mask_ones, zero_bias, unit_ln):
    """Returns (in_maps, label_const): per-core device inputs and the
    host-computed label-only CRF numerator sum over the whole batch."""
    input_ids = np.asarray(inputs["input_ids"]).astype(np.int32)
    attention_mask = np.asarray(inputs["attention_mask"]).astype(np.int32)
    labels = np.asarray(inputs["labels"]).astype(np.int64)

    word = _bf(inputs["word_emb"])
    pt = _bf((_f32(inputs["pos_emb"][:S]) + _f32(inputs["type_emb"][0])[None, :])
             .reshape(2, P, H).transpose(1, 0, 2))
    wq = _f8(inputs["Wq"], WS).reshape(L, 3, 2, P, H).transpose(
        0, 3, 1, 2, 4).copy()
    wk = _f8(inputs["Wk"], WS).reshape(L, 3, 2, P, H).transpose(
        0, 3, 1, 2, 4).copy()
    wv = _f8(inputs["Wv"], WS).reshape(L, 3, 2, P, H).transpose(
        0, 3, 1, 2, 4).copy()
    wo = _f8(inputs["Wo"], WS).reshape(L, 3, 2, P, H).transpose(
        0, 3, 1, 2, 4).copy()
    w1 = (_f8(inputs["W1"], WS).reshape(L, 3, 2, P, FC, P)
          .transpose(0, 4, 3, 1, 2, 5).copy())
    w2 = (_f8(inputs["W2"], WS).reshape(L, FC // 2, 2, P, H)
          .transpose(0, 1, 3, 2, 4).copy())
    cwpad = np.zeros((H, 16), np.float32)
    cwpad[:, :K] = _f32(inputs["cls_W"])
    clsw = _f8(cwpad, WS).reshape(3, 2, P, 16).transpose(2, 0, 1, 3).copy()
    clsb = (_f32(inputs["cls_b"]) - np.float32(C_SHIFT)).reshape(K, 1)

    trans = _f32(inputs["crf_trans"]).reshape(K, K)
    startv = _f32(inputs["crf_start"]).reshape(K)
    endv = _f32(inputs["crf_end"]).reshape(K)

    shared = dict(
        word=word, pt=pt, wq=wq, wk=wk, wv=wv, wo=wo, w1=w1, w2=w2,
        clsw=clsw, clsb=clsb,
        mexp=np.exp(trans).astype(np.float32),
        mexpt=np.ascontiguousarray(np.exp(trans).T.astype(np.float32)),
        mrep=np.ascontiguousarray(np.broadcast_to(
            np.exp(trans).astype(np.float32)[:, None, None, :],
            (K, BC, 16, K))),
        expstart=np.exp(startv).astype(np.float32).reshape(K, 1),
        expend=np.exp(endv).astype(np.float32).reshape(K, 1),
    )
    if not zero_bias:
        shared.update(
            bq=_f32(inputs["bq"]).reshape(L, HC, P).transpose(0, 2, 1).copy(),
            bk=_f32(inputs["bk"]).reshape(L, HC, P).transpose(0, 2, 1).copy(),
            b1=_f32(inputs["b1"]).reshape(L, FC, P).transpose(0, 2, 1).copy(),
            bvrow=_bf(_f32(inputs["bv"]) / DQ_WX).reshape(L, 1, H),
            borow=_bf(_f32(inputs["bo"]) / DQ_WX).reshape(L, 1, H),
            b2row=_bf(_f32(inputs["b2"]) / DQ_W).reshape(L, 1, H),
        )
    if not unit_ln:
        shared.update(
            lng=np.stack([_bf(inputs["ln1_g"]), _bf(inputs["ln2_g"])],
                         axis=1).reshape(L, 2, 1, H),
            lnb=np.stack([_bf(inputs["ln1_b"]), _bf(inputs["ln2_b"])],
                         axis=1).reshape(L, 2, 1, H),
            elng=np.stack([_bf(inputs["emb_ln_g"]), _bf(inputs["emb_ln_b"])],
                          axis=0).reshape(2, 1, H),
        )

    # ---- host label-only numerator + per-core oh9w ----
    mf = attention_mask.astype(np.float32)               # [B, S]
    w9 = mf.copy()
    w9[:, 0] = 1.0                                       # t=0 emission always counted
    trans_gold = trans[labels[:, :-1], labels[:, 1:]]    # [B, S-1]
    last_idx = attention_mask.astype(np.int64).sum(axis=1) - 1
    label_num = (startv[labels[:, 0]]
                 + (trans_gold * mf[:, 1:]).sum(axis=1)
                 + endv[labels[np.arange(B), last_idx]])  # [B]
    label_const = float(np.float32(label_num.astype(np.float32).sum()))

    in_maps = []
    for c in range(CORES):
        ids_c = input_ids[BC * c:BC * (c + 1)].reshape(NT, P, 1).copy()
        lab_c = labels[BC * c:BC * (c + 1)]              # [BC, S]
        w9_c = w9[BC * c:BC * (c + 1)]                   # [BC, S]
        oh = np.zeros((K, BC, S), np.float32)
        oh[lab_c.reshape(-1), np.repeat(np.arange(BC), S),
           np.tile(np.arange(S), BC)] = w9_c.reshape(-1)
        msk_c = attention_mask[BC * c:BC * (c + 1)]
        m = dict(shared)
        m["ids"] = ids_c
        m["oh9w"] = oh.reshape(K, T).copy()
        if not mask_ones:
            m["maskrow"] = msk_c.reshape(1, T).copy()
            m["maskcols"] = (msk_c.reshape(BC, 2, P).transpose(2, 1, 0)
                             .astype(np.int32).copy())
        in_maps.append(m)
    return in_maps, label_const


def kernel(**inputs) -> np.ndarray:
    attention_mask = np.asarray(inputs["attention_mask"])
    assert np.asarray(inputs["input_ids"]).shape == (B, S)

    mask_ones = bool(np.all(attention_mask == 1))
    zero_bias = all(
        not np.any(np.asarray(inputs[k]))
        for k in ("bq", "bk", "bv", "bo", "b1", "b2"))
    unit_ln = (all(np.all(np.asarray(inputs[k]) == 1.0)
                   for k in ("emb_ln_g", "ln1_g", "ln2_g"))
               and all(not np.any(np.asarray(inputs[k]))
                       for k in ("emb_ln_b", "ln1_b", "ln2_b")))

    n_layers = int(os.environ.get("BERTCRF_LAYERS", L))
    debug = bool(int(os.environ.get("BERTCRF_DEBUG", "0")))
    nc = _get_nc((n_layers, mask_ones, zero_bias, unit_ln, debug))
    in_maps, label_const = prepare_maps(inputs, mask_ones, zero_bias, unit_ln)

    res = run_bass_kernel_spmd(nc, in_maps, core_ids=list(range(CORES)))
    total = np.float32(0.0)
    for c in range(CORES):
        total += np.float32(res.results[c]["out"][0, 0])
    return np.float32(total - np.float32(label_const))


if __name__ == "__main__":
    import jax
    jax.config.update("jax_platforms", "cpu")
    import reference
    inp = reference.setup_inputs()
    outv = kernel(**{k: np.asarray(v) for k, v in inp.items()})
    print("kernel:", outv)


# revision 36
# speedup vs baseline: 1.1085x; 1.1085x over previous
"""BertCRF forward (BERT-base encoder + CRF NLL) on 8 Trainium2 NeuronCores.

Strategy: data-parallel over the batch (32 examples -> 4 per core), params
replicated.  Each core runs the full 12-layer encoder on its 1024 tokens with
fp8 (DoubleRow) matmuls for the dense projections, bf16 attention, fp32
layernorm, and a max-free softmax whose normalizer is computed by an extra
ones-matmul on the PE and inverted with a fast DVE reciprocal.  The CRF
numerator's label-only terms (start/transition/end) are computed on the host;
the device computes the gold-emission dot product and the exact linear-space
forward scan with a fixed per-step shift (cancels exactly between numerator
and denominator).  The host shards inputs, pre-arranges weight layouts, and
sums the 8 per-core partial NLLs plus the host-side label constant.
"""

import contextlib
import os

import numpy as np
import ml_dtypes

import concourse.bass as bass  # noqa: F401
import concourse.mybir as mybir
import concourse.tile as tile
from concourse import bacc
from concourse.bass import IndirectOffsetOnAxis
from concourse.bass_utils import run_bass_kernel_spmd
from concourse.masks import make_identity

# ---- problem constants (hardcoded per the task spec) ----
L, H, NH, DH, FF, V, K = 12, 768, 12, 64, 3072, 30522, 9
B, S = 32, 256
CORES = 8
BC = B // CORES          # 4 examples per core
T = BC * S               # 1024 tokens per core
P = 128
NT = T // P              # 8 token tiles
HC = H // P              # 6 hidden chunks
FC = FF // P             # 24 ff chunks
C_SHIFT = 2.35           # per-step CRF shift (cancels exactly in num-denom)

F32 = mybir.dt.float32
BF16 = mybir.dt.bfloat16
FP8 = mybir.dt.float8e4
I32 = mybir.dt.int32
AX = mybir.AxisListType
OP = mybir.AluOpType
AF = mybir.ActivationFunctionType
DR = mybir.MatmulPerfMode.DoubleRow

BF = ml_dtypes.bfloat16

# fp8 quantization scales (exact powers of two)
WS = 1024.0              # weight scale into fp8e4
XS = 32.0                # activation scale into fp8e4
DQ_WX = 2.0 ** -15       # dequant for w*x products
DQ_W = 2.0 ** -10        # dequant when only the weight was scaled


def _bf(x):
    return np.ascontiguousarray(np.asarray(x, dtype=np.float32)).astype(BF)


def _f8(x, scale):
    return np.ascontiguousarray(np.clip(
        np.asarray(x, dtype=np.float32) * scale, -240.0, 240.0)
    ).astype(ml_dtypes.float8_e4m3)


def _f32(x):
    return np.ascontiguousarray(np.asarray(x, dtype=np.float32))


# ---------------------------------------------------------------------------
# device program
# ---------------------------------------------------------------------------

def _layernorm(nc, tmp, out_bf, xf, gb, s1=None, s2=None, sq_act=True):
    """LN over the free dim of xf [P, H] f32 -> out_bf (bf16).

    s1/s2, if given, are [P, 1] tiles already holding sum(xf) / sum(xf^2)
    (s2 typically from per-half Square ops emitted right behind the residual
    halves, which shortens the LN chain's critical path).  sq_act picks the
    engine for the sum-of-squares pass when it is computed here: ACT (Square
    is in every table set) when ACT has headroom, DVE otherwise.
    """
    if s1 is None:
        s1 = tmp.tile([P, 1], F32, tag="s1", name="s1")
        nc.vector.tensor_reduce(out=s1[:], in_=xf[:], axis=AX.X, op=OP.add)
    if s2 is None:
        sq = tmp.tile([P, H], F32, tag="sq", name="sq")
        s2 = tmp.tile([P, 1], F32, tag="s2", name="s2")
        if sq_act:
            nc.scalar.activation(sq[:], xf[:], AF.Square, accum_out=s2[:])
        else:
            nc.vector.scalar_tensor_tensor(out=sq[:], in0=xf[:], scalar=1.0,
                                           in1=xf[:], op0=OP.mult,
                                           op1=OP.mult, accum_out=s2[:])
    m = tmp.tile([P, 1], F32, tag="m", name="m")
    nc.vector.tensor_scalar(out=m[:], in0=s1[:], scalar1=1.0 / H, scalar2=None,
                            op0=OP.mult)
    msq = tmp.tile([P, 1], F32, tag="msq", name="msq")
    nc.vector.tensor_tensor(out=msq[:], in0=m[:], in1=m[:], op=OP.mult)
    var = tmp.tile([P, 1], F32, tag="var", name="var")
    nc.vector.tensor_scalar(out=var[:], in0=s2[:], scalar1=1.0 / H,
                            scalar2=msq[:, :1], op0=OP.mult, op1=OP.subtract)
    # eps=1e-12 is below f32 resolution for var~O(1); bias=0.0 is identical
    sd = tmp.tile([P, 1], F32, tag="sd", name="sd")
    nc.scalar.activation(sd[:], var[:], AF.Sqrt, bias=0.0)
    rs = tmp.tile([P, 1], F32, tag="rs", name="rs")
    nc.vector.reciprocal_approx_fast(rs[:], sd[:])
    if gb is None:
        # out = rs*x - m*rs, one half on DVE, one half on ACT (in parallel)
        nmrs = tmp.tile([P, 1], F32, tag="nmrs", name="nmrs")
        nc.vector.tensor_scalar(out=nmrs[:], in0=m[:], scalar1=-1.0,
                                scalar2=rs[:, :1], op0=OP.mult, op1=OP.mult)
        HH = H // 2
        nc.vector.tensor_scalar(out=out_bf[:, 0:HH], in0=xf[:, 0:HH],
                                scalar1=m[:, :1], scalar2=rs[:, :1],
                                op0=OP.subtract, op1=OP.mult)
        nc.scalar.activation(out_bf[:, HH:H], xf[:, HH:H], AF.Identity,
                             bias=nmrs[:, :1], scale=rs[:, :1])
    else:
        G, Bb = gb
        t2 = tmp.tile([P, H], F32, tag="t2", name="t2")
        nc.vector.tensor_scalar(out=t2[:], in0=xf[:], scalar1=m[:, :1],
                                scalar2=rs[:, :1], op0=OP.subtract, op1=OP.mult)
        t3 = tmp.tile([P, H], F32, tag="t3", name="t3")
        nc.vector.tensor_tensor(out=t3[:], in0=t2[:], in1=G[:], op=OP.mult)
        nc.vector.tensor_tensor(out=out_bf, in0=t3[:], in1=Bb[:], op=OP.add)


def _transpose_tiles(nc, ps, xT, x_sb, ident, tiles):
    """x_sb [P, NT, H] token-major -> xT [P, HC, T] feature-major, per tile."""
    for g in tiles:
        for cg in range(2):          # chunk groups of 3
            pt_ = ps.tile([P, 1024], BF16, tag="ps", name="ptp", space="PSUM")
            for ci in range(3):
                c = cg * 3 + ci
                nc.tensor.matmul(
                    pt_[:, ci * P:(ci + 1) * P],
                    lhsT=x_sb[:, g, c * P:(c + 1) * P], rhs=ident[:],
                    start=True, stop=True, is_transpose=True)
            nc.scalar.activation(
                xT[:, cg * 3:cg * 3 + 3, g * P:(g + 1) * P],
                pt_[:, :384], AF.Identity, scale=XS)


def _bcast_row(nc, ps, tmp, dst, row_dram, ones_bf):
    """dst [P, H] bf16 = broadcast of a [1, H] bf16 dram row across partitions."""
    row = tmp.tile([1, H], BF16, tag="brow", name="brow")
    nc.sync.dma_start(row[:], row_dram)
    for fh in range(2):
        pb = ps.tile([P, 512], F32, tag="ps", name="pbc", space="PSUM")
        nc.tensor.matmul(pb[:, :384], lhsT=ones_bf[:, :P],
                         rhs=row[:, fh * 384:(fh + 1) * 384],
                         start=True, stop=True)
        nc.scalar.activation(dst[:, fh * 384:(fh + 1) * 384], pb[:, :384],
                             AF.Identity)


def build_nc(n_layers=L, mask_ones=True, zero_bias=True, unit_ln=True,
             debug=False):
    nc = bacc.Bacc("TRN2", target_bir_lowering=False, debug=False)

    dd = {}

    def din(name, shape, dtype):
        dd[name] = nc.dram_tensor(name, list(shape), dtype, kind="ExternalInput")
        return dd[name]

    def dout(name, shape, dtype):
        dd[name] = nc.dram_tensor(name, list(shape), dtype, kind="ExternalOutput")
        return dd[name]

    din("word", [V, H], BF16)
    din("ids", [NT, P, 1], I32)
    din("pt", [P, 2, H], BF16)
    din("wq", [L, P, 3, 2, H], FP8)
    din("wk", [L, P, 3, 2, H], FP8)
    din("wv", [L, P, 3, 2, H], FP8)
    din("wo", [L, P, 3, 2, H], FP8)
    din("w1", [L, FC, P, 3, 2, P], FP8)   # [l, j, ki, c2, ko, m]
    din("w2", [L, FC // 2, P, 2, H], FP8)  # [l, c2, ki, ko, n]
    din("clsw", [P, 3, 2, 16], FP8)      # K padded to 16
    din("clsb", [K, 1], F32)             # already shifted by -C_SHIFT
    din("mexp", [K, K], F32)             # exp(crf_trans)
    din("mexpt", [K, K], F32)            # exp(crf_trans).T
    din("mrep", [K, BC, 16, K], F32)     # exp(crf_trans) replicated 64x
    din("expstart", [K, 1], F32)         # exp(crf_start)
    din("expend", [K, 1], F32)           # exp(crf_end)
    din("oh9w", [K, T], F32)             # one-hot(labels) * emission weight
    if not mask_ones:
        din("maskrow", [1, T], I32)
        din("maskcols", [P, 2, BC], I32)
    if not zero_bias:
        din("bq", [L, P, HC], F32)
        din("bk", [L, P, HC], F32)
        din("b1", [L, P, FC], F32)
        din("bvrow", [L, 1, H], BF16)
        din("borow", [L, 1, H], BF16)
        din("b2row", [L, 1, H], BF16)
    if not unit_ln:
        din("lng", [L, 2, 1, H], BF16)
        din("lnb", [L, 2, 1, H], BF16)
        din("elng", [2, 1, H], BF16)

    dout("out", [1, 4], F32)
    if debug:
        dout("dbg_x0", [P, NT, H], BF16)
        dout("dbg_x", [P, NT, H], BF16)
        dout("dbg_em", [K, T], F32)

    _build_body(nc, dd, n_layers, mask_ones, zero_bias, unit_ln, debug)
    nc.compile()
    return nc


def _build_body(nc, dd, n_layers, mask_ones, zero_bias, unit_ln, debug):
    (word, ids, pt, wq_d, wk_d, wv_d, wo_d, w1_d, w2_d, clsw_d, clsb_d,
     mexp_d, expstart_d, expend_d, oh9w_d, out_d) = (
        dd["word"], dd["ids"], dd["pt"], dd["wq"], dd["wk"], dd["wv"],
        dd["wo"], dd["w1"], dd["w2"], dd["clsw"], dd["clsb"], dd["mexp"],
        dd["expstart"], dd["expend"], dd["oh9w"], dd["out"])
    if not mask_ones:
        maskrow_d = dd["maskrow"]
        maskcols_d = dd["maskcols"]
    if not zero_bias:
        bq_d, bk_d, b1_d = dd["bq"], dd["bk"], dd["b1"]
        bvrow_d, borow_d, b2row_d = dd["bvrow"], dd["borow"], dd["b2row"]
    if not unit_ln:
        lng_d, lnb_d, elng_d = dd["lng"], dd["lnb"], dd["elng"]
    with tile.TileContext(nc) as tc, contextlib.ExitStack() as octx:
        cst = octx.enter_context(tc.tile_pool(name="cst", bufs=1))
        act = octx.enter_context(tc.tile_pool(name="act", bufs=1))
        ps = octx.enter_context(tc.tile_pool(name="ps", bufs=8, space="PSUM"))

        # ---- persistent activation buffers ----
        x_sb = act.tile([P, NT, H], BF16, tag="x_sb", name="x_sb")
        xT = act.tile([P, HC, T], FP8, tag="xT", name="xT")
        qT = act.tile([P, HC, T], BF16, tag="qT", name="qT")
        kT = act.tile([P, HC, T], BF16, tag="kT", name="kT")
        vS = act.tile([P, NT, NH, DH], BF16, tag="vS", name="vS")
        cT = act.tile([P, HC, T], FP8, tag="cT", name="cT")
        hT = act.tile([P, FC, T // 2], FP8, tag="hT", name="hT")
        emT = act.tile([K, T], F32, tag="emT", name="emT")
        expEm = act.tile([K, T], F32, tag="expEm", name="expEm")

        # ---- constants ----
        ident = cst.tile([P, P], BF16, tag="ident", name="ident")
        make_identity(nc, ident[:])
        ones_bf = cst.tile([1, P], BF16, tag="ones_bf", name="ones_bf")
        nc.vector.memset(ones_bf[:], 1.0)
        # ones64: [128, 64] all-ones lhsT; sum over keys of exp(scores) into
        # one 64-partition half of the normalizer PSUM tile per head
        ones64 = cst.tile([P, DH], BF16, tag="ones64", name="ones64")
        nc.vector.memset(ones64[:], 1.0)
        onescol_f = cst.tile([P, 1], F32, tag="onescol_f", name="onescol_f")
        nc.vector.memset(onescol_f[:], 1.0)

        pt_sb = cst.tile([P, 2, H], BF16, tag="pt_sb", name="pt_sb")
        nc.sync.dma_start(pt_sb[:], pt[:])
        if not mask_ones:
            mcol = cst.tile([P, 2, BC], F32, tag="mcol", name="mcol")
        if not unit_ln:
            elnG = cst.tile([P, H], BF16, tag="elnG", name="elnG")
            elnB = cst.tile([P, H], BF16, tag="elnB", name="elnB")

        with contextlib.ExitStack() as ictx:
            wts = ictx.enter_context(tc.tile_pool(name="wts", bufs=1))
            tmp = ictx.enter_context(tc.tile_pool(name="tmp", bufs=3))

            if not unit_ln:
                _bcast_row(nc, ps, tmp, elnG, elng_d[0], ones_bf)
                _bcast_row(nc, ps, tmp, elnB, elng_d[1], ones_bf)

            # =========== embeddings ===========
            for g in range(NT):
                idx = tmp.tile([P, 1], I32, tag="idx", name="idx")
                nc.sync.dma_start(idx[:], ids[g])
                emb = tmp.tile([P, H], BF16, tag="emb", name="emb")
                nc.gpsimd.indirect_dma_start(
                    out=emb[:], out_offset=None, in_=word[:],
                    in_offset=IndirectOffsetOnAxis(ap=idx[:, :1], axis=0),
                )
                xf = tmp.tile([P, H], F32, tag="xf", name="xf")
                s1e = tmp.tile([P, 1], F32, tag="s1e", name="s1e")
                nc.vector.scalar_tensor_tensor(
                    out=xf[:], in0=emb[:], scalar=0.0,
                    in1=pt_sb[:, g % 2, :], op0=OP.add, op1=OP.add,
                    accum_out=s1e[:])
                _layernorm(nc, tmp, x_sb[:, g, :], xf,
                           None if unit_ln else (elnG, elnB), s1=s1e)
            if debug:
                nc.sync.dma_start(dd["dbg_x0"][:], x_sb[:])
            # prime xT tiles 0-3 for layer 0's QK proj t2=0
            _transpose_tiles(nc, ps, xT, x_sb, ident, range(4))

            if not mask_ones:
                mi = tmp.tile([P, 2, BC], I32, tag="mi", name="mi")
                nc.sync.dma_start(mi[:], maskcols_d[:])
                nc.vector.tensor_scalar(out=mcol[:], in0=mi[:], scalar1=1.0,
                                        scalar2=10000.0, op0=OP.subtract,
                                        op1=OP.mult)

            # =========== encoder layers ===========
            for l in range(n_layers):
                wq = wts.tile([P, 3, 2, H], FP8, tag="wq", name="wq")
                wk = wts.tile([P, 3, 2, H], FP8, tag="wk", name="wk")
                wv = wts.tile([P, 3, 2, H], FP8, tag="wv", name="wv")
                wo = wts.tile([P, 3, 2, H], FP8, tag="wo", name="wo")
                nc.sync.dma_start(wq[:], wq_d[l])
                nc.sync.dma_start(wk[:], wk_d[l])
                nc.sync.dma_start(wv[:], wv_d[l])
                nc.sync.dma_start(wo[:], wo_d[l])

                if not zero_bias:
                    bq_sb = wts.tile([P, HC], F32, tag="bq", name="bq")
                    bk_sb = wts.tile([P, HC], F32, tag="bk", name="bk")
                    b1_sb = wts.tile([P, FC], F32, tag="b1", name="b1")
                    nc.sync.dma_start(bq_sb[:], bq_d[l])
                    nc.sync.dma_start(bk_sb[:], bk_d[l])
                    nc.sync.dma_start(b1_sb[:], b1_d[l])
                    bvrow = wts.tile([1, H], BF16, tag="bvrow", name="bvrow")
                    borow = wts.tile([1, H], BF16, tag="borow", name="borow")
                    b2row = wts.tile([1, H], BF16, tag="b2row", name="b2row")
                    nc.sync.dma_start(bvrow[:], bvrow_d[l])
                    nc.sync.dma_start(borow[:], borow_d[l])
                    nc.sync.dma_start(b2row[:], b2row_d[l])
                if not unit_ln:
                    G1 = wts.tile([P, H], BF16, tag="G1", name="G1")
                    B1t = wts.tile([P, H], BF16, tag="B1t", name="B1t")
                    G2 = wts.tile([P, H], BF16, tag="G2", name="G2")
                    B2t = wts.tile([P, H], BF16, tag="B2t", name="B2t")
                    _bcast_row(nc, ps, tmp, G1, lng_d[l, 0], ones_bf)
                    _bcast_row(nc, ps, tmp, B1t, lnb_d[l, 0], ones_bf)
                    _bcast_row(nc, ps, tmp, G2, lng_d[l, 1], ones_bf)
                    _bcast_row(nc, ps, tmp, B2t, lnb_d[l, 1], ones_bf)

                # ---- qT/kT projections, token-half pipelined.  xT tiles 0-3
                #      were already transposed in the previous layer's FFN
                #      tail (or right after the embeddings for layer 0), so
                #      QK proj t2=0 can start while the previous layer's
                #      second-half LN2 chain is still draining; tiles 4-7 are
                #      transposed here in between. ----
                for t2 in range(2):
                    if t2 == 1:
                        _transpose_tiles(nc, ps, xT, x_sb, ident, range(4, 8))
                    for wmat, bname, dst in ((wq, "bq", qT), (wk, "bk", kT)):
                        for f in range(HC):
                            pm = ps.tile([P, 512], F32, tag="ps", name="pqk",
                                         space="PSUM")
                            for c2 in range(3):
                                nc.tensor.matmul(
                                    pm[:],
                                    lhsT=wmat[:, c2, :, f * P:(f + 1) * P],
                                    rhs=xT[:, 2 * c2:2 * c2 + 2,
                                           t2 * 512:(t2 + 1) * 512],
                                    start=(c2 == 0), stop=(c2 == 2),
                                    perf_mode=DR)
                            if zero_bias:
                                nc.vector.tensor_scalar(
                                    out=dst[:, f, t2 * 512:(t2 + 1) * 512],
                                    in0=pm[:], scalar1=DQ_WX, scalar2=None,
                                    op0=OP.mult)
                            else:
                                bias = (bq_sb if bname == "bq"
                                        else bk_sb)[:, f:f + 1]
                                nc.scalar.activation(
                                    dst[:, f, t2 * 512:(t2 + 1) * 512], pm[:],
                                    AF.Identity, bias=bias, scale=DQ_WX)

                # ---- V projection (token-major into vS) ----
                for g in range(NT):
                    for fh in range(2):
                        pm = ps.tile([P, 512], F32, tag="ps", name="pv",
                                     space="PSUM")
                        for c2 in range(3):
                            nc.tensor.matmul(
                                pm[:, :384],
                                lhsT=xT[:, 2 * c2:2 * c2 + 2,
                                        g * P:(g + 1) * P],
                                rhs=wv[:, c2, :, fh * 384:(fh + 1) * 384],
                                start=(c2 == 0),
                                stop=(c2 == 2 and zero_bias),
                                perf_mode=DR)
                        if not zero_bias:
                            nc.tensor.matmul(
                                pm[:, :384], lhsT=ones_bf[:, :P],
                                rhs=bvrow[:, fh * 384:(fh + 1) * 384],
                                start=False, stop=True)
                        nc.vector.tensor_scalar(
                            out=vS[:, g, 6 * fh:6 * fh + 6, :],
                            in0=pm[:, :384], scalar1=DQ_WX, scalar2=None,
                            op0=OP.mult)

                # ---- output proj + residual + LN1 (interleaved
                #      into the attention loop, per example) ----
                def _oproj_ln1(g):
                    xf = tmp.tile([P, H], F32, tag="xf", name="xf")
                    sqh = tmp.tile([P, H], F32, tag="sq", name="sqh")
                    s1a = tmp.tile([P, 1], F32, tag="s1a", name="s1a")
                    s1b = tmp.tile([P, 1], F32, tag="s1b", name="s1b")
                    s2a = tmp.tile([P, 1], F32, tag="s2a", name="s2a")
                    s2b = tmp.tile([P, 1], F32, tag="s2b", name="s2b")
                    for fh in range(2):
                        pm = ps.tile([P, 512], F32, tag="ps", name="po",
                                     space="PSUM")
                        for c2 in range(3):
                            nc.tensor.matmul(
                                pm[:, :384],
                                lhsT=cT[:, 2 * c2:2 * c2 + 2,
                                        g * P:(g + 1) * P],
                                rhs=wo[:, c2, :, fh * 384:(fh + 1) * 384],
                                start=(c2 == 0),
                                stop=(c2 == 2 and zero_bias),
                                perf_mode=DR)
                        if not zero_bias:
                            nc.tensor.matmul(
                                pm[:, :384], lhsT=ones_bf[:, :P],
                                rhs=borow[:, fh * 384:(fh + 1) * 384],
                                start=False, stop=True)
                        nc.vector.scalar_tensor_tensor(
                            out=xf[:, fh * 384:(fh + 1) * 384],
                            in0=pm[:, :384], scalar=DQ_WX,
                            in1=x_sb[:, g, fh * 384:(fh + 1) * 384],
                            op0=OP.mult, op1=OP.add,
                            accum_out=(s1a[:] if fh == 0 else s1b[:]))
                        # sum-of-squares per half on ACT, right behind the
                        # residual: halves the LN chain's serial depth
                        nc.scalar.activation(
                            sqh[:, fh * 384:(fh + 1) * 384],
                            xf[:, fh * 384:(fh + 1) * 384], AF.Square,
                            accum_out=(s2a[:] if fh == 0 else s2b[:]))
                    s1g = tmp.tile([P, 1], F32, tag="s1g", name="s1g",
                                   bufs=4)
                    nc.vector.tensor_tensor(out=s1g[:], in0=s1a[:],
                                            in1=s1b[:], op=OP.add)
                    s2g = tmp.tile([P, 1], F32, tag="s2g", name="s2g",
                                   bufs=4)
                    nc.vector.tensor_tensor(out=s2g[:], in0=s2a[:],
                                            in1=s2b[:], op=OP.add)
                    _layernorm(nc, tmp, x_sb[:, g, :], xf,
                               None if unit_ln else (G1, B1t), s1=s1g,
                               s2=s2g)

                # ---- attention.  Per head-pair: QK^T (2 heads in separate
                #      PE row-groups), fused exp on ACT, then per head both
                #      the AxV matmul and a ones-matmul normalizer sum (z)
                #      on PE.  The normalize tail (fast reciprocal of z +
                #      multiply) is deferred one pair so it overlaps the
                #      next pair's matmul/exp front.  PSUM: 4 banks/pair ->
                #      two pairs in flight. ----
                def _attn_tail(st):
                    e, ch, prbz, pcx = st
                    rb2 = tmp.tile([P, S], F32, tag="rb", name="rb")
                    nc.vector.reciprocal_approx_fast(rb2[:], prbz[:, :S])
                    for hh in range(2):
                        r0 = hh * DH
                        nc.vector.tensor_tensor(
                            out=cT[r0:r0 + DH, ch, e * S:(e + 1) * S],
                            in0=pcx[:DH, hh * S:(hh + 1) * S],
                            in1=rb2[r0:r0 + DH, :], op=OP.mult)

                # QK^T contracts only 64 partitions; heads A/B live in
                # PE row-groups {0,1}/{2,3} (lhsT base 0/64), so
                # alternating their matmuls runs them concurrently.
                # The QK matmuls of pair i+1 are EMITTED before the AV/z
                # matmuls of pair i: PE executes in order, so this gives it
                # work to do while pair i's exp runs on ACT.
                def _emit_qk(e, ch):
                    pscs = [ps.tile([P, 512], F32, tag="ps", name="psc",
                                    space="PSUM") for _ in range(2)]
                    for kt in range(2):
                        for hh in range(2):
                            r0 = hh * DH
                            nc.tensor.matmul(
                                pscs[hh][:, kt * S:(kt + 1) * S],
                                lhsT=kT[r0:r0 + DH, ch,
                                        e * S + kt * P:
                                        e * S + (kt + 1) * P],
                                rhs=qT[r0:r0 + DH, ch, e * S:(e + 1) * S],
                                start=True, stop=True)
                    return pscs

                pairs = [(e, ch) for e in range(BC) for ch in range(NH // 2)]
                prev_st = None
                pscs = _emit_qk(*pairs[0])
                for i, (e, ch) in enumerate(pairs):
                    next_pscs = (_emit_qk(*pairs[i + 1])
                                 if i + 1 < len(pairs) else None)
                    ET2 = tmp.tile([P, 2, 2, S], BF16, tag="ET", name="ET")
                    pcx = ps.tile([P, 512], F32, tag="ps", name="pcx",
                                  space="PSUM")
                    prbz = ps.tile([P, 512], F32, tag="ps", name="prb",
                                   space="PSUM")
                    for hh in range(2):
                        r0 = hh * DH
                        psc = pscs[hh]
                        if mask_ones:
                            # one fused exp over both key tiles
                            nc.scalar.activation(
                                ET2[:, hh].rearrange("p k s -> p (k s)"),
                                psc[:], AF.Exp, bias=0.0, scale=0.125)
                        else:
                            for kt in range(2):
                                nc.scalar.activation(
                                    ET2[:, hh, kt, :],
                                    psc[:, kt * S:(kt + 1) * S],
                                    AF.Exp, bias=mcol[:, kt, e:e + 1],
                                    scale=0.125)
                        h = 2 * ch + hh
                        for kt in range(2):
                            nc.tensor.matmul(
                                pcx[:DH, hh * S:(hh + 1) * S],
                                lhsT=vS[:, 2 * e + kt, h, :],
                                rhs=ET2[:, hh, kt, :],
                                start=(kt == 0), stop=(kt == 1))
                        for kt in range(2):
                            nc.tensor.matmul(
                                prbz[r0:r0 + DH, :S],
                                lhsT=ones64[:],
                                rhs=ET2[:, hh, kt, :],
                                start=(kt == 0), stop=(kt == 1))
                    if prev_st is not None:
                        _attn_tail(prev_st)
                    prev_st = (e, ch, prbz, pcx)
                    pscs = next_pscs
                _attn_tail(prev_st)
                for g in range(NT):
                    _oproj_ln1(g)

                # ---- FFN (two token-half passes) ----
                for th in range(2):
                    tiles = list(range(4 * th, 4 * th + 4))
                    _transpose_tiles(nc, ps, xT, x_sb, ident, tiles)
                    for j in range(FC):
                        if th == 1 and j == FC - 1:
                            # post-LN2 re-transpose of tiles 0-3 for the next
                            # layer (or classifier), emitted here so its
                            # PSUM->xT copies drain on ACT during the FFN2
                            # window instead of behind the LN2 chain
                            _transpose_tiles(nc, ps, xT, x_sb, ident, range(4))
                        w1j = wts.tile([P, 3, 2, P], FP8, tag="w1j",
                                       name="w1j", bufs=4)
                        nc.sync.dma_start(w1j[:], w1_d[l, j])
                        pm = ps.tile([P, 512], F32, tag="ps", name="ph",
                                     space="PSUM")
                        for c2 in range(3):
                            nc.tensor.matmul(
                                pm[:], lhsT=w1j[:, c2],
                                rhs=xT[:, 2 * c2:2 * c2 + 2,
                                       th * 512:(th + 1) * 512],
                                start=(c2 == 0), stop=(c2 == 2),
                                perf_mode=DR)
                        bias = 0.0 if zero_bias else b1_sb[:, j:j + 1]
                        nc.scalar.activation(hT[:, j, :], pm[:], AF.Gelu,
                                             bias=bias, scale=DQ_WX)
                    # FFN2: f-half outer so W2 streams once per (th, fh)
                    xfs = [tmp.tile([P, H], F32, tag="xff", name="xff", bufs=4)
                           for _ in range(4)]
                    sqhs = [tmp.tile([P, H], F32, tag="sqf", name="sqf",
                                     bufs=4) for _ in range(4)]
                    s1as = [tmp.tile([P, 1], F32, tag="s1fa", name="s1fa",
                                     bufs=4) for _ in range(4)]
                    s1bs = [tmp.tile([P, 1], F32, tag="s1fb", name="s1fb",
                                     bufs=4) for _ in range(4)]
                    s2as = [tmp.tile([P, 1], F32, tag="s2fa", name="s2fa",
                                     bufs=4) for _ in range(4)]
                    s2bs = [tmp.tile([P, 1], F32, tag="s2fb", name="s2fb",
                                     bufs=4) for _ in range(4)]
                    for fh in range(2):
                        pms = [ps.tile([P, 512], F32, tag="ps", name="pf2",
                                       space="PSUM") for _ in range(4)]
                        for c2 in range(FC // 2):
                            w2c = wts.tile([P, 2, 384], FP8, tag="w2c",
                                           name="w2c", bufs=6)
                            nc.sync.dma_start(
                                w2c[:],
                                w2_d[l, c2, :, :, fh * 384:(fh + 1) * 384])
                            for gi in range(4):
                                nc.tensor.matmul(
                                    pms[gi][:, :384],
                                    lhsT=hT[:, 2 * c2:2 * c2 + 2,
                                            gi * P:(gi + 1) * P],
                                    rhs=w2c[:],
                                    start=(c2 == 0),
                                    stop=(c2 == FC // 2 - 1 and zero_bias),
                                    perf_mode=DR)
                        if not zero_bias:
                            for gi in range(4):
                                nc.tensor.matmul(
                                    pms[gi][:, :384], lhsT=ones_bf[:, :P],
                                    rhs=b2row[:, fh * 384:(fh + 1) * 384],
                                    start=False, stop=True)
                        for gi in range(4):
                            g = tiles[gi]
                            nc.vector.scalar_tensor_tensor(
                                out=xfs[gi][:, fh * 384:(fh + 1) * 384],
                                in0=pms[gi][:, :384], scalar=DQ_W,
                                in1=x_sb[:, g, fh * 384:(fh + 1) * 384],
                                op0=OP.mult, op1=OP.add,
                                accum_out=(s1as[gi][:] if fh == 0
                                           else s1bs[gi][:]))
                            nc.scalar.activation(
                                sqhs[gi][:, fh * 384:(fh + 1) * 384],
                                xfs[gi][:, fh * 384:(fh + 1) * 384],
                                AF.Square,
                                accum_out=(s2as[gi][:] if fh == 0
                                           else s2bs[gi][:]))
                    for gi in range(4):
                        s1g = tmp.tile([P, 1], F32, tag="s1g", name="s1g",
                                       bufs=4)
                        nc.vector.tensor_tensor(out=s1g[:], in0=s1as[gi][:],
                                                in1=s1bs[gi][:], op=OP.add)
                        s2g = tmp.tile([P, 1], F32, tag="s2g", name="s2g",
                                       bufs=4)
                        nc.vector.tensor_tensor(out=s2g[:], in0=s2as[gi][:],
                                                in1=s2bs[gi][:], op=OP.add)
                        _layernorm(nc, tmp, x_sb[:, tiles[gi], :], xfs[gi],
                                   None if unit_ln else (G2, B2t), s1=s1g,
                                   s2=s2g)

            if debug:
                nc.sync.dma_start(dd["dbg_x"][:], x_sb[:])

            # =========== classifier ===========
            clsw = cst.tile([P, 3, 2, 16], FP8, tag="clsw", name="clsw")
            nc.sync.dma_start(clsw[:], clsw_d[:])
            clsb = cst.tile([K, 1], F32, tag="clsb", name="clsb")
            nc.sync.dma_start(clsb[:], clsb_d[:])
            # tiles 0-3 already re-transposed in the last layer's FFN tail
            _transpose_tiles(nc, ps, xT, x_sb, ident, range(4, NT))
            for t2 in range(2):
                pm = ps.tile([P, 512], F32, tag="ps", name="pcls", space="PSUM")
                for c2 in range(3):
                    nc.tensor.matmul(
                        pm[:K, :], lhsT=clsw[:, c2, :, 0:K],
                        rhs=xT[:, 2 * c2:2 * c2 + 2, t2 * 512:(t2 + 1) * 512],
                        start=(c2 == 0), stop=(c2 == 2), perf_mode=DR)
                nc.scalar.activation(emT[:, t2 * 512:(t2 + 1) * 512],
                                     pm[:K, :], AF.Identity, bias=clsb[:, :1],
                                     scale=DQ_WX)
            nc.scalar.activation(expEm[:], emT[:], AF.Exp)
            if debug:
                nc.sync.dma_start(dd["dbg_em"][:], emT[:])

        # =========== CRF (weights/tmp pools closed; SBUF freed) ===========
        with contextlib.ExitStack() as cctx:
            crf = cctx.enter_context(tc.tile_pool(name="crf", bufs=1))
            ctmp = cctx.enter_context(tc.tile_pool(name="ctmp", bufs=4))

            def ct(name, shape, dtype=F32):
                return crf.tile(shape, dtype, tag=name, name=name)

            Mexp = ct("Mexp", [K, K])
            nc.sync.dma_start(Mexp[:], mexp_d[:])
            expStart = ct("expStart", [K, 1])
            expEnd = ct("expEnd", [K, 1])
            nc.sync.dma_start(expStart[:], expstart_d[:])
            nc.sync.dma_start(expEnd[:], expend_d[:])
            oh9w = ct("oh9w", [K, T])
            nc.sync.dma_start(oh9w[:], oh9w_d[:])

            # gold-emission dot product: num_dev = sum(emT * oh9w)
            sink9 = ct("sink9", [K, T])
            accK = ct("accK", [K, 1])
            nc.vector.scalar_tensor_tensor(
                out=sink9[:], in0=emT[:], scalar=1.0, in1=oh9w[:],
                op0=OP.mult, op1=OP.mult, accum_out=accK[:])

            # ---- linear-space scan ----
            expEm4 = expEm[:].rearrange("k (b s) -> k b s", b=BC)
            if mask_ones:
                # Chunked scan: alpha_255 = D_255 G_15...G_0 (M^T alpha_0)
                # with B_t = M^T D_t and G_c = B_{16c+16}...B_{16c+1}
                # (G_15 ends at B_254).  The 16 chunk factors G_c^T are built
                # simultaneously, 15 batched rounds of one matmul + one
                # row-scale over all (example, chunk) blocks:
                #   Pt <- D_t (M @ Pt),  t descending within each chunk.
                # The sequential fold is then only 16 steps deep per example.
                CH, CL = 16, 16
                # em_rep[k, t, j] = expEm[k, t]  (j-broadcast via 9 copies,
                # split ACT/DVE; both engines' copies avoid table reloads)
                em_rep = ct("em_rep", [K, T, K])
                srcEm = expEm[:].rearrange("k (t o) -> k t o", o=1)
                for j in range(K):
                    if j % 2 == 0:
                        nc.vector.tensor_copy(em_rep[:, :, j:j + 1], srcEm)
                    else:
                        nc.scalar.copy(em_rep[:, :, j:j + 1], srcEm)
                emr = em_rep[:].rearrange("k (b c s) j -> k b c s j",
                                          b=BC, c=CH)
                mexptS = ct("mexptS", [K, K])
                nc.sync.dma_start(mexptS[:], dd["mexpt"][:])
                mrepS = ct("mrepS", [K, BC, CH, K])
                nc.sync.dma_start(mrepS[:], dd["mrep"][:])
                PtS = ct("PtS", [K, BC, CH, K])
                # init chunks 0..14 at t=16(c+1); chunk 15 at t=254
                nc.vector.tensor_tensor(
                    out=PtS[:, :, 0:CH - 1, :], in0=mrepS[:, :, 0:CH - 1, :],
                    in1=emr[:, :, 1:CH, 0, :], op=OP.mult)
                nc.vector.tensor_tensor(
                    out=PtS[:, :, CH - 1, :], in0=mrepS[:, :, CH - 1, :],
                    in1=emr[:, :, CH - 1, 14, :], op=OP.mult)
                for r in range(1, CL):
                    cmax = CH - 1 if r <= 2 else CH
                    for h in range(2):
                        pu = ps.tile([P, 512], F32, tag="ps", name="pu",
                                     space="PSUM")
                        nc.tensor.matmul(
                            pu[:K, :2 * cmax * K], lhsT=mexptS[:],
                            rhs=PtS[:, 2 * h:2 * h + 2, 0:cmax, :],
                            start=True, stop=True)
                        nc.vector.tensor_tensor(
                            out=PtS[:, 2 * h:2 * h + 2, 0:cmax, :],
                            in0=pu[:K, :2 * cmax * K].rearrange(
                                "k (b c j) -> k b c j", b=2, c=cmax),
                            in1=emr[:, 2 * h:2 * h + 2, 0:cmax, CL - r, :],
                            op=OP.mult)
                # fold: beta0 = M^T @ (expStart * em_0), then 16 steps/example
                a0 = ctmp.tile([K, BC], F32, tag="a0", name="a0")
                nc.vector.tensor_scalar(
                    out=a0[:], in0=expEm4[:, :, 0],
                    scalar1=expStart[:, :1], scalar2=None, op0=OP.mult)
                endem = ctmp.tile([K, BC], F32, tag="endem", name="endem")
                nc.vector.tensor_scalar(
                    out=endem[:], in0=expEm4[:, :, S - 1],
                    scalar1=expEnd[:, :1], scalar2=None, op0=OP.mult)
                pb0 = ps.tile([P, 512], F32, tag="ps", name="pb0",
                              space="PSUM")
                nc.tensor.matmul(pb0[:K, :BC], lhsT=Mexp[:], rhs=a0[:],
                                 start=True, stop=True)
                gams = []
                for b in range(BC):
                    g0 = ctmp.tile([K, 1], F32, tag=f"g{b}", name=f"g0_{b}")
                    nc.vector.tensor_copy(g0[:], pb0[:K, b:b + 1])
                    gams.append(g0)
                F_ = ctmp.tile([K, BC], F32, tag="F", name="F_")
                for c in range(CH):
                    for b in range(BC):
                        pg = ps.tile([P, 512], F32, tag="ps", name="pg",
                                     space="PSUM")
                        nc.tensor.matmul(pg[:K, :1], lhsT=PtS[:, b, c, :],
                                         rhs=gams[b][:], start=True,
                                         stop=True)
                        if c == CH - 1:
                            nc.vector.tensor_tensor(
                                out=F_[:, b:b + 1], in0=pg[:K, :1],
                                in1=endem[:, b:b + 1], op=OP.mult)
                        else:
                            gn = ctmp.tile([K, 1], F32, tag=f"g{b}",
                                           name=f"g{c}_{b}")
                            nc.vector.tensor_copy(gn[:], pg[:K, :1])
                            gams[b] = gn
            else:
                mrow_i = ct("mrow_i", [1, T], I32)
                nc.sync.dma_start(mrow_i[:], dd["maskrow"][:])
                mrow = ct("mrow", [1, T])
                nc.vector.tensor_copy(mrow[:], mrow_i[:])
                inv9 = ct("inv9", [K, T])
                mb9 = ct("mb9", [K, T])
                for i in range(2):
                    pb = ps.tile([P, 512], F32, tag="ps", name="pmb",
                                 space="PSUM")
                    nc.tensor.matmul(pb[:K, :], lhsT=onescol_f[:1, :K],
                                     rhs=mrow[:, i * 512:(i + 1) * 512],
                                     start=True, stop=True)
                    nc.scalar.activation(mb9[:, i * 512:(i + 1) * 512],
                                         pb[:K, :], AF.Identity)
                nc.vector.tensor_scalar(out=inv9[:], in0=mb9[:], scalar1=0.0,
                                        scalar2=None, op0=OP.is_equal)
                inv4 = inv9[:].rearrange("k (b s) -> k b s", b=BC)
                # two independent 2-example chains
                Ecurs = []
                for hf in range(2):
                    Ec = ctmp.tile([K, 2], F32, tag=f"E{hf}", name=f"E0_{hf}")
                    nc.vector.tensor_scalar(
                        out=Ec[:], in0=expEm4[:, 2 * hf:2 * hf + 2, 0],
                        scalar1=expStart[:, :1], scalar2=None, op0=OP.mult)
                    Ecurs.append(Ec)
                for t in range(1, S):
                    for hf in range(2):
                        psn = ps.tile([P, 512], F32, tag="ps", name="pcrf",
                                      space="PSUM")
                        nc.tensor.matmul(psn[:K, :2], lhsT=Mexp[:],
                                         rhs=Ecurs[hf][:],
                                         start=True, stop=True)
                        Enew = ctmp.tile([K, 2], F32, tag=f"E{hf}",
                                         name=f"E{t}_{hf}")
                        nc.vector.tensor_tensor(
                            out=Enew[:], in0=psn[:K, :2],
                            in1=expEm4[:, 2 * hf:2 * hf + 2, t], op=OP.mult)
                        nc.vector.copy_predicated(
                            Enew[:], inv4[:, 2 * hf:2 * hf + 2, t],
                            Ecurs[hf][:])
                        Ecurs[hf] = Enew

                F_ = ctmp.tile([K, BC], F32, tag="F", name="F_")
                for hf in range(2):
                    nc.vector.tensor_scalar(
                        out=F_[:, 2 * hf:2 * hf + 2], in0=Ecurs[hf][:],
                        scalar1=expEnd[:, :1], scalar2=None, op0=OP.mult)
            psd = ps.tile([P, 512], F32, tag="ps", name="psd", space="PSUM")
            nc.tensor.matmul(psd[:1, :BC], lhsT=onescol_f[:K, :], rhs=F_[:],
                             start=True, stop=True)
            denomv = ctmp.tile([1, BC], F32, tag="denomv", name="denomv")
            denom_tot = ct("denom_tot", [1, 1])
            nc.scalar.activation(denomv[:], psd[:1, :BC], AF.Ln,
                                 accum_out=denom_tot[:])

            psn2 = ps.tile([P, 512], F32, tag="ps", name="psn2", space="PSUM")
            nc.tensor.matmul(psn2[:1, :1], lhsT=onescol_f[:K, :],
                             rhs=accK[:], start=True, stop=True)
            num_tot = ct("num_tot", [1, 1])
            nc.vector.tensor_copy(num_tot[:], psn2[:1, :1])
            out_sb = ct("out_sb", [1, 4])
            nc.vector.memset(out_sb[:], 0.0)
            nc.vector.tensor_tensor(out=out_sb[:, 0:1], in0=denom_tot[:],
                                    in1=num_tot[:], op=OP.subtract)
            nc.vector.tensor_copy(out_sb[:, 1:2], num_tot[:])
            nc.vector.tensor_copy(out_sb[:, 2:3], denom_tot[:])
            nc.sync.dma_start(out_d[:], out_sb[:])


# ---------------------------------------------------------------------------
# host wrapper
# ---------------------------------------------------------------------------

_NC_CACHE = {}


def _get_nc(key):
    if key not in _NC_CACHE:
        _NC_CACHE[key] = build_nc(*key)
    return _NC_CACHE[key]


def prepare_maps(inputs, mask_ones, zero_bias, unit_ln):
    """Returns (in_maps, label_const): per-core device inputs and the
    host-computed label-only CRF numerator sum over the whole batch."""
    input_ids = np.asarray(inputs["input_ids"]).astype(np.int32)
    attention_mask = np.asarray(inputs["attention_mask"]).astype(np.int32)
    labels = np.asarray(inputs["labels"]).astype(np.int64)

    word = _bf(inputs["word_emb"])
    pt = _bf((_f32(inputs["pos_emb"][:S]) + _f32(inputs["type_emb"][0])[None, :])
             .reshape(2, P, H).transpose(1, 0, 2))
    wq = _f8(inputs["Wq"], WS).reshape(L, 3, 2, P, H).transpose(
        0, 3, 1, 2, 4).copy()
    wk = _f8(inputs["Wk"], WS).reshape(L, 3, 2, P, H).transpose(
        0, 3, 1, 2, 4).copy()
    wv = _f8(inputs["Wv"], WS).reshape(L, 3, 2, P, H).transpose(
        0, 3, 1, 2, 4).copy()
    wo = _f8(inputs["Wo"], WS).reshape(L, 3, 2, P, H).transpose(
        0, 3, 1, 2, 4).copy()
    w1 = (_f8(inputs["W1"], WS).reshape(L, 3, 2, P, FC, P)
          .transpose(0, 4, 3, 1, 2, 5).copy())
    w2 = (_f8(inputs["W2"], WS).reshape(L, FC // 2, 2, P, H)
          .transpose(0, 1, 3, 2, 4).copy())
    cwpad = np.zeros((H, 16), np.float32)
    cwpad[:, :K] = _f32(inputs["cls_W"])
    clsw = _f8(cwpad, WS).reshape(3, 2, P, 16).transpose(2, 0, 1, 3).copy()
    clsb = (_f32(inputs["cls_b"]) - np.float32(C_SHIFT)).reshape(K, 1)

    trans = _f32(inputs["crf_trans"]).reshape(K, K)
    startv = _f32(inputs["crf_start"]).reshape(K)
    endv = _f32(inputs["crf_end"]).reshape(K)

    shared = dict(
        word=word, pt=pt, wq=wq, wk=wk, wv=wv, wo=wo, w1=w1, w2=w2,
        clsw=clsw, clsb=clsb,
        mexp=np.exp(trans).astype(np.float32),
        mexpt=np.ascontiguousarray(np.exp(trans).T.astype(np.float32)),
        mrep=np.ascontiguousarray(np.broadcast_to(
            np.exp(trans).astype(np.float32)[:, None, None, :],
            (K, BC, 16, K))),
        expstart=np.exp(startv).astype(np.float32).reshape(K, 1),
        expend=np.exp(endv).astype(np.float32).reshape(K, 1),
    )
    if not zero_bias:
        shared.update(
            bq=_f32(inputs["bq"]).reshape(L, HC, P).transpose(0, 2, 1).copy(),
            bk=_f32(inputs["bk"]).reshape(L, HC, P).transpose(0, 2, 1).copy(),
            b1=_f32(inputs["b1"]).reshape(L, FC, P).transpose(0, 2, 1).copy(),
            bvrow=_bf(_f32(inputs["bv"]) / DQ_WX).reshape(L, 1, H),
            borow=_bf(_f32(inputs["bo"]) / DQ_WX).reshape(L, 1, H),
            b2row=_bf(_f32(inputs["b2"]) / DQ_W).reshape(L, 1, H),
        )
    if not unit_ln:
        shared.update(
            lng=np.stack([_bf(inputs["ln1_g"]), _bf(inputs["ln2_g"])],
                         axis=1).reshape(L, 2, 1, H),
            lnb=np.stack([_bf(inputs["ln1_b"]), _bf(inputs["ln2_b"])],
                         axis=1).reshape(L, 2, 1, H),
            elng=np.stack([_bf(inputs["emb_ln_g"]), _bf(inputs["emb_ln_b"])],
                          axis=0).reshape(2, 1, H),
        )

    # ---- host label-only numerator + per-core oh9w ----
    mf = attention_mask.astype(np.float32)               # [B, S]
    w9 = mf.copy()
    w9[:, 0] = 1.0                                       # t=0 emission always counted
    trans_gold = trans[labels[:, :-1], labels[:, 1:]]    # [B, S-1]
    last_idx = attention_mask.astype(np.int64).sum(axis=1) - 1
    label_num = (startv[labels[:, 0]]
                 + (trans_gold * mf[:, 1:]).sum(axis=1)
                 + endv[labels[np.arange(B), last_idx]])  # [B]
    label_const = float(np.float32(label_num.astype(np.float32).sum()))

    in_maps = []
    for c in range(CORES):
        ids_c = input_ids[BC * c:BC * (c + 1)].reshape(NT, P, 1).copy()
        lab_c = labels[BC * c:BC * (c + 1)]              # [BC, S]
        w9_c = w9[BC * c:BC * (c + 1)]                   # [BC, S]
        oh = np.zeros((K, BC, S), np.float32)
        oh[lab_c.reshape(-1), np.repeat(np.arange(BC), S),
           np.tile(np.arange(S), BC)] = w9_c.reshape(-1)
        msk_c = attention_mask[BC * c:BC * (c + 1)]
        m = dict(shared)
        m["ids"] = ids_c
        m["oh9w"] = oh.reshape(K, T).copy()
        if not mask_ones:
            m["maskrow"] = msk_c.reshape(1, T).copy()
            m["maskcols"] = (msk_c.reshape(BC, 2, P).transpose(2, 1, 0)
                             .astype(np.int32).copy())
        in_maps.append(m)
    return in_maps, label_const


def kernel(**inputs) -> np.ndarray:
    attention_mask = np.asarray(inputs["attention_mask"])
    assert np.asarray(inputs["input_ids"]).shape == (B, S)

    mask_ones = bool(np.all(attention_mask == 1))
    zero_bias = all(
        not np.any(np.asarray(inputs[k]))
        for k in ("bq", "bk", "bv", "bo", "b1", "b2"))
    unit_ln = (all(np.all(np.asarray(inputs[k]) == 1.0)
                   for k in ("emb_ln_g", "ln1_g", "ln2_g"))
               and all(not np.any(np.asarray(inputs[k]))
                       for k in ("emb_ln_b", "ln1_b", "ln2_b")))

    n_layers = int(os.environ.get("BERTCRF_LAYERS", L))
    debug = bool(int(os.environ.get("BERTCRF_DEBUG", "0")))
    nc = _get_nc((n_layers, mask_ones, zero_bias, unit_ln, debug))
    in_maps, label_const = prepare_maps(inputs, mask_ones, zero_bias, unit_ln)

    res = run_bass_kernel_spmd(nc, in_maps, core_ids=list(range(CORES)))
    total = np.float32(0.0)
    for c in range(CORES):
        total += np.float32(res.results[c]["out"][0, 0])
    return np.float32(total - np.float32(label_const))


if __name__ == "__main__":
    import jax
    jax.config.update("jax_platforms", "cpu")
    import reference
    inp = reference.setup_inputs()
    outv = kernel(**{k: np.asarray(v) for k, v in inp.items()})
    print("kernel:", outv)


# revision 37
# speedup vs baseline: 1.1413x; 1.0297x over previous
"""BertCRF forward (BERT-base encoder + CRF NLL) on 8 Trainium2 NeuronCores.

Strategy: data-parallel over the batch (32 examples -> 4 per core), params
replicated.  Each core runs the full 12-layer encoder on its 1024 tokens with
fp8 (DoubleRow) matmuls for the dense projections, bf16 attention, fp32
layernorm, and a max-free softmax whose normalizer is computed by an extra
ones-matmul on the PE and inverted with a fast DVE reciprocal.  The CRF
numerator's label-only terms (start/transition/end) are computed on the host;
the device computes the gold-emission dot product and the exact linear-space
forward scan with a fixed per-step shift (cancels exactly between numerator
and denominator).  The host shards inputs, pre-arranges weight layouts, and
sums the 8 per-core partial NLLs plus the host-side label constant.
"""

import contextlib
import os

import numpy as np
import ml_dtypes

import concourse.bass as bass  # noqa: F401
import concourse.mybir as mybir
import concourse.tile as tile
from concourse import bacc
from concourse.bass import IndirectOffsetOnAxis
from concourse.bass_utils import run_bass_kernel_spmd
from concourse.masks import make_identity

# ---- problem constants (hardcoded per the task spec) ----
L, H, NH, DH, FF, V, K = 12, 768, 12, 64, 3072, 30522, 9
B, S = 32, 256
CORES = 8
BC = B // CORES          # 4 examples per core
T = BC * S               # 1024 tokens per core
P = 128
NT = T // P              # 8 token tiles
HC = H // P              # 6 hidden chunks
FC = FF // P             # 24 ff chunks
C_SHIFT = 2.35           # per-step CRF shift (cancels exactly in num-denom)

F32 = mybir.dt.float32
BF16 = mybir.dt.bfloat16
FP8 = mybir.dt.float8e4
I32 = mybir.dt.int32
AX = mybir.AxisListType
OP = mybir.AluOpType
AF = mybir.ActivationFunctionType
DR = mybir.MatmulPerfMode.DoubleRow

BF = ml_dtypes.bfloat16

# fp8 quantization scales (exact powers of two)
WS = 1024.0              # weight scale into fp8e4
XS = 32.0                # activation scale into fp8e4
DQ_WX = 2.0 ** -15       # dequant for w*x products
DQ_W = 2.0 ** -10        # dequant when only the weight was scaled


def _bf(x):
    return np.ascontiguousarray(np.asarray(x, dtype=np.float32)).astype(BF)


def _f8(x, scale):
    return np.ascontiguousarray(np.clip(
        np.asarray(x, dtype=np.float32) * scale, -240.0, 240.0)
    ).astype(ml_dtypes.float8_e4m3)


def _f32(x):
    return np.ascontiguousarray(np.asarray(x, dtype=np.float32))


# ---------------------------------------------------------------------------
# device program
# ---------------------------------------------------------------------------

def _layernorm(nc, tmp, out_bf, xf, gb, s1=None, sq_act=True):
    """LN over the free dim of xf [P, H] f32 -> out_bf (bf16).

    s1, if given, is a [P, 1] tile already holding sum(xf) (computed for free
    via accum_out on the op that produced xf).  sq_act picks the engine for
    the sum-of-squares pass: ACT (Square is in every table set) when ACT has
    headroom, DVE when ACT is the busier engine (attention phase).
    """
    if s1 is None:
        s1 = tmp.tile([P, 1], F32, tag="s1", name="s1")
        nc.vector.tensor_reduce(out=s1[:], in_=xf[:], axis=AX.X, op=OP.add)
    sq = tmp.tile([P, H], F32, tag="sq", name="sq")
    s2 = tmp.tile([P, 1], F32, tag="s2", name="s2")
    if sq_act:
        nc.scalar.activation(sq[:], xf[:], AF.Square, accum_out=s2[:])
    else:
        nc.vector.scalar_tensor_tensor(out=sq[:], in0=xf[:], scalar=1.0,
                                       in1=xf[:], op0=OP.mult, op1=OP.mult,
                                       accum_out=s2[:])
    m = tmp.tile([P, 1], F32, tag="m", name="m")
    nc.vector.tensor_scalar(out=m[:], in0=s1[:], scalar1=1.0 / H, scalar2=None,
                            op0=OP.mult)
    msq = tmp.tile([P, 1], F32, tag="msq", name="msq")
    nc.vector.tensor_tensor(out=msq[:], in0=m[:], in1=m[:], op=OP.mult)
    var = tmp.tile([P, 1], F32, tag="var", name="var")
    nc.vector.tensor_scalar(out=var[:], in0=s2[:], scalar1=1.0 / H,
                            scalar2=msq[:, :1], op0=OP.mult, op1=OP.subtract)
    # eps=1e-12 is below f32 resolution for var~O(1); bias=0.0 is identical
    sd = tmp.tile([P, 1], F32, tag="sd", name="sd")
    nc.scalar.activation(sd[:], var[:], AF.Sqrt, bias=0.0)
    rs = tmp.tile([P, 1], F32, tag="rs", name="rs")
    nc.vector.reciprocal_approx_fast(rs[:], sd[:])
    if gb is None:
        # out = rs*x - m*rs, one half on DVE, one half on ACT (in parallel)
        nmrs = tmp.tile([P, 1], F32, tag="nmrs", name="nmrs")
        nc.vector.tensor_scalar(out=nmrs[:], in0=m[:], scalar1=-1.0,
                                scalar2=rs[:, :1], op0=OP.mult, op1=OP.mult)
        HH = H // 2
        nc.vector.tensor_scalar(out=out_bf[:, 0:HH], in0=xf[:, 0:HH],
                                scalar1=m[:, :1], scalar2=rs[:, :1],
                                op0=OP.subtract, op1=OP.mult)
        nc.scalar.activation(out_bf[:, HH:H], xf[:, HH:H], AF.Identity,
                             bias=nmrs[:, :1], scale=rs[:, :1])
    else:
        G, Bb = gb
        t2 = tmp.tile([P, H], F32, tag="t2", name="t2")
        nc.vector.tensor_scalar(out=t2[:], in0=xf[:], scalar1=m[:, :1],
                                scalar2=rs[:, :1], op0=OP.subtract, op1=OP.mult)
        t3 = tmp.tile([P, H], F32, tag="t3", name="t3")
        nc.vector.tensor_tensor(out=t3[:], in0=t2[:], in1=G[:], op=OP.mult)
        nc.vector.tensor_tensor(out=out_bf, in0=t3[:], in1=Bb[:], op=OP.add)


def _transpose_tiles(nc, ps, xT, x_sb, ident, tiles):
    """x_sb [P, NT, H] token-major -> xT [P, HC, T] feature-major, per tile."""
    for g in tiles:
        for cg in range(2):          # chunk groups of 3
            pt_ = ps.tile([P, 1024], BF16, tag="ps", name="ptp", space="PSUM")
            for ci in range(3):
                c = cg * 3 + ci
                nc.tensor.matmul(
                    pt_[:, ci * P:(ci + 1) * P],
                    lhsT=x_sb[:, g, c * P:(c + 1) * P], rhs=ident[:],
                    start=True, stop=True, is_transpose=True)
            nc.scalar.activation(
                xT[:, cg * 3:cg * 3 + 3, g * P:(g + 1) * P],
                pt_[:, :384], AF.Identity, scale=XS)


def _bcast_row(nc, ps, tmp, dst, row_dram, ones_bf):
    """dst [P, H] bf16 = broadcast of a [1, H] bf16 dram row across partitions."""
    row = tmp.tile([1, H], BF16, tag="brow", name="brow")
    nc.sync.dma_start(row[:], row_dram)
    for fh in range(2):
        pb = ps.tile([P, 512], F32, tag="ps", name="pbc", space="PSUM")
        nc.tensor.matmul(pb[:, :384], lhsT=ones_bf[:, :P],
                         rhs=row[:, fh * 384:(fh + 1) * 384],
                         start=True, stop=True)
        nc.scalar.activation(dst[:, fh * 384:(fh + 1) * 384], pb[:, :384],
                             AF.Identity)


def build_nc(n_layers=L, mask_ones=True, zero_bias=True, unit_ln=True,
             debug=False):
    nc = bacc.Bacc("TRN2", target_bir_lowering=False, debug=False)

    dd = {}

    def din(name, shape, dtype):
        dd[name] = nc.dram_tensor(name, list(shape), dtype, kind="ExternalInput")
        return dd[name]

    def dout(name, shape, dtype):
        dd[name] = nc.dram_tensor(name, list(shape), dtype, kind="ExternalOutput")
        return dd[name]

    din("word", [V, H], BF16)
    din("ids", [NT, P, 1], I32)
    din("pt", [P, 2, H], BF16)
    din("wq", [L, P, 3, 2, H], FP8)
    din("wk", [L, P, 3, 2, H], FP8)
    din("wv", [L, P, 3, 2, H], FP8)
    din("wo", [L, P, 3, 2, H], FP8)
    din("w1", [L, FC, P, 3, 2, P], FP8)   # [l, j, ki, c2, ko, m]
    din("w2", [L, FC // 2, P, 2, H], FP8)  # [l, c2, ki, ko, n]
    din("clsw", [P, 3, 2, 16], FP8)      # K padded to 16
    din("clsb", [K, 1], F32)             # already shifted by -C_SHIFT
    din("mexp", [K, K], F32)             # exp(crf_trans)
    din("mexpt", [K, K], F32)            # exp(crf_trans).T
    din("mrep", [K, BC, 16, K], F32)     # exp(crf_trans) replicated 64x
    din("expstart", [K, 1], F32)         # exp(crf_start)
    din("expend", [K, 1], F32)           # exp(crf_end)
    din("oh9w", [K, T], F32)             # one-hot(labels) * emission weight
    if not mask_ones:
        din("maskrow", [1, T], I32)
        din("maskcols", [P, 2, BC], I32)
    if not zero_bias:
        din("bq", [L, P, HC], F32)
        din("bk", [L, P, HC], F32)
        din("b1", [L, P, FC], F32)
        din("bvrow", [L, 1, H], BF16)
        din("borow", [L, 1, H], BF16)
        din("b2row", [L, 1, H], BF16)
    if not unit_ln:
        din("lng", [L, 2, 1, H], BF16)
        din("lnb", [L, 2, 1, H], BF16)
        din("elng", [2, 1, H], BF16)

    dout("out", [1, 4], F32)
    if debug:
        dout("dbg_x0", [P, NT, H], BF16)
        dout("dbg_x", [P, NT, H], BF16)
        dout("dbg_em", [K, T], F32)

    _build_body(nc, dd, n_layers, mask_ones, zero_bias, unit_ln, debug)
    nc.compile()
    return nc


def _build_body(nc, dd, n_layers, mask_ones, zero_bias, unit_ln, debug):
    (word, ids, pt, wq_d, wk_d, wv_d, wo_d, w1_d, w2_d, clsw_d, clsb_d,
     mexp_d, expstart_d, expend_d, oh9w_d, out_d) = (
        dd["word"], dd["ids"], dd["pt"], dd["wq"], dd["wk"], dd["wv"],
        dd["wo"], dd["w1"], dd["w2"], dd["clsw"], dd["clsb"], dd["mexp"],
        dd["expstart"], dd["expend"], dd["oh9w"], dd["out"])
    if not mask_ones:
        maskrow_d = dd["maskrow"]
        maskcols_d = dd["maskcols"]
    if not zero_bias:
        bq_d, bk_d, b1_d = dd["bq"], dd["bk"], dd["b1"]
        bvrow_d, borow_d, b2row_d = dd["bvrow"], dd["borow"], dd["b2row"]
    if not unit_ln:
        lng_d, lnb_d, elng_d = dd["lng"], dd["lnb"], dd["elng"]
    with tile.TileContext(nc) as tc, contextlib.ExitStack() as octx:
        cst = octx.enter_context(tc.tile_pool(name="cst", bufs=1))
        act = octx.enter_context(tc.tile_pool(name="act", bufs=1))
        ps = octx.enter_context(tc.tile_pool(name="ps", bufs=8, space="PSUM"))

        # ---- persistent activation buffers ----
        x_sb = act.tile([P, NT, H], BF16, tag="x_sb", name="x_sb")
        xT = act.tile([P, HC, T], FP8, tag="xT", name="xT")
        qT = act.tile([P, HC, T], BF16, tag="qT", name="qT")
        kT = act.tile([P, HC, T], BF16, tag="kT", name="kT")
        vS = act.tile([P, NT, NH, DH], BF16, tag="vS", name="vS")
        cT = act.tile([P, HC, T], FP8, tag="cT", name="cT")
        hT = act.tile([P, FC, T // 2], FP8, tag="hT", name="hT")
        emT = act.tile([K, T], F32, tag="emT", name="emT")
        expEm = act.tile([K, T], F32, tag="expEm", name="expEm")

        # ---- constants ----
        ident = cst.tile([P, P], BF16, tag="ident", name="ident")
        make_identity(nc, ident[:])
        ones_bf = cst.tile([1, P], BF16, tag="ones_bf", name="ones_bf")
        nc.vector.memset(ones_bf[:], 1.0)
        # ones64: [128, 64] all-ones lhsT; sum over keys of exp(scores) into
        # one 64-partition half of the normalizer PSUM tile per head
        ones64 = cst.tile([P, DH], BF16, tag="ones64", name="ones64")
        nc.vector.memset(ones64[:], 1.0)
        onescol_f = cst.tile([P, 1], F32, tag="onescol_f", name="onescol_f")
        nc.vector.memset(onescol_f[:], 1.0)

        pt_sb = cst.tile([P, 2, H], BF16, tag="pt_sb", name="pt_sb")
        nc.sync.dma_start(pt_sb[:], pt[:])
        if not mask_ones:
            mcol = cst.tile([P, 2, BC], F32, tag="mcol", name="mcol")
        if not unit_ln:
            elnG = cst.tile([P, H], BF16, tag="elnG", name="elnG")
            elnB = cst.tile([P, H], BF16, tag="elnB", name="elnB")

        with contextlib.ExitStack() as ictx:
            wts = ictx.enter_context(tc.tile_pool(name="wts", bufs=1))
            tmp = ictx.enter_context(tc.tile_pool(name="tmp", bufs=3))

            if not unit_ln:
                _bcast_row(nc, ps, tmp, elnG, elng_d[0], ones_bf)
                _bcast_row(nc, ps, tmp, elnB, elng_d[1], ones_bf)

            # =========== embeddings ===========
            for g in range(NT):
                idx = tmp.tile([P, 1], I32, tag="idx", name="idx")
                nc.sync.dma_start(idx[:], ids[g])
                emb = tmp.tile([P, H], BF16, tag="emb", name="emb")
                nc.gpsimd.indirect_dma_start(
                    out=emb[:], out_offset=None, in_=word[:],
                    in_offset=IndirectOffsetOnAxis(ap=idx[:, :1], axis=0),
                )
                xf = tmp.tile([P, H], F32, tag="xf", name="xf")
                s1e = tmp.tile([P, 1], F32, tag="s1e", name="s1e")
                nc.vector.scalar_tensor_tensor(
                    out=xf[:], in0=emb[:], scalar=0.0,
                    in1=pt_sb[:, g % 2, :], op0=OP.add, op1=OP.add,
                    accum_out=s1e[:])
                _layernorm(nc, tmp, x_sb[:, g, :], xf,
                           None if unit_ln else (elnG, elnB), s1=s1e)
            if debug:
                nc.sync.dma_start(dd["dbg_x0"][:], x_sb[:])
            # prime xT tiles 0-3 for layer 0's QK proj t2=0
            _transpose_tiles(nc, ps, xT, x_sb, ident, range(4))

            if not mask_ones:
                mi = tmp.tile([P, 2, BC], I32, tag="mi", name="mi")
                nc.sync.dma_start(mi[:], maskcols_d[:])
                nc.vector.tensor_scalar(out=mcol[:], in0=mi[:], scalar1=1.0,
                                        scalar2=10000.0, op0=OP.subtract,
                                        op1=OP.mult)

            # =========== encoder layers ===========
            for l in range(n_layers):
                wq = wts.tile([P, 3, 2, H], FP8, tag="wq", name="wq")
                wk = wts.tile([P, 3, 2, H], FP8, tag="wk", name="wk")
                wv = wts.tile([P, 3, 2, H], FP8, tag="wv", name="wv")
                wo = wts.tile([P, 3, 2, H], FP8, tag="wo", name="wo")
                nc.sync.dma_start(wq[:], wq_d[l])
                nc.sync.dma_start(wk[:], wk_d[l])
                nc.sync.dma_start(wv[:], wv_d[l])
                nc.sync.dma_start(wo[:], wo_d[l])

                if not zero_bias:
                    bq_sb = wts.tile([P, HC], F32, tag="bq", name="bq")
                    bk_sb = wts.tile([P, HC], F32, tag="bk", name="bk")
                    b1_sb = wts.tile([P, FC], F32, tag="b1", name="b1")
                    nc.sync.dma_start(bq_sb[:], bq_d[l])
                    nc.sync.dma_start(bk_sb[:], bk_d[l])
                    nc.sync.dma_start(b1_sb[:], b1_d[l])
                    bvrow = wts.tile([1, H], BF16, tag="bvrow", name="bvrow")
                    borow = wts.tile([1, H], BF16, tag="borow", name="borow")
                    b2row = wts.tile([1, H], BF16, tag="b2row", name="b2row")
                    nc.sync.dma_start(bvrow[:], bvrow_d[l])
                    nc.sync.dma_start(borow[:], borow_d[l])
                    nc.sync.dma_start(b2row[:], b2row_d[l])
                if not unit_ln:
                    G1 = wts.tile([P, H], BF16, tag="G1", name="G1")
                    B1t = wts.tile([P, H], BF16, tag="B1t", name="B1t")
                    G2 = wts.tile([P, H], BF16, tag="G2", name="G2")
                    B2t = wts.tile([P, H], BF16, tag="B2t", name="B2t")
                    _bcast_row(nc, ps, tmp, G1, lng_d[l, 0], ones_bf)
                    _bcast_row(nc, ps, tmp, B1t, lnb_d[l, 0], ones_bf)
                    _bcast_row(nc, ps, tmp, G2, lng_d[l, 1], ones_bf)
                    _bcast_row(nc, ps, tmp, B2t, lnb_d[l, 1], ones_bf)

                # ---- qT/kT projections, token-half pipelined.  xT tiles 0-3
                #      were already transposed in the previous layer's FFN
                #      tail (or right after the embeddings for layer 0), so
                #      QK proj t2=0 can start while the previous layer's
                #      second-half LN2 chain is still draining; tiles 4-7 are
                #      transposed here in between. ----
                for t2 in range(2):
                    if t2 == 1:
                        _transpose_tiles(nc, ps, xT, x_sb, ident, range(4, 8))
                    for wmat, bname, dst in ((wq, "bq", qT), (wk, "bk", kT)):
                        for f in range(HC):
                            pm = ps.tile([P, 512], F32, tag="ps", name="pqk",
                                         space="PSUM")
                            for c2 in range(3):
                                nc.tensor.matmul(
                                    pm[:],
                                    lhsT=wmat[:, c2, :, f * P:(f + 1) * P],
                                    rhs=xT[:, 2 * c2:2 * c2 + 2,
                                           t2 * 512:(t2 + 1) * 512],
                                    start=(c2 == 0), stop=(c2 == 2),
                                    perf_mode=DR)
                            if zero_bias:
                                nc.vector.tensor_scalar(
                                    out=dst[:, f, t2 * 512:(t2 + 1) * 512],
                                    in0=pm[:], scalar1=DQ_WX, scalar2=None,
                                    op0=OP.mult)
                            else:
                                bias = (bq_sb if bname == "bq"
                                        else bk_sb)[:, f:f + 1]
                                nc.scalar.activation(
                                    dst[:, f, t2 * 512:(t2 + 1) * 512], pm[:],
                                    AF.Identity, bias=bias, scale=DQ_WX)

                # ---- V projection (token-major into vS) ----
                for g in range(NT):
                    for fh in range(2):
                        pm = ps.tile([P, 512], F32, tag="ps", name="pv",
                                     space="PSUM")
                        for c2 in range(3):
                            nc.tensor.matmul(
                                pm[:, :384],
                                lhsT=xT[:, 2 * c2:2 * c2 + 2,
                                        g * P:(g + 1) * P],
                                rhs=wv[:, c2, :, fh * 384:(fh + 1) * 384],
                                start=(c2 == 0),
                                stop=(c2 == 2 and zero_bias),
                                perf_mode=DR)
                        if not zero_bias:
                            nc.tensor.matmul(
                                pm[:, :384], lhsT=ones_bf[:, :P],
                                rhs=bvrow[:, fh * 384:(fh + 1) * 384],
                                start=False, stop=True)
                        nc.vector.tensor_scalar(
                            out=vS[:, g, 6 * fh:6 * fh + 6, :],
                            in0=pm[:, :384], scalar1=DQ_WX, scalar2=None,
                            op0=OP.mult)

                # ---- output proj + residual + LN1 (interleaved
                #      into the attention loop, per example) ----
                def _oproj_ln1(g):
                    xf = tmp.tile([P, H], F32, tag="xf", name="xf")
                    s1a = tmp.tile([P, 1], F32, tag="s1a", name="s1a")
                    s1b = tmp.tile([P, 1], F32, tag="s1b", name="s1b")
                    for fh in range(2):
                        pm = ps.tile([P, 512], F32, tag="ps", name="po",
                                     space="PSUM")
                        for c2 in range(3):
                            nc.tensor.matmul(
                                pm[:, :384],
                                lhsT=cT[:, 2 * c2:2 * c2 + 2,
                                        g * P:(g + 1) * P],
                                rhs=wo[:, c2, :, fh * 384:(fh + 1) * 384],
                                start=(c2 == 0),
                                stop=(c2 == 2 and zero_bias),
                                perf_mode=DR)
                        if not zero_bias:
                            nc.tensor.matmul(
                                pm[:, :384], lhsT=ones_bf[:, :P],
                                rhs=borow[:, fh * 384:(fh + 1) * 384],
                                start=False, stop=True)
                        nc.vector.scalar_tensor_tensor(
                            out=xf[:, fh * 384:(fh + 1) * 384],
                            in0=pm[:, :384], scalar=DQ_WX,
                            in1=x_sb[:, g, fh * 384:(fh + 1) * 384],
                            op0=OP.mult, op1=OP.add,
                            accum_out=(s1a[:] if fh == 0 else s1b[:]))
                    s1g = tmp.tile([P, 1], F32, tag="s1g", name="s1g",
                                   bufs=4)
                    nc.vector.tensor_tensor(out=s1g[:], in0=s1a[:],
                                            in1=s1b[:], op=OP.add)
                    _layernorm(nc, tmp, x_sb[:, g, :], xf,
                               None if unit_ln else (G1, B1t), s1=s1g)

                # ---- attention.  Per head-pair: QK^T (2 heads in separate
                #      PE row-groups), fused exp on ACT, then per head both
                #      the AxV matmul and a ones-matmul normalizer sum (z)
                #      on PE.  The normalize tail (fast reciprocal of z +
                #      multiply) is deferred one pair so it overlaps the
                #      next pair's matmul/exp front.  PSUM: 4 banks/pair ->
                #      two pairs in flight. ----
                def _attn_tail(st):
                    e, ch, prbz, pcx = st
                    rb2 = tmp.tile([P, S], F32, tag="rb", name="rb")
                    nc.vector.reciprocal_approx_fast(rb2[:], prbz[:, :S])
                    for hh in range(2):
                        r0 = hh * DH
                        nc.vector.tensor_tensor(
                            out=cT[r0:r0 + DH, ch, e * S:(e + 1) * S],
                            in0=pcx[:DH, hh * S:(hh + 1) * S],
                            in1=rb2[r0:r0 + DH, :], op=OP.mult)

                # QK^T contracts only 64 partitions; heads A/B live in
                # PE row-groups {0,1}/{2,3} (lhsT base 0/64), so
                # alternating their matmuls runs them concurrently.
                # The QK matmuls of pair i+1 are EMITTED before the AV/z
                # matmuls of pair i: PE executes in order, so this gives it
                # work to do while pair i's exp runs on ACT.
                def _emit_qk(e, ch):
                    pscs = [ps.tile([P, 512], F32, tag="ps", name="psc",
                                    space="PSUM") for _ in range(2)]
                    for kt in range(2):
                        for hh in range(2):
                            r0 = hh * DH
                            nc.tensor.matmul(
                                pscs[hh][:, kt * S:(kt + 1) * S],
                                lhsT=kT[r0:r0 + DH, ch,
                                        e * S + kt * P:
                                        e * S + (kt + 1) * P],
                                rhs=qT[r0:r0 + DH, ch, e * S:(e + 1) * S],
                                start=True, stop=True)
                    return pscs

                pairs = [(e, ch) for e in range(BC) for ch in range(NH // 2)]
                prev_st = None
                pscs = _emit_qk(*pairs[0])
                for i, (e, ch) in enumerate(pairs):
                    next_pscs = (_emit_qk(*pairs[i + 1])
                                 if i + 1 < len(pairs) else None)
                    ET2 = tmp.tile([P, 2, 2, S], BF16, tag="ET", name="ET")
                    pcx = ps.tile([P, 512], F32, tag="ps", name="pcx",
                                  space="PSUM")
                    prbz = ps.tile([P, 512], F32, tag="ps", name="prb",
                                   space="PSUM")
                    for hh in range(2):
                        r0 = hh * DH
                        psc = pscs[hh]
                        if mask_ones:
                            # one fused exp over both key tiles
                            nc.scalar.activation(
                                ET2[:, hh].rearrange("p k s -> p (k s)"),
                                psc[:], AF.Exp, bias=0.0, scale=0.125)
                        else:
                            for kt in range(2):
                                nc.scalar.activation(
                                    ET2[:, hh, kt, :],
                                    psc[:, kt * S:(kt + 1) * S],
                                    AF.Exp, bias=mcol[:, kt, e:e + 1],
                                    scale=0.125)
                        h = 2 * ch + hh
                        for kt in range(2):
                            nc.tensor.matmul(
                                pcx[:DH, hh * S:(hh + 1) * S],
                                lhsT=vS[:, 2 * e + kt, h, :],
                                rhs=ET2[:, hh, kt, :],
                                start=(kt == 0), stop=(kt == 1))
                        for kt in range(2):
                            nc.tensor.matmul(
                                prbz[r0:r0 + DH, :S],
                                lhsT=ones64[:],
                                rhs=ET2[:, hh, kt, :],
                                start=(kt == 0), stop=(kt == 1))
                    if prev_st is not None:
                        _attn_tail(prev_st)
                    prev_st = (e, ch, prbz, pcx)
                    pscs = next_pscs
                _attn_tail(prev_st)
                for g in range(NT):
                    _oproj_ln1(g)

                # ---- FFN (two token-half passes) ----
                for th in range(2):
                    tiles = list(range(4 * th, 4 * th + 4))
                    _transpose_tiles(nc, ps, xT, x_sb, ident, tiles)
                    for j in range(FC):
                        if th == 1 and j == FC - 1:
                            # post-LN2 re-transpose of tiles 0-3 for the next
                            # layer (or classifier), emitted here so its
                            # PSUM->xT copies drain on ACT during the FFN2
                            # window instead of behind the LN2 chain
                            _transpose_tiles(nc, ps, xT, x_sb, ident, range(4))
                        w1j = wts.tile([P, 3, 2, P], FP8, tag="w1j",
                                       name="w1j", bufs=4)
                        nc.sync.dma_start(w1j[:], w1_d[l, j])
                        pm = ps.tile([P, 512], F32, tag="ps", name="ph",
                                     space="PSUM")
                        for c2 in range(3):
                            nc.tensor.matmul(
                                pm[:], lhsT=w1j[:, c2],
                                rhs=xT[:, 2 * c2:2 * c2 + 2,
                                       th * 512:(th + 1) * 512],
                                start=(c2 == 0), stop=(c2 == 2),
                                perf_mode=DR)
                        bias = 0.0 if zero_bias else b1_sb[:, j:j + 1]
                        nc.scalar.activation(hT[:, j, :], pm[:], AF.Gelu,
                                             bias=bias, scale=DQ_WX)
                    # FFN2: f-half outer so W2 streams once per (th, fh)
                    xfs = [tmp.tile([P, H], F32, tag="xff", name="xff", bufs=4)
                           for _ in range(4)]
                    s1as = [tmp.tile([P, 1], F32, tag="s1fa", name="s1fa",
                                     bufs=4) for _ in range(4)]
                    s1bs = [tmp.tile([P, 1], F32, tag="s1fb", name="s1fb",
                                     bufs=4) for _ in range(4)]
                    for fh in range(2):
                        pms = [ps.tile([P, 512], F32, tag="ps", name="pf2",
                                       space="PSUM") for _ in range(4)]
                        for c2 in range(FC // 2):
                            w2c = wts.tile([P, 2, 384], FP8, tag="w2c",
                                           name="w2c", bufs=6)
                            nc.sync.dma_start(
                                w2c[:],
                                w2_d[l, c2, :, :, fh * 384:(fh + 1) * 384])
                            for gi in range(4):
                                nc.tensor.matmul(
                                    pms[gi][:, :384],
                                    lhsT=hT[:, 2 * c2:2 * c2 + 2,
                                            gi * P:(gi + 1) * P],
                                    rhs=w2c[:],
                                    start=(c2 == 0),
                                    stop=(c2 == FC // 2 - 1 and zero_bias),
                                    perf_mode=DR)
                        if not zero_bias:
                            for gi in range(4):
                                nc.tensor.matmul(
                                    pms[gi][:, :384], lhsT=ones_bf[:, :P],
                                    rhs=b2row[:, fh * 384:(fh + 1) * 384],
                                    start=False, stop=True)
                        for gi in range(4):
                            g = tiles[gi]
                            nc.vector.scalar_tensor_tensor(
                                out=xfs[gi][:, fh * 384:(fh + 1) * 384],
                                in0=pms[gi][:, :384], scalar=DQ_W,
                                in1=x_sb[:, g, fh * 384:(fh + 1) * 384],
                                op0=OP.mult, op1=OP.add,
                                accum_out=(s1as[gi][:] if fh == 0
                                           else s1bs[gi][:]))
                    for gi in range(4):
                        s1g = tmp.tile([P, 1], F32, tag="s1g", name="s1g",
                                       bufs=4)
                        nc.vector.tensor_tensor(out=s1g[:], in0=s1as[gi][:],
                                                in1=s1bs[gi][:], op=OP.add)
                        _layernorm(nc, tmp, x_sb[:, tiles[gi], :], xfs[gi],
                                   None if unit_ln else (G2, B2t), s1=s1g)

            if debug:
                nc.sync.dma_start(dd["dbg_x"][:], x_sb[:])

            # =========== classifier ===========
            clsw = cst.tile([P, 3, 2, 16], FP8, tag="clsw", name="clsw")
            nc.sync.dma_start(clsw[:], clsw_d[:])
            clsb = cst.tile([K, 1], F32, tag="clsb", name="clsb")
            nc.sync.dma_start(clsb[:], clsb_d[:])
            # tiles 0-3 already re-transposed in the last layer's FFN tail
            _transpose_tiles(nc, ps, xT, x_sb, ident, range(4, NT))
            for t2 in range(2):
                pm = ps.tile([P, 512], F32, tag="ps", name="pcls", space="PSUM")
                for c2 in range(3):
                    nc.tensor.matmul(
                        pm[:K, :], lhsT=clsw[:, c2, :, 0:K],
                        rhs=xT[:, 2 * c2:2 * c2 + 2, t2 * 512:(t2 + 1) * 512],
                        start=(c2 == 0), stop=(c2 == 2), perf_mode=DR)
                nc.scalar.activation(emT[:, t2 * 512:(t2 + 1) * 512],
                                     pm[:K, :], AF.Identity, bias=clsb[:, :1],
                                     scale=DQ_WX)
            nc.scalar.activation(expEm[:], emT[:], AF.Exp)
            if debug:
                nc.sync.dma_start(dd["dbg_em"][:], emT[:])

        # =========== CRF (weights/tmp pools closed; SBUF freed) ===========
        with contextlib.ExitStack() as cctx:
            crf = cctx.enter_context(tc.tile_pool(name="crf", bufs=1))
            ctmp = cctx.enter_context(tc.tile_pool(name="ctmp", bufs=4))

            def ct(name, shape, dtype=F32):
                return crf.tile(shape, dtype, tag=name, name=name)

            Mexp = ct("Mexp", [K, K])
            nc.sync.dma_start(Mexp[:], mexp_d[:])
            expStart = ct("expStart", [K, 1])
            expEnd = ct("expEnd", [K, 1])
            nc.sync.dma_start(expStart[:], expstart_d[:])
            nc.sync.dma_start(expEnd[:], expend_d[:])
            oh9w = ct("oh9w", [K, T])
            nc.sync.dma_start(oh9w[:], oh9w_d[:])

            # gold-emission dot product: num_dev = sum(emT * oh9w)
            sink9 = ct("sink9", [K, T])
            accK = ct("accK", [K, 1])
            nc.vector.scalar_tensor_tensor(
                out=sink9[:], in0=emT[:], scalar=1.0, in1=oh9w[:],
                op0=OP.mult, op1=OP.mult, accum_out=accK[:])

            # ---- linear-space scan ----
            expEm4 = expEm[:].rearrange("k (b s) -> k b s", b=BC)
            if mask_ones:
                # Chunked scan: alpha_255 = D_255 G_15...G_0 (M^T alpha_0)
                # with B_t = M^T D_t and G_c = B_{16c+16}...B_{16c+1}
                # (G_15 ends at B_254).  The 16 chunk factors G_c^T are built
                # simultaneously, 15 batched rounds of one matmul + one
                # row-scale over all (example, chunk) blocks:
                #   Pt <- D_t (M @ Pt),  t descending within each chunk.
                # The sequential fold is then only 16 steps deep per example.
                CH, CL = 16, 16
                # em_rep[k, t, j] = expEm[k, t]  (j-broadcast via 9 copies,
                # split ACT/DVE; both engines' copies avoid table reloads)
                em_rep = ct("em_rep", [K, T, K])
                srcEm = expEm[:].rearrange("k (t o) -> k t o", o=1)
                for j in range(K):
                    if j % 2 == 0:
                        nc.vector.tensor_copy(em_rep[:, :, j:j + 1], srcEm)
                    else:
                        nc.scalar.copy(em_rep[:, :, j:j + 1], srcEm)
                emr = em_rep[:].rearrange("k (b c s) j -> k b c s j",
                                          b=BC, c=CH)
                mexptS = ct("mexptS", [K, K])
                nc.sync.dma_start(mexptS[:], dd["mexpt"][:])
                mrepS = ct("mrepS", [K, BC, CH, K])
                nc.sync.dma_start(mrepS[:], dd["mrep"][:])
                PtS = ct("PtS", [K, BC, CH, K])
                # init chunks 0..14 at t=16(c+1); chunk 15 at t=254
                nc.vector.tensor_tensor(
                    out=PtS[:, :, 0:CH - 1, :], in0=mrepS[:, :, 0:CH - 1, :],
                    in1=emr[:, :, 1:CH, 0, :], op=OP.mult)
                nc.vector.tensor_tensor(
                    out=PtS[:, :, CH - 1, :], in0=mrepS[:, :, CH - 1, :],
                    in1=emr[:, :, CH - 1, 14, :], op=OP.mult)
                for r in range(1, CL):
                    cmax = CH - 1 if r <= 2 else CH
                    for h in range(2):
                        pu = ps.tile([P, 512], F32, tag="ps", name="pu",
                                     space="PSUM")
                        nc.tensor.matmul(
                            pu[:K, :2 * cmax * K], lhsT=mexptS[:],
                            rhs=PtS[:, 2 * h:2 * h + 2, 0:cmax, :],
                            start=True, stop=True)
                        nc.vector.tensor_tensor(
                            out=PtS[:, 2 * h:2 * h + 2, 0:cmax, :],
                            in0=pu[:K, :2 * cmax * K].rearrange(
                                "k (b c j) -> k b c j", b=2, c=cmax),
                            in1=emr[:, 2 * h:2 * h + 2, 0:cmax, CL - r, :],
                            op=OP.mult)
                # fold: beta0 = M^T @ (expStart * em_0), then 16 steps/example
                a0 = ctmp.tile([K, BC], F32, tag="a0", name="a0")
                nc.vector.tensor_scalar(
                    out=a0[:], in0=expEm4[:, :, 0],
                    scalar1=expStart[:, :1], scalar2=None, op0=OP.mult)
                endem = ctmp.tile([K, BC], F32, tag="endem", name="endem")
                nc.vector.tensor_scalar(
                    out=endem[:], in0=expEm4[:, :, S - 1],
                    scalar1=expEnd[:, :1], scalar2=None, op0=OP.mult)
                pb0 = ps.tile([P, 512], F32, tag="ps", name="pb0",
                              space="PSUM")
                nc.tensor.matmul(pb0[:K, :BC], lhsT=Mexp[:], rhs=a0[:],
                                 start=True, stop=True)
                gams = []
                for b in range(BC):
                    g0 = ctmp.tile([K, 1], F32, tag=f"g{b}", name=f"g0_{b}")
                    nc.vector.tensor_copy(g0[:], pb0[:K, b:b + 1])
                    gams.append(g0)
                F_ = ctmp.tile([K, BC], F32, tag="F", name="F_")
                for c in range(CH):
                    for b in range(BC):
                        pg = ps.tile([P, 512], F32, tag="ps", name="pg",
                                     space="PSUM")
                        nc.tensor.matmul(pg[:K, :1], lhsT=PtS[:, b, c, :],
                                         rhs=gams[b][:], start=True,
                                         stop=True)
                        if c == CH - 1:
                            nc.vector.tensor_tensor(
                                out=F_[:, b:b + 1], in0=pg[:K, :1],
                                in1=endem[:, b:b + 1], op=OP.mult)
                        else:
                            gn = ctmp.tile([K, 1], F32, tag=f"g{b}",
                                           name=f"g{c}_{b}")
                            nc.vector.tensor_copy(gn[:], pg[:K, :1])
                            gams[b] = gn
            else:
                mrow_i = ct("mrow_i", [1, T], I32)
                nc.sync.dma_start(mrow_i[:], dd["maskrow"][:])
                mrow = ct("mrow", [1, T])
                nc.vector.tensor_copy(mrow[:], mrow_i[:])
                inv9 = ct("inv9", [K, T])
                mb9 = ct("mb9", [K, T])
                for i in range(2):
                    pb = ps.tile([P, 512], F32, tag="ps", name="pmb",
                                 space="PSUM")
                    nc.tensor.matmul(pb[:K, :], lhsT=onescol_f[:1, :K],
                                     rhs=mrow[:, i * 512:(i + 1) * 512],
                                     start=True, stop=True)
                    nc.scalar.activation(mb9[:, i * 512:(i + 1) * 512],
                                         pb[:K, :], AF.Identity)
                nc.vector.tensor_scalar(out=inv9[:], in0=mb9[:], scalar1=0.0,
                                        scalar2=None, op0=OP.is_equal)
                inv4 = inv9[:].rearrange("k (b s) -> k b s", b=BC)
                # two independent 2-example chains
                Ecurs = []
                for hf in range(2):
                    Ec = ctmp.tile([K, 2], F32, tag=f"E{hf}", name=f"E0_{hf}")
                    nc.vector.tensor_scalar(
                        out=Ec[:], in0=expEm4[:, 2 * hf:2 * hf + 2, 0],
                        scalar1=expStart[:, :1], scalar2=None, op0=OP.mult)
                    Ecurs.append(Ec)
                for t in range(1, S):
                    for hf in range(2):
                        psn = ps.tile([P, 512], F32, tag="ps", name="pcrf",
                                      space="PSUM")
                        nc.tensor.matmul(psn[:K, :2], lhsT=Mexp[:],
                                         rhs=Ecurs[hf][:],
                                         start=True, stop=True)
                        Enew = ctmp.tile([K, 2], F32, tag=f"E{hf}",
                                         name=f"E{t}_{hf}")
                        nc.vector.tensor_tensor(
                            out=Enew[:], in0=psn[:K, :2],
                            in1=expEm4[:, 2 * hf:2 * hf + 2, t], op=OP.mult)
                        nc.vector.copy_predicated(
                            Enew[:], inv4[:, 2 * hf:2 * hf + 2, t],
                            Ecurs[hf][:])
                        Ecurs[hf] = Enew

                F_ = ctmp.tile([K, BC], F32, tag="F", name="F_")
                for hf in range(2):
                    nc.vector.tensor_scalar(
                        out=F_[:, 2 * hf:2 * hf + 2], in0=Ecurs[hf][:],
                        scalar1=expEnd[:, :1], scalar2=None, op0=OP.mult)
            psd = ps.tile([P, 512], F32, tag="ps", name="psd", space="PSUM")
            nc.tensor.matmul(psd[:1, :BC], lhsT=onescol_f[:K, :], rhs=F_[:],
                             start=True, stop=True)
            denomv = ctmp.tile([1, BC], F32, tag="denomv", name="denomv")
            denom_tot = ct("denom_tot", [1, 1])
            nc.scalar.activation(denomv[:], psd[:1, :BC], AF.Ln,
                                 accum_out=denom_tot[:])

            psn2 = ps.tile([P, 512], F32, tag="ps", name="psn2", space="PSUM")
            nc.tensor.matmul(psn2[:1, :1], lhsT=onescol_f[:K, :],
                             rhs=accK[:], start=True, stop=True)
            num_tot = ct("num_tot", [1, 1])
            nc.vector.tensor_copy(num_tot[:], psn2[:1, :1])
            out_sb = ct("out_sb", [1, 4])
            nc.vector.memset(out_sb[:], 0.0)
            nc.vector.tensor_tensor(out=out_sb[:, 0:1], in0=denom_tot[:],
                                    in1=num_tot[:], op=OP.subtract)
            nc.vector.tensor_copy(out_sb[:, 1:2], num_tot[:])
            nc.vector.tensor_copy(out_sb[:, 2:3], denom_tot[:])
            nc.sync.dma_start(out_d[:], out_sb[:])


# ---------------------------------------------------------------------------
# host wrapper
# ---------------------------------------------------------------------------

_NC_CACHE = {}


def _get_nc(key):
    if key not in _NC_CACHE:
        _NC_CACHE[key] = build_nc(*key)
    return _NC_CACHE[key]


def prepare_maps(inputs, mask_ones, zero_bias, unit_ln):
    """Returns (in_maps, label_const): per-core device inputs and the
    host-computed label-only CRF numerator sum over the whole batch."""
    input_ids = np.asarray(inputs["input_ids"]).astype(np.int32)
    attention_mask = np.asarray(inputs["attention_mask"]).astype(np.int32)
    labels = np.asarray(inputs["labels"]).astype(np.int64)

    word = _bf(inputs["word_emb"])
    pt = _bf((_f32(inputs["pos_emb"][:S]) + _f32(inputs["type_emb"][0])[None, :])
             .reshape(2, P, H).transpose(1, 0, 2))
    wq = _f8(inputs["Wq"], WS).reshape(L, 3, 2, P, H).transpose(
        0, 3, 1, 2, 4).copy()
    wk = _f8(inputs["Wk"], WS).reshape(L, 3, 2, P, H).transpose(
        0, 3, 1, 2, 4).copy()
    wv = _f8(inputs["Wv"], WS).reshape(L, 3, 2, P, H).transpose(
        0, 3, 1, 2, 4).copy()
    wo = _f8(inputs["Wo"], WS).reshape(L, 3, 2, P, H).transpose(
        0, 3, 1, 2, 4).copy()
    w1 = (_f8(inputs["W1"], WS).reshape(L, 3, 2, P, FC, P)
          .transpose(0, 4, 3, 1, 2, 5).copy())
    w2 = (_f8(inputs["W2"], WS).reshape(L, FC // 2, 2, P, H)
          .transpose(0, 1, 3, 2, 4).copy())
    cwpad = np.zeros((H, 16), np.float32)
    cwpad[:, :K] = _f32(inputs["cls_W"])
    clsw = _f8(cwpad, WS).reshape(3, 2, P, 16).transpose(2, 0, 1, 3).copy()
    clsb = (_f32(inputs["cls_b"]) - np.float32(C_SHIFT)).reshape(K, 1)

    trans = _f32(inputs["crf_trans"]).reshape(K, K)
    startv = _f32(inputs["crf_start"]).reshape(K)
    endv = _f32(inputs["crf_end"]).reshape(K)

    shared = dict(
        word=word, pt=pt, wq=wq, wk=wk, wv=wv, wo=wo, w1=w1, w2=w2,
        clsw=clsw, clsb=clsb,
        mexp=np.exp(trans).astype(np.float32),
        mexpt=np.ascontiguousarray(np.exp(trans).T.astype(np.float32)),
        mrep=np.ascontiguousarray(np.broadcast_to(
            np.exp(trans).astype(np.float32)[:, None, None, :],
            (K, BC, 16, K))),
        expstart=np.exp(startv).astype(np.float32).reshape(K, 1),
        expend=np.exp(endv).astype(np.float32).reshape(K, 1),
    )
    if not zero_bias:
        shared.update(
            bq=_f32(inputs["bq"]).reshape(L, HC, P).transpose(0, 2, 1).copy(),
            bk=_f32(inputs["bk"]).reshape(L, HC, P).transpose(0, 2, 1).copy(),
            b1=_f32(inputs["b1"]).reshape(L, FC, P).transpose(0, 2, 1).copy(),
            bvrow=_bf(_f32(inputs["bv"]) / DQ_WX).reshape(L, 1, H),
            borow=_bf(_f32(inputs["bo"]) / DQ_WX).reshape(L, 1, H),
            b2row=_bf(_f32(inputs["b2"]) / DQ_W).reshape(L, 1, H),
        )
    if not unit_ln:
        shared.update(
            lng=np.stack([_bf(inputs["ln1_g"]), _bf(inputs["ln2_g"])],
                         axis=1).reshape(L, 2, 1, H),
            lnb=np.stack([_bf(inputs["ln1_b"]), _bf(inputs["ln2_b"])],
                         axis=1).reshape(L, 2, 1, H),
            elng=np.stack([_bf(inputs["emb_ln_g"]), _bf(inputs["emb_ln_b"])],
                          axis=0).reshape(2, 1, H),
        )

    # ---- host label-only numerator + per-core oh9w ----
    mf = attention_mask.astype(np.float32)               # [B, S]
    w9 = mf.copy()
    w9[:, 0] = 1.0                                       # t=0 emission always counted
    trans_gold = trans[labels[:, :-1], labels[:, 1:]]    # [B, S-1]
    last_idx = attention_mask.astype(np.int64).sum(axis=1) - 1
    label_num = (startv[labels[:, 0]]
                 + (trans_gold * mf[:, 1:]).sum(axis=1)
                 + endv[labels[np.arange(B), last_idx]])  # [B]
    label_const = float(np.float32(label_num.astype(np.float32).sum()))

    in_maps = []
    for c in range(CORES):
        ids_c = input_ids[BC * c:BC * (c + 1)].reshape(NT, P, 1).copy()
        lab_c = labels[BC * c:BC * (c + 1)]              # [BC, S]
        w9_c = w9[BC * c:BC * (c + 1)]                   # [BC, S]
        oh = np.zeros((K, BC, S), np.float32)
        oh[lab_c.reshape(-1), np.repeat(np.arange(BC), S),
           np.tile(np.arange(S), BC)] = w9_c.reshape(-1)
        msk_c = attention_mask[BC * c:BC * (c + 1)]
        m = dict(shared)
        m["ids"] = ids_c
        m["oh9w"] = oh.reshape(K, T).copy()
        if not mask_ones:
            m["maskrow"] = msk_c.reshape(1, T).copy()
            m["maskcols"] = (msk_c.reshape(BC, 2, P).transpose(2, 1, 0)
                             .astype(np.int32).copy())
        in_maps.append(m)
    return in_maps, label_const


def kernel(**inputs) -> np.ndarray:
    attention_mask = np.asarray(inputs["attention_mask"])
    assert np.asarray(inputs["input_ids"]).shape == (B, S)

    mask_ones = bool(np.all(attention_mask == 1))
    zero_bias = all(
        not np.any(np.asarray(inputs[k]))
        for k in ("bq", "bk", "bv", "bo", "b1", "b2"))
    unit_ln = (all(np.all(np.asarray(inputs[k]) == 1.0)
                   for k in ("emb_ln_g", "ln1_g", "ln2_g"))
               and all(not np.any(np.asarray(inputs[k]))
                       for k in ("emb_ln_b", "ln1_b", "ln2_b")))

    n_layers = int(os.environ.get("BERTCRF_LAYERS", L))
    debug = bool(int(os.environ.get("BERTCRF_DEBUG", "0")))
    nc = _get_nc((n_layers, mask_ones, zero_bias, unit_ln, debug))
    in_maps, label_const = prepare_maps(inputs, mask_ones, zero_bias, unit_ln)

    res = run_bass_kernel_spmd(nc, in_maps, core_ids=list(range(CORES)))
    total = np.float32(0.0)
    for c in range(CORES):
        total += np.float32(res.results[c]["out"][0, 0])
    return np.float32(total - np.float32(label_const))


if __name__ == "__main__":
    import jax
    jax.config.update("jax_platforms", "cpu")
    import reference
    inp = reference.setup_inputs()
    outv = kernel(**{k: np.asarray(v) for k, v in inp.items()})
    print("kernel:", outv)


# revision 40
# speedup vs baseline: 1.1544x; 1.0115x over previous
"""BertCRF forward (BERT-base encoder + CRF NLL) on 8 Trainium2 NeuronCores.

Strategy: data-parallel over the batch (32 examples -> 4 per core), params
replicated.  Each core runs the full 12-layer encoder on its 1024 tokens with
fp8 (DoubleRow) matmuls for the dense projections, bf16 attention, fp32
layernorm, and a max-free softmax whose normalizer is computed by an extra
ones-matmul on the PE and inverted with a fast DVE reciprocal.  The CRF
numerator's label-only terms (start/transition/end) are computed on the host;
the device computes the gold-emission dot product and the exact linear-space
forward scan with a fixed per-step shift (cancels exactly between numerator
and denominator).  The host shards inputs, pre-arranges weight layouts, and
sums the 8 per-core partial NLLs plus the host-side label constant.
"""

import contextlib
import os

import numpy as np
import ml_dtypes

import concourse.bass as bass  # noqa: F401
import concourse.mybir as mybir
import concourse.tile as tile
from concourse import bacc
from concourse.bass import IndirectOffsetOnAxis
from concourse.bass_utils import run_bass_kernel_spmd
from concourse.masks import make_identity

# ---- problem constants (hardcoded per the task spec) ----
L, H, NH, DH, FF, V, K = 12, 768, 12, 64, 3072, 30522, 9
B, S = 32, 256
CORES = 8
BC = B // CORES          # 4 examples per core
T = BC * S               # 1024 tokens per core
P = 128
NT = T // P              # 8 token tiles
HC = H // P              # 6 hidden chunks
FC = FF // P             # 24 ff chunks
C_SHIFT = 2.35           # per-step CRF shift (cancels exactly in num-denom)

F32 = mybir.dt.float32
BF16 = mybir.dt.bfloat16
FP8 = mybir.dt.float8e4
I32 = mybir.dt.int32
AX = mybir.AxisListType
OP = mybir.AluOpType
AF = mybir.ActivationFunctionType
DR = mybir.MatmulPerfMode.DoubleRow

BF = ml_dtypes.bfloat16

# fp8 quantization scales (exact powers of two)
WS = 1024.0              # weight scale into fp8e4
XS = 32.0                # activation scale into fp8e4
DQ_WX = 2.0 ** -15       # dequant for w*x products
DQ_W = 2.0 ** -10        # dequant when only the weight was scaled


def _bf(x):
    return np.ascontiguousarray(np.asarray(x, dtype=np.float32)).astype(BF)


def _f8(x, scale):
    return np.ascontiguousarray(np.clip(
        np.asarray(x, dtype=np.float32) * scale, -240.0, 240.0)
    ).astype(ml_dtypes.float8_e4m3)


def _f32(x):
    return np.ascontiguousarray(np.asarray(x, dtype=np.float32))


# ---------------------------------------------------------------------------
# device program
# ---------------------------------------------------------------------------

def _layernorm(nc, tmp, out_bf, xf, gb, s1=None, sq_act=True):
    """LN over the free dim of xf [P, H] f32 -> out_bf (bf16).

    s1, if given, is a [P, 1] tile already holding sum(xf) (computed for free
    via accum_out on the op that produced xf).  sq_act picks the engine for
    the sum-of-squares pass: ACT (Square is in every table set) when ACT has
    headroom, DVE when ACT is the busier engine (attention phase).
    """
    if s1 is None:
        s1 = tmp.tile([P, 1], F32, tag="s1", name="s1")
        nc.vector.tensor_reduce(out=s1[:], in_=xf[:], axis=AX.X, op=OP.add)
    sq = tmp.tile([P, H], F32, tag="sq", name="sq")
    s2 = tmp.tile([P, 1], F32, tag="s2", name="s2")
    if sq_act:
        nc.scalar.activation(sq[:], xf[:], AF.Square, accum_out=s2[:])
    else:
        nc.vector.scalar_tensor_tensor(out=sq[:], in0=xf[:], scalar=1.0,
                                       in1=xf[:], op0=OP.mult, op1=OP.mult,
                                       accum_out=s2[:])
    m = tmp.tile([P, 1], F32, tag="m", name="m")
    nc.vector.tensor_scalar(out=m[:], in0=s1[:], scalar1=1.0 / H, scalar2=None,
                            op0=OP.mult)
    msq = tmp.tile([P, 1], F32, tag="msq", name="msq")
    nc.vector.tensor_tensor(out=msq[:], in0=m[:], in1=m[:], op=OP.mult)
    var = tmp.tile([P, 1], F32, tag="var", name="var")
    nc.vector.tensor_scalar(out=var[:], in0=s2[:], scalar1=1.0 / H,
                            scalar2=msq[:, :1], op0=OP.mult, op1=OP.subtract)
    # eps=1e-12 is below f32 resolution for var~O(1); bias=0.0 is identical
    sd = tmp.tile([P, 1], F32, tag="sd", name="sd")
    nc.scalar.activation(sd[:], var[:], AF.Sqrt, bias=0.0)
    rs = tmp.tile([P, 1], F32, tag="rs", name="rs")
    nc.vector.reciprocal_approx_fast(rs[:], sd[:])
    if gb is None:
        # out = rs*x - m*rs, one half on DVE, one half on ACT (in parallel)
        nmrs = tmp.tile([P, 1], F32, tag="nmrs", name="nmrs")
        nc.vector.tensor_scalar(out=nmrs[:], in0=m[:], scalar1=-1.0,
                                scalar2=rs[:, :1], op0=OP.mult, op1=OP.mult)
        HH = H // 2
        nc.vector.tensor_scalar(out=out_bf[:, 0:HH], in0=xf[:, 0:HH],
                                scalar1=m[:, :1], scalar2=rs[:, :1],
                                op0=OP.subtract, op1=OP.mult)
        nc.scalar.activation(out_bf[:, HH:H], xf[:, HH:H], AF.Identity,
                             bias=nmrs[:, :1], scale=rs[:, :1])
    else:
        G, Bb = gb
        t2 = tmp.tile([P, H], F32, tag="t2", name="t2")
        nc.vector.tensor_scalar(out=t2[:], in0=xf[:], scalar1=m[:, :1],
                                scalar2=rs[:, :1], op0=OP.subtract, op1=OP.mult)
        t3 = tmp.tile([P, H], F32, tag="t3", name="t3")
        nc.vector.tensor_tensor(out=t3[:], in0=t2[:], in1=G[:], op=OP.mult)
        nc.vector.tensor_tensor(out=out_bf, in0=t3[:], in1=Bb[:], op=OP.add)


def _transpose_tiles(nc, ps, xT, x_sb, ident, tiles):
    """x_sb [P, NT, H] token-major -> xT [P, HC, T] feature-major, per tile."""
    for g in tiles:
        for cg in range(2):          # chunk groups of 3
            pt_ = ps.tile([P, 1024], BF16, tag="ps", name="ptp", space="PSUM")
            for ci in range(3):
                c = cg * 3 + ci
                nc.tensor.matmul(
                    pt_[:, ci * P:(ci + 1) * P],
                    lhsT=x_sb[:, g, c * P:(c + 1) * P], rhs=ident[:],
                    start=True, stop=True, is_transpose=True)
            nc.scalar.activation(
                xT[:, cg * 3:cg * 3 + 3, g * P:(g + 1) * P],
                pt_[:, :384], AF.Identity, scale=XS)


def _bcast_row(nc, ps, tmp, dst, row_dram, ones_bf):
    """dst [P, H] bf16 = broadcast of a [1, H] bf16 dram row across partitions."""
    row = tmp.tile([1, H], BF16, tag="brow", name="brow")
    nc.sync.dma_start(row[:], row_dram)
    for fh in range(2):
        pb = ps.tile([P, 512], F32, tag="ps", name="pbc", space="PSUM")
        nc.tensor.matmul(pb[:, :384], lhsT=ones_bf[:, :P],
                         rhs=row[:, fh * 384:(fh + 1) * 384],
                         start=True, stop=True)
        nc.scalar.activation(dst[:, fh * 384:(fh + 1) * 384], pb[:, :384],
                             AF.Identity)


def build_nc(n_layers=L, mask_ones=True, zero_bias=True, unit_ln=True,
             debug=False):
    nc = bacc.Bacc("TRN2", target_bir_lowering=False, debug=False)

    dd = {}

    def din(name, shape, dtype):
        dd[name] = nc.dram_tensor(name, list(shape), dtype, kind="ExternalInput")
        return dd[name]

    def dout(name, shape, dtype):
        dd[name] = nc.dram_tensor(name, list(shape), dtype, kind="ExternalOutput")
        return dd[name]

    din("word", [V, H], BF16)
    din("ids", [NT, P, 1], I32)
    din("pt", [P, 2, H], BF16)
    din("wq", [L, P, 3, 2, H], FP8)
    din("wk", [L, P, 3, 2, H], FP8)
    din("wv", [L, P, 3, 2, H], FP8)
    din("wo", [L, P, 3, 2, H], FP8)
    din("w1", [L, FC, P, 3, 2, P], FP8)   # [l, j, ki, c2, ko, m]
    din("w2", [L, FC // 2, P, 2, H], FP8)  # [l, c2, ki, ko, n]
    din("clsw", [P, 3, 2, 16], FP8)      # K padded to 16
    din("clsb", [K, 1], F32)             # already shifted by -C_SHIFT
    din("mexp", [K, K], F32)             # exp(crf_trans)
    din("mexpt", [K, K], F32)            # exp(crf_trans).T
    din("mrep", [K, BC, 16, K], F32)     # exp(crf_trans) replicated 64x
    din("expstart", [K, 1], F32)         # exp(crf_start)
    din("expend", [K, 1], F32)           # exp(crf_end)
    din("oh9w", [K, T], F32)             # one-hot(labels) * emission weight
    if not mask_ones:
        din("maskrow", [1, T], I32)
        din("maskcols", [P, 2, BC], I32)
    if not zero_bias:
        din("bq", [L, P, HC], F32)
        din("bk", [L, P, HC], F32)
        din("b1", [L, P, FC], F32)
        din("bvrow", [L, 1, H], BF16)
        din("borow", [L, 1, H], BF16)
        din("b2row", [L, 1, H], BF16)
    if not unit_ln:
        din("lng", [L, 2, 1, H], BF16)
        din("lnb", [L, 2, 1, H], BF16)
        din("elng", [2, 1, H], BF16)

    dout("out", [1, 4], F32)
    if debug:
        dout("dbg_x0", [P, NT, H], BF16)
        dout("dbg_x", [P, NT, H], BF16)
        dout("dbg_em", [K, T], F32)

    _build_body(nc, dd, n_layers, mask_ones, zero_bias, unit_ln, debug)
    nc.compile()
    return nc


def _build_body(nc, dd, n_layers, mask_ones, zero_bias, unit_ln, debug):
    (word, ids, pt, wq_d, wk_d, wv_d, wo_d, w1_d, w2_d, clsw_d, clsb_d,
     mexp_d, expstart_d, expend_d, oh9w_d, out_d) = (
        dd["word"], dd["ids"], dd["pt"], dd["wq"], dd["wk"], dd["wv"],
        dd["wo"], dd["w1"], dd["w2"], dd["clsw"], dd["clsb"], dd["mexp"],
        dd["expstart"], dd["expend"], dd["oh9w"], dd["out"])
    if not mask_ones:
        maskrow_d = dd["maskrow"]
        maskcols_d = dd["maskcols"]
    if not zero_bias:
        bq_d, bk_d, b1_d = dd["bq"], dd["bk"], dd["b1"]
        bvrow_d, borow_d, b2row_d = dd["bvrow"], dd["borow"], dd["b2row"]
    if not unit_ln:
        lng_d, lnb_d, elng_d = dd["lng"], dd["lnb"], dd["elng"]
    with tile.TileContext(nc) as tc, contextlib.ExitStack() as octx:
        cst = octx.enter_context(tc.tile_pool(name="cst", bufs=1))
        act = octx.enter_context(tc.tile_pool(name="act", bufs=1))
        ps = octx.enter_context(tc.tile_pool(name="ps", bufs=8, space="PSUM"))

        # ---- persistent activation buffers ----
        x_sb = act.tile([P, NT, H], BF16, tag="x_sb", name="x_sb")
        xT = act.tile([P, HC, T], FP8, tag="xT", name="xT")
        qT = act.tile([P, HC, T], BF16, tag="qT", name="qT")
        kT = act.tile([P, HC, T], BF16, tag="kT", name="kT")
        vS = act.tile([P, NT, NH, DH], BF16, tag="vS", name="vS")
        cT = act.tile([P, HC, T], FP8, tag="cT", name="cT")
        hT = act.tile([P, FC, T // 2], FP8, tag="hT", name="hT")
        emT = act.tile([K, T], F32, tag="emT", name="emT")
        expEm = act.tile([K, T], F32, tag="expEm", name="expEm")

        # ---- constants ----
        ident = cst.tile([P, P], BF16, tag="ident", name="ident")
        make_identity(nc, ident[:])
        ones_bf = cst.tile([1, P], BF16, tag="ones_bf", name="ones_bf")
        nc.vector.memset(ones_bf[:], 1.0)
        # ones64: [128, 64] all-ones lhsT; sum over keys of exp(scores) into
        # one 64-partition half of the normalizer PSUM tile per head
        ones64 = cst.tile([P, DH], BF16, tag="ones64", name="ones64")
        nc.vector.memset(ones64[:], 1.0)
        onescol_f = cst.tile([P, 1], F32, tag="onescol_f", name="onescol_f")
        nc.vector.memset(onescol_f[:], 1.0)

        pt_sb = cst.tile([P, 2, H], BF16, tag="pt_sb", name="pt_sb")
        nc.sync.dma_start(pt_sb[:], pt[:])
        if not mask_ones:
            mcol = cst.tile([P, 2, BC], F32, tag="mcol", name="mcol")
        if not unit_ln:
            elnG = cst.tile([P, H], BF16, tag="elnG", name="elnG")
            elnB = cst.tile([P, H], BF16, tag="elnB", name="elnB")

        with contextlib.ExitStack() as ictx:
            wts = ictx.enter_context(tc.tile_pool(name="wts", bufs=1))
            tmp = ictx.enter_context(tc.tile_pool(name="tmp", bufs=3))

            if not unit_ln:
                _bcast_row(nc, ps, tmp, elnG, elng_d[0], ones_bf)
                _bcast_row(nc, ps, tmp, elnB, elng_d[1], ones_bf)

            # =========== embeddings ===========
            for g in range(NT):
                idx = tmp.tile([P, 1], I32, tag="idx", name="idx")
                nc.sync.dma_start(idx[:], ids[g])
                emb = tmp.tile([P, H], BF16, tag="emb", name="emb")
                nc.gpsimd.indirect_dma_start(
                    out=emb[:], out_offset=None, in_=word[:],
                    in_offset=IndirectOffsetOnAxis(ap=idx[:, :1], axis=0),
                )
                xf = tmp.tile([P, H], F32, tag="xf", name="xf")
                s1e = tmp.tile([P, 1], F32, tag="s1e", name="s1e")
                nc.vector.scalar_tensor_tensor(
                    out=xf[:], in0=emb[:], scalar=0.0,
                    in1=pt_sb[:, g % 2, :], op0=OP.add, op1=OP.add,
                    accum_out=s1e[:])
                _layernorm(nc, tmp, x_sb[:, g, :], xf,
                           None if unit_ln else (elnG, elnB), s1=s1e)
            if debug:
                nc.sync.dma_start(dd["dbg_x0"][:], x_sb[:])
            # prime xT tiles 0-3 for layer 0's QK proj t2=0
            _transpose_tiles(nc, ps, xT, x_sb, ident, range(4))

            if not mask_ones:
                mi = tmp.tile([P, 2, BC], I32, tag="mi", name="mi")
                nc.sync.dma_start(mi[:], maskcols_d[:])
                nc.vector.tensor_scalar(out=mcol[:], in0=mi[:], scalar1=1.0,
                                        scalar2=10000.0, op0=OP.subtract,
                                        op1=OP.mult)

            # =========== encoder layers ===========
            for l in range(n_layers):
                wq = wts.tile([P, 3, 2, H], FP8, tag="wq", name="wq")
                wk = wts.tile([P, 3, 2, H], FP8, tag="wk", name="wk")
                wv = wts.tile([P, 3, 2, H], FP8, tag="wv", name="wv")
                wo = wts.tile([P, 3, 2, H], FP8, tag="wo", name="wo")
                nc.sync.dma_start(wq[:], wq_d[l])
                nc.sync.dma_start(wk[:], wk_d[l])
                nc.sync.dma_start(wv[:], wv_d[l])
                nc.sync.dma_start(wo[:], wo_d[l])
                # preload the whole layer's W1 once (18.4KB/partition): the
                # 24 transfers stream during attention, FFN1 reads them from
                # SBUF in both token-half passes — halves W1 HBM traffic and
                # removes the just-in-time DMA stalls inside the FFN1 loop
                w1js = []
                for j in range(FC):
                    w1j = wts.tile([P, 3, 2, P], FP8, tag="w1j",
                                   name=f"w1j{j}", bufs=FC)
                    nc.sync.dma_start(w1j[:], w1_d[l, j])
                    w1js.append(w1j)

                if not zero_bias:
                    bq_sb = wts.tile([P, HC], F32, tag="bq", name="bq")
                    bk_sb = wts.tile([P, HC], F32, tag="bk", name="bk")
                    b1_sb = wts.tile([P, FC], F32, tag="b1", name="b1")
                    nc.sync.dma_start(bq_sb[:], bq_d[l])
                    nc.sync.dma_start(bk_sb[:], bk_d[l])
                    nc.sync.dma_start(b1_sb[:], b1_d[l])
                    bvrow = wts.tile([1, H], BF16, tag="bvrow", name="bvrow")
                    borow = wts.tile([1, H], BF16, tag="borow", name="borow")
                    b2row = wts.tile([1, H], BF16, tag="b2row", name="b2row")
                    nc.sync.dma_start(bvrow[:], bvrow_d[l])
                    nc.sync.dma_start(borow[:], borow_d[l])
                    nc.sync.dma_start(b2row[:], b2row_d[l])
                if not unit_ln:
                    G1 = wts.tile([P, H], BF16, tag="G1", name="G1")
                    B1t = wts.tile([P, H], BF16, tag="B1t", name="B1t")
                    G2 = wts.tile([P, H], BF16, tag="G2", name="G2")
                    B2t = wts.tile([P, H], BF16, tag="B2t", name="B2t")
                    _bcast_row(nc, ps, tmp, G1, lng_d[l, 0], ones_bf)
                    _bcast_row(nc, ps, tmp, B1t, lnb_d[l, 0], ones_bf)
                    _bcast_row(nc, ps, tmp, G2, lng_d[l, 1], ones_bf)
                    _bcast_row(nc, ps, tmp, B2t, lnb_d[l, 1], ones_bf)

                # ---- qT/kT projections, token-half pipelined.  xT tiles 0-3
                #      were already transposed in the previous layer's FFN
                #      tail (or right after the embeddings for layer 0), so
                #      QK proj t2=0 can start while the previous layer's
                #      second-half LN2 chain is still draining; tiles 4-7 are
                #      transposed here in between. ----
                for t2 in range(2):
                    if t2 == 1:
                        _transpose_tiles(nc, ps, xT, x_sb, ident, range(4, 8))
                    for wmat, bname, dst in ((wq, "bq", qT), (wk, "bk", kT)):
                        for f in range(HC):
                            pm = ps.tile([P, 512], F32, tag="ps", name="pqk",
                                         space="PSUM")
                            for c2 in range(3):
                                nc.tensor.matmul(
                                    pm[:],
                                    lhsT=wmat[:, c2, :, f * P:(f + 1) * P],
                                    rhs=xT[:, 2 * c2:2 * c2 + 2,
                                           t2 * 512:(t2 + 1) * 512],
                                    start=(c2 == 0), stop=(c2 == 2),
                                    perf_mode=DR)
                            if zero_bias:
                                nc.vector.tensor_scalar(
                                    out=dst[:, f, t2 * 512:(t2 + 1) * 512],
                                    in0=pm[:], scalar1=DQ_WX, scalar2=None,
                                    op0=OP.mult)
                            else:
                                bias = (bq_sb if bname == "bq"
                                        else bk_sb)[:, f:f + 1]
                                nc.scalar.activation(
                                    dst[:, f, t2 * 512:(t2 + 1) * 512], pm[:],
                                    AF.Identity, bias=bias, scale=DQ_WX)

                # ---- V projection (token-major into vS) ----
                for g in range(NT):
                    for fh in range(2):
                        pm = ps.tile([P, 512], F32, tag="ps", name="pv",
                                     space="PSUM")
                        for c2 in range(3):
                            nc.tensor.matmul(
                                pm[:, :384],
                                lhsT=xT[:, 2 * c2:2 * c2 + 2,
                                        g * P:(g + 1) * P],
                                rhs=wv[:, c2, :, fh * 384:(fh + 1) * 384],
                                start=(c2 == 0),
                                stop=(c2 == 2 and zero_bias),
                                perf_mode=DR)
                        if not zero_bias:
                            nc.tensor.matmul(
                                pm[:, :384], lhsT=ones_bf[:, :P],
                                rhs=bvrow[:, fh * 384:(fh + 1) * 384],
                                start=False, stop=True)
                        nc.vector.tensor_scalar(
                            out=vS[:, g, 6 * fh:6 * fh + 6, :],
                            in0=pm[:, :384], scalar1=DQ_WX, scalar2=None,
                            op0=OP.mult)

                # ---- output proj + residual + LN1 (interleaved
                #      into the attention loop, per example) ----
                def _oproj_ln1(g):
                    xf = tmp.tile([P, H], F32, tag="xf", name="xf")
                    s1a = tmp.tile([P, 1], F32, tag="s1a", name="s1a")
                    s1b = tmp.tile([P, 1], F32, tag="s1b", name="s1b")
                    for fh in range(2):
                        pm = ps.tile([P, 512], F32, tag="ps", name="po",
                                     space="PSUM")
                        for c2 in range(3):
                            nc.tensor.matmul(
                                pm[:, :384],
                                lhsT=cT[:, 2 * c2:2 * c2 + 2,
                                        g * P:(g + 1) * P],
                                rhs=wo[:, c2, :, fh * 384:(fh + 1) * 384],
                                start=(c2 == 0),
                                stop=(c2 == 2 and zero_bias),
                                perf_mode=DR)
                        if not zero_bias:
                            nc.tensor.matmul(
                                pm[:, :384], lhsT=ones_bf[:, :P],
                                rhs=borow[:, fh * 384:(fh + 1) * 384],
                                start=False, stop=True)
                        nc.vector.scalar_tensor_tensor(
                            out=xf[:, fh * 384:(fh + 1) * 384],
                            in0=pm[:, :384], scalar=DQ_WX,
                            in1=x_sb[:, g, fh * 384:(fh + 1) * 384],
                            op0=OP.mult, op1=OP.add,
                            accum_out=(s1a[:] if fh == 0 else s1b[:]))
                    s1g = tmp.tile([P, 1], F32, tag="s1g", name="s1g",
                                   bufs=4)
                    nc.vector.tensor_tensor(out=s1g[:], in0=s1a[:],
                                            in1=s1b[:], op=OP.add)
                    _layernorm(nc, tmp, x_sb[:, g, :], xf,
                               None if unit_ln else (G1, B1t), s1=s1g)

                # ---- attention.  Per head-pair: QK^T (2 heads in separate
                #      PE row-groups), fused exp on ACT, then per head both
                #      the AxV matmul and a ones-matmul normalizer sum (z)
                #      on PE.  The normalize tail (fast reciprocal of z +
                #      multiply) is deferred one pair so it overlaps the
                #      next pair's matmul/exp front.  PSUM: 4 banks/pair ->
                #      two pairs in flight. ----
                def _attn_tail(st):
                    e, ch, prbz, pcx = st
                    rb2 = tmp.tile([P, S], F32, tag="rb", name="rb")
                    nc.vector.reciprocal_approx_fast(rb2[:], prbz[:, :S])
                    for hh in range(2):
                        r0 = hh * DH
                        nc.vector.tensor_tensor(
                            out=cT[r0:r0 + DH, ch, e * S:(e + 1) * S],
                            in0=pcx[:DH, hh * S:(hh + 1) * S],
                            in1=rb2[r0:r0 + DH, :], op=OP.mult)

                # QK^T contracts only 64 partitions; heads A/B live in
                # PE row-groups {0,1}/{2,3} (lhsT base 0/64), so
                # alternating their matmuls runs them concurrently.
                # The QK matmuls of pair i+1 are EMITTED before the AV/z
                # matmuls of pair i: PE executes in order, so this gives it
                # work to do while pair i's exp runs on ACT.
                def _emit_qk(e, ch):
                    pscs = [ps.tile([P, 512], F32, tag="ps", name="psc",
                                    space="PSUM") for _ in range(2)]
                    for kt in range(2):
                        for hh in range(2):
                            r0 = hh * DH
                            nc.tensor.matmul(
                                pscs[hh][:, kt * S:(kt + 1) * S],
                                lhsT=kT[r0:r0 + DH, ch,
                                        e * S + kt * P:
                                        e * S + (kt + 1) * P],
                                rhs=qT[r0:r0 + DH, ch, e * S:(e + 1) * S],
                                start=True, stop=True)
                    return pscs

                pairs = [(e, ch) for e in range(BC) for ch in range(NH // 2)]
                prev_st = None
                pscs = _emit_qk(*pairs[0])
                for i, (e, ch) in enumerate(pairs):
                    next_pscs = (_emit_qk(*pairs[i + 1])
                                 if i + 1 < len(pairs) else None)
                    ET2 = tmp.tile([P, 2, 2, S], BF16, tag="ET", name="ET")
                    pcx = ps.tile([P, 512], F32, tag="ps", name="pcx",
                                  space="PSUM")
                    prbz = ps.tile([P, 512], F32, tag="ps", name="prb",
                                   space="PSUM")
                    for hh in range(2):
                        r0 = hh * DH
                        psc = pscs[hh]
                        if mask_ones:
                            # one fused exp over both key tiles
                            nc.scalar.activation(
                                ET2[:, hh].rearrange("p k s -> p (k s)"),
                                psc[:], AF.Exp, bias=0.0, scale=0.125)
                        else:
                            for kt in range(2):
                                nc.scalar.activation(
                                    ET2[:, hh, kt, :],
                                    psc[:, kt * S:(kt + 1) * S],
                                    AF.Exp, bias=mcol[:, kt, e:e + 1],
                                    scale=0.125)
                        h = 2 * ch + hh
                        for kt in range(2):
                            nc.tensor.matmul(
                                pcx[:DH, hh * S:(hh + 1) * S],
                                lhsT=vS[:, 2 * e + kt, h, :],
                                rhs=ET2[:, hh, kt, :],
                                start=(kt == 0), stop=(kt == 1))
                        for kt in range(2):
                            nc.tensor.matmul(
                                prbz[r0:r0 + DH, :S],
                                lhsT=ones64[:],
                                rhs=ET2[:, hh, kt, :],
                                start=(kt == 0), stop=(kt == 1))
                    if prev_st is not None:
                        _attn_tail(prev_st)
                    prev_st = (e, ch, prbz, pcx)
                    pscs = next_pscs
                _attn_tail(prev_st)
                for g in range(NT):
                    _oproj_ln1(g)

                # ---- FFN (two token-half passes) ----
                for th in range(2):
                    tiles = list(range(4 * th, 4 * th + 4))
                    _transpose_tiles(nc, ps, xT, x_sb, ident, tiles)
                    for j in range(FC):
                        if th == 1 and j == FC - 1:
                            # post-LN2 re-transpose of tiles 0-3 for the next
                            # layer (or classifier), emitted here so its
                            # PSUM->xT copies drain on ACT during the FFN2
                            # window instead of behind the LN2 chain
                            _transpose_tiles(nc, ps, xT, x_sb, ident, range(4))
                        pm = ps.tile([P, 512], F32, tag="ps", name="ph",
                                     space="PSUM")
                        for c2 in range(3):
                            nc.tensor.matmul(
                                pm[:], lhsT=w1js[j][:, c2],
                                rhs=xT[:, 2 * c2:2 * c2 + 2,
                                       th * 512:(th + 1) * 512],
                                start=(c2 == 0), stop=(c2 == 2),
                                perf_mode=DR)
                        bias = 0.0 if zero_bias else b1_sb[:, j:j + 1]
                        nc.scalar.activation(hT[:, j, :], pm[:], AF.Gelu,
                                             bias=bias, scale=DQ_WX)
                    # FFN2: f-half outer so W2 streams once per (th, fh)
                    xfs = [tmp.tile([P, H], F32, tag="xff", name="xff", bufs=4)
                           for _ in range(4)]
                    s1as = [tmp.tile([P, 1], F32, tag="s1fa", name="s1fa",
                                     bufs=4) for _ in range(4)]
                    s1bs = [tmp.tile([P, 1], F32, tag="s1fb", name="s1fb",
                                     bufs=4) for _ in range(4)]
                    for fh in range(2):
                        pms = [ps.tile([P, 512], F32, tag="ps", name="pf2",
                                       space="PSUM") for _ in range(4)]
                        for c2 in range(FC // 2):
                            w2c = wts.tile([P, 2, 384], FP8, tag="w2c",
                                           name="w2c", bufs=8)
                            nc.sync.dma_start(
                                w2c[:],
                                w2_d[l, c2, :, :, fh * 384:(fh + 1) * 384])
                            for gi in range(4):
                                nc.tensor.matmul(
                                    pms[gi][:, :384],
                                    lhsT=hT[:, 2 * c2:2 * c2 + 2,
                                            gi * P:(gi + 1) * P],
                                    rhs=w2c[:],
                                    start=(c2 == 0),
                                    stop=(c2 == FC // 2 - 1 and zero_bias),
                                    perf_mode=DR)
                        if not zero_bias:
                            for gi in range(4):
                                nc.tensor.matmul(
                                    pms[gi][:, :384], lhsT=ones_bf[:, :P],
                                    rhs=b2row[:, fh * 384:(fh + 1) * 384],
                                    start=False, stop=True)
                        for gi in range(4):
                            g = tiles[gi]
                            nc.vector.scalar_tensor_tensor(
                                out=xfs[gi][:, fh * 384:(fh + 1) * 384],
                                in0=pms[gi][:, :384], scalar=DQ_W,
                                in1=x_sb[:, g, fh * 384:(fh + 1) * 384],
                                op0=OP.mult, op1=OP.add,
                                accum_out=(s1as[gi][:] if fh == 0
                                           else s1bs[gi][:]))
                    for gi in range(4):
                        s1g = tmp.tile([P, 1], F32, tag="s1g", name="s1g",
                                       bufs=4)
                        nc.vector.tensor_tensor(out=s1g[:], in0=s1as[gi][:],
                                                in1=s1bs[gi][:], op=OP.add)
                        _layernorm(nc, tmp, x_sb[:, tiles[gi], :], xfs[gi],
                                   None if unit_ln else (G2, B2t), s1=s1g)

            if debug:
                nc.sync.dma_start(dd["dbg_x"][:], x_sb[:])

            # =========== classifier ===========
            clsw = cst.tile([P, 3, 2, 16], FP8, tag="clsw", name="clsw")
            nc.sync.dma_start(clsw[:], clsw_d[:])
            clsb = cst.tile([K, 1], F32, tag="clsb", name="clsb")
            nc.sync.dma_start(clsb[:], clsb_d[:])
            # tiles 0-3 already re-transposed in the last layer's FFN tail
            _transpose_tiles(nc, ps, xT, x_sb, ident, range(4, NT))
            for t2 in range(2):
                pm = ps.tile([P, 512], F32, tag="ps", name="pcls", space="PSUM")
                for c2 in range(3):
                    nc.tensor.matmul(
                        pm[:K, :], lhsT=clsw[:, c2, :, 0:K],
                        rhs=xT[:, 2 * c2:2 * c2 + 2, t2 * 512:(t2 + 1) * 512],
                        start=(c2 == 0), stop=(c2 == 2), perf_mode=DR)
                nc.scalar.activation(emT[:, t2 * 512:(t2 + 1) * 512],
                                     pm[:K, :], AF.Identity, bias=clsb[:, :1],
                                     scale=DQ_WX)
            nc.scalar.activation(expEm[:], emT[:], AF.Exp)
            if debug:
                nc.sync.dma_start(dd["dbg_em"][:], emT[:])

        # =========== CRF (weights/tmp pools closed; SBUF freed) ===========
        with contextlib.ExitStack() as cctx:
            crf = cctx.enter_context(tc.tile_pool(name="crf", bufs=1))
            ctmp = cctx.enter_context(tc.tile_pool(name="ctmp", bufs=4))

            def ct(name, shape, dtype=F32):
                return crf.tile(shape, dtype, tag=name, name=name)

            Mexp = ct("Mexp", [K, K])
            nc.sync.dma_start(Mexp[:], mexp_d[:])
            expStart = ct("expStart", [K, 1])
            expEnd = ct("expEnd", [K, 1])
            nc.sync.dma_start(expStart[:], expstart_d[:])
            nc.sync.dma_start(expEnd[:], expend_d[:])
            oh9w = ct("oh9w", [K, T])
            nc.sync.dma_start(oh9w[:], oh9w_d[:])

            # gold-emission dot product: num_dev = sum(emT * oh9w)
            sink9 = ct("sink9", [K, T])
            accK = ct("accK", [K, 1])
            nc.vector.scalar_tensor_tensor(
                out=sink9[:], in0=emT[:], scalar=1.0, in1=oh9w[:],
                op0=OP.mult, op1=OP.mult, accum_out=accK[:])

            # ---- linear-space scan ----
            expEm4 = expEm[:].rearrange("k (b s) -> k b s", b=BC)
            if mask_ones:
                # Chunked scan: alpha_255 = D_255 G_15...G_0 (M^T alpha_0)
                # with B_t = M^T D_t and G_c = B_{16c+16}...B_{16c+1}
                # (G_15 ends at B_254).  The 16 chunk factors G_c^T are built
                # simultaneously, 15 batched rounds of one matmul + one
                # row-scale over all (example, chunk) blocks:
                #   Pt <- D_t (M @ Pt),  t descending within each chunk.
                # The sequential fold is then only 16 steps deep per example.
                CH, CL = 16, 16
                # em_rep[k, t, j] = expEm[k, t]  (j-broadcast via 9 copies,
                # split ACT/DVE; both engines' copies avoid table reloads)
                em_rep = ct("em_rep", [K, T, K])
                srcEm = expEm[:].rearrange("k (t o) -> k t o", o=1)
                for j in range(K):
                    if j % 2 == 0:
                        nc.vector.tensor_copy(em_rep[:, :, j:j + 1], srcEm)
                    else:
                        nc.scalar.copy(em_rep[:, :, j:j + 1], srcEm)
                emr = em_rep[:].rearrange("k (b c s) j -> k b c s j",
                                          b=BC, c=CH)
                mexptS = ct("mexptS", [K, K])
                nc.sync.dma_start(mexptS[:], dd["mexpt"][:])
                mrepS = ct("mrepS", [K, BC, CH, K])
                nc.sync.dma_start(mrepS[:], dd["mrep"][:])
                PtS = ct("PtS", [K, BC, CH, K])
                # init chunks 0..14 at t=16(c+1); chunk 15 at t=254
                nc.vector.tensor_tensor(
                    out=PtS[:, :, 0:CH - 1, :], in0=mrepS[:, :, 0:CH - 1, :],
                    in1=emr[:, :, 1:CH, 0, :], op=OP.mult)
                nc.vector.tensor_tensor(
                    out=PtS[:, :, CH - 1, :], in0=mrepS[:, :, CH - 1, :],
                    in1=emr[:, :, CH - 1, 14, :], op=OP.mult)
                for r in range(1, CL):
                    cmax = CH - 1 if r <= 2 else CH
                    for h in range(2):
                        pu = ps.tile([P, 512], F32, tag="ps", name="pu",
                                     space="PSUM")
                        nc.tensor.matmul(
                            pu[:K, :2 * cmax * K], lhsT=mexptS[:],
                            rhs=PtS[:, 2 * h:2 * h + 2, 0:cmax, :],
                            start=True, stop=True)
                        nc.vector.tensor_tensor(
                            out=PtS[:, 2 * h:2 * h + 2, 0:cmax, :],
                            in0=pu[:K, :2 * cmax * K].rearrange(
                                "k (b c j) -> k b c j", b=2, c=cmax),
                            in1=emr[:, 2 * h:2 * h + 2, 0:cmax, CL - r, :],
                            op=OP.mult)
                # fold: beta0 = M^T @ (expStart * em_0), then 16 steps/example
                a0 = ctmp.tile([K, BC], F32, tag="a0", name="a0")
                nc.vector.tensor_scalar(
                    out=a0[:], in0=expEm4[:, :, 0],
                    scalar1=expStart[:, :1], scalar2=None, op0=OP.mult)
                endem = ctmp.tile([K, BC], F32, tag="endem", name="endem")
                nc.vector.tensor_scalar(
                    out=endem[:], in0=expEm4[:, :, S - 1],
                    scalar1=expEnd[:, :1], scalar2=None, op0=OP.mult)
                pb0 = ps.tile([P, 512], F32, tag="ps", name="pb0",
                              space="PSUM")
                nc.tensor.matmul(pb0[:K, :BC], lhsT=Mexp[:], rhs=a0[:],
                                 start=True, stop=True)
                gams = []
                for b in range(BC):
                    g0 = ctmp.tile([K, 1], F32, tag=f"g{b}", name=f"g0_{b}")
                    nc.vector.tensor_copy(g0[:], pb0[:K, b:b + 1])
                    gams.append(g0)
                F_ = ctmp.tile([K, BC], F32, tag="F", name="F_")
                for c in range(CH):
                    for b in range(BC):
                        pg = ps.tile([P, 512], F32, tag="ps", name="pg",
                                     space="PSUM")
                        nc.tensor.matmul(pg[:K, :1], lhsT=PtS[:, b, c, :],
                                         rhs=gams[b][:], start=True,
                                         stop=True)
                        if c == CH - 1:
                            nc.vector.tensor_tensor(
                                out=F_[:, b:b + 1], in0=pg[:K, :1],
                                in1=endem[:, b:b + 1], op=OP.mult)
                        else:
                            gn = ctmp.tile([K, 1], F32, tag=f"g{b}",
                                           name=f"g{c}_{b}")
                            nc.vector.tensor_copy(gn[:], pg[:K, :1])
                            gams[b] = gn
            else:
                mrow_i = ct("mrow_i", [1, T], I32)
                nc.sync.dma_start(mrow_i[:], dd["maskrow"][:])
                mrow = ct("mrow", [1, T])
                nc.vector.tensor_copy(mrow[:], mrow_i[:])
                inv9 = ct("inv9", [K, T])
                mb9 = ct("mb9", [K, T])
                for i in range(2):
                    pb = ps.tile([P, 512], F32, tag="ps", name="pmb",
                                 space="PSUM")
                    nc.tensor.matmul(pb[:K, :], lhsT=onescol_f[:1, :K],
                                     rhs=mrow[:, i * 512:(i + 1) * 512],
                                     start=True, stop=True)
                    nc.scalar.activation(mb9[:, i * 512:(i + 1) * 512],
                                         pb[:K, :], AF.Identity)
                nc.vector.tensor_scalar(out=inv9[:], in0=mb9[:], scalar1=0.0,
                                        scalar2=None, op0=OP.is_equal)
                inv4 = inv9[:].rearrange("k (b s) -> k b s", b=BC)
                # two independent 2-example chains
                Ecurs = []
                for hf in range(2):
                    Ec = ctmp.tile([K, 2], F32, tag=f"E{hf}", name=f"E0_{hf}")
                    nc.vector.tensor_scalar(
                        out=Ec[:], in0=expEm4[:, 2 * hf:2 * hf + 2, 0],
                        scalar1=expStart[:, :1], scalar2=None, op0=OP.mult)
                    Ecurs.append(Ec)
                for t in range(1, S):
                    for hf in range(2):
                        psn = ps.tile([P, 512], F32, tag="ps", name="pcrf",
                                      space="PSUM")
                        nc.tensor.matmul(psn[:K, :2], lhsT=Mexp[:],
                                         rhs=Ecurs[hf][:],
                                         start=True, stop=True)
                        Enew = ctmp.tile([K, 2], F32, tag=f"E{hf}",
                                         name=f"E{t}_{hf}")
                        nc.vector.tensor_tensor(
                            out=Enew[:], in0=psn[:K, :2],
                            in1=expEm4[:, 2 * hf:2 * hf + 2, t], op=OP.mult)
                        nc.vector.copy_predicated(
                            Enew[:], inv4[:, 2 * hf:2 * hf + 2, t],
                            Ecurs[hf][:])
                        Ecurs[hf] = Enew

                F_ = ctmp.tile([K, BC], F32, tag="F", name="F_")
                for hf in range(2):
                    nc.vector.tensor_scalar(
                        out=F_[:, 2 * hf:2 * hf + 2], in0=Ecurs[hf][:],
                        scalar1=expEnd[:, :1], scalar2=None, op0=OP.mult)
            psd = ps.tile([P, 512], F32, tag="ps", name="psd", space="PSUM")
            nc.tensor.matmul(psd[:1, :BC], lhsT=onescol_f[:K, :], rhs=F_[:],
                             start=True, stop=True)
            denomv = ctmp.tile([1, BC], F32, tag="denomv", name="denomv")
            denom_tot = ct("denom_tot", [1, 1])
            nc.scalar.activation(denomv[:], psd[:1, :BC], AF.Ln,
                                 accum_out=denom_tot[:])

            psn2 = ps.tile([P, 512], F32, tag="ps", name="psn2", space="PSUM")
            nc.tensor.matmul(psn2[:1, :1], lhsT=onescol_f[:K, :],
                             rhs=accK[:], start=True, stop=True)
            num_tot = ct("num_tot", [1, 1])
            nc.vector.tensor_copy(num_tot[:], psn2[:1, :1])
            out_sb = ct("out_sb", [1, 4])
            nc.vector.memset(out_sb[:], 0.0)
            nc.vector.tensor_tensor(out=out_sb[:, 0:1], in0=denom_tot[:],
                                    in1=num_tot[:], op=OP.subtract)
            nc.vector.tensor_copy(out_sb[:, 1:2], num_tot[:])
            nc.vector.tensor_copy(out_sb[:, 2:3], denom_tot[:])
            nc.sync.dma_start(out_d[:], out_sb[:])


# ---------------------------------------------------------------------------
# host wrapper
# ---------------------------------------------------------------------------

_NC_CACHE = {}


def _get_nc(key):
    if key not in _NC_CACHE:
        _NC_CACHE[key] = build_nc(*key)
    return _NC_CACHE[key]


def prepare_maps(inputs, mask_ones, zero_bias, unit_ln):
    """Returns (in_maps, label_const): per-core device inputs and the
    host-computed label-only CRF numerator sum over the whole batch."""
    input_ids = np.asarray(inputs["input_ids"]).astype(np.int32)
    attention_mask = np.asarray(inputs["attention_mask"]).astype(np.int32)
    labels = np.asarray(inputs["labels"]).astype(np.int64)

    word = _bf(inputs["word_emb"])
    pt = _bf((_f32(inputs["pos_emb"][:S]) + _f32(inputs["type_emb"][0])[None, :])
             .reshape(2, P, H).transpose(1, 0, 2))
    wq = _f8(inputs["Wq"], WS).reshape(L, 3, 2, P, H).transpose(
        0, 3, 1, 2, 4).copy()
    wk = _f8(inputs["Wk"], WS).reshape(L, 3, 2, P, H).transpose(
        0, 3, 1, 2, 4).copy()
    wv = _f8(inputs["Wv"], WS).reshape(L, 3, 2, P, H).transpose(
        0, 3, 1, 2, 4).copy()
    wo = _f8(inputs["Wo"], WS).reshape(L, 3, 2, P, H).transpose(
        0, 3, 1, 2, 4).copy()
    w1 = (_f8(inputs["W1"], WS).reshape(L, 3, 2, P, FC, P)
          .transpose(0, 4, 3, 1, 2, 5).copy())
    w2 = (_f8(inputs["W2"], WS).reshape(L, FC // 2, 2, P, H)
          .transpose(0, 1, 3, 2, 4).copy())
    cwpad = np.zeros((H, 16), np.float32)
    cwpad[:, :K] = _f32(inputs["cls_W"])
    clsw = _f8(cwpad, WS).reshape(3, 2, P, 16).transpose(2, 0, 1, 3).copy()
    clsb = (_f32(inputs["cls_b"]) - np.float32(C_SHIFT)).reshape(K, 1)

    trans = _f32(inputs["crf_trans"]).reshape(K, K)
    startv = _f32(inputs["crf_start"]).reshape(K)
    endv = _f32(inputs["crf_end"]).reshape(K)

    shared = dict(
        word=word, pt=pt, wq=wq, wk=wk, wv=wv, wo=wo, w1=w1, w2=w2,
        clsw=clsw, clsb=clsb,
        mexp=np.exp(trans).astype(np.float32),
        mexpt=np.ascontiguousarray(np.exp(trans).T.astype(np.float32)),
        mrep=np.ascontiguousarray(np.broadcast_to(
            np.exp(trans).astype(np.float32)[:, None, None, :],
            (K, BC, 16, K))),
        expstart=np.exp(startv).astype(np.float32).reshape(K, 1),
        expend=np.exp(endv).astype(np.float32).reshape(K, 1),
    )
    if not zero_bias:
        shared.update(
            bq=_f32(inputs["bq"]).reshape(L, HC, P).transpose(0, 2, 1).copy(),
            bk=_f32(inputs["bk"]).reshape(L, HC, P).transpose(0, 2, 1).copy(),
            b1=_f32(inputs["b1"]).reshape(L, FC, P).transpose(0, 2, 1).copy(),
            bvrow=_bf(_f32(inputs["bv"]) / DQ_WX).reshape(L, 1, H),
            borow=_bf(_f32(inputs["bo"]) / DQ_WX).reshape(L, 1, H),
            b2row=_bf(_f32(inputs["b2"]) / DQ_W).reshape(L, 1, H),
        )
    if not unit_ln:
        shared.update(
            lng=np.stack([_bf(inputs["ln1_g"]), _bf(inputs["ln2_g"])],
                         axis=1).reshape(L, 2, 1, H),
            lnb=np.stack([_bf(inputs["ln1_b"]), _bf(inputs["ln2_b"])],
                         axis=1).reshape(L, 2, 1, H),
            elng=np.stack([_bf(inputs["emb_ln_g"]), _bf(inputs["emb_ln_b"])],
                          axis=0).reshape(2, 1, H),
        )

    # ---- host label-only numerator + per-core oh9w ----
    mf = attention_mask.astype(np.float32)               # [B, S]
    w9 = mf.copy()
    w9[:, 0] = 1.0                                       # t=0 emission always counted
    trans_gold = trans[labels[:, :-1], labels[:, 1:]]    # [B, S-1]
    last_idx = attention_mask.astype(np.int64).sum(axis=1) - 1
    label_num = (startv[labels[:, 0]]
                 + (trans_gold * mf[:, 1:]).sum(axis=1)
                 + endv[labels[np.arange(B), last_idx]])  # [B]
    label_const = float(np.float32(label_num.astype(np.float32).sum()))

    in_maps = []
    for c in range(CORES):
        ids_c = input_ids[BC * c:BC * (c + 1)].reshape(NT, P, 1).copy()
        lab_c = labels[BC * c:BC * (c + 1)]              # [BC, S]
        w9_c = w9[BC * c:BC * (c + 1)]                   # [BC, S]
        oh = np.zeros((K, BC, S), np.float32)
        oh[lab_c.reshape(-1), np.repeat(np.arange(BC), S),
           np.tile(np.arange(S), BC)] = w9_c.reshape(-1)
        msk_c = attention_mask[BC * c:BC * (c + 1)]
        m = dict(shared)
        m["ids"] = ids_c
        m["oh9w"] = oh.reshape(K, T).copy()
        if not mask_ones:
            m["maskrow"] = msk_c.reshape(1, T).copy()
            m["maskcols"] = (msk_c.reshape(BC, 2, P).transpose(2, 1, 0)
                             .astype(np.int32).copy())
        in_maps.append(m)
    return in_maps, label_const


def kernel(**inputs) -> np.ndarray:
    attention_mask = np.asarray(inputs["attention_mask"])
    assert np.asarray(inputs["input_ids"]).shape == (B, S)

    mask_ones = bool(np.all(attention_mask == 1))
    zero_bias = all(
        not np.any(np.asarray(inputs[k]))
        for k in ("bq", "bk", "bv", "bo", "b1", "b2"))
    unit_ln = (all(np.all(np.asarray(inputs[k]) == 1.0)
                   for k in ("emb_ln_g", "ln1_g", "ln2_g"))
               and all(not np.any(np.asarray(inputs[k]))
                       for k in ("emb_ln_b", "ln1_b", "ln2_b")))

    n_layers = int(os.environ.get("BERTCRF_LAYERS", L))
    debug = bool(int(os.environ.get("BERTCRF_DEBUG", "0")))
    nc = _get_nc((n_layers, mask_ones, zero_bias, unit_ln, debug))
    in_maps, label_const = prepare_maps(inputs, mask_ones, zero_bias, unit_ln)

    res = run_bass_kernel_spmd(nc, in_maps, core_ids=list(range(CORES)))
    total = np.float32(0.0)
    for c in range(CORES):
        total += np.float32(res.results[c]["out"][0, 0])
    return np.float32(total - np.float32(label_const))


if __name__ == "__main__":
    import jax
    jax.config.update("jax_platforms", "cpu")
    import reference
    inp = reference.setup_inputs()
    outv = kernel(**{k: np.asarray(v) for k, v in inp.items()})
    print("kernel:", outv)


# revision 41
# speedup vs baseline: 1.1776x; 1.0201x over previous
"""BertCRF forward (BERT-base encoder + CRF NLL) on 8 Trainium2 NeuronCores.

Strategy: data-parallel over the batch (32 examples -> 4 per core), params
replicated.  Each core runs the full 12-layer encoder on its 1024 tokens with
fp8 (DoubleRow) matmuls for the dense projections, bf16 attention, fp32
layernorm, and a max-free softmax whose normalizer is computed by an extra
ones-matmul on the PE and inverted with a fast DVE reciprocal.  The CRF
numerator's label-only terms (start/transition/end) are computed on the host;
the device computes the gold-emission dot product and the exact linear-space
forward scan with a fixed per-step shift (cancels exactly between numerator
and denominator).  The host shards inputs, pre-arranges weight layouts, and
sums the 8 per-core partial NLLs plus the host-side label constant.
"""

import contextlib
import os

import numpy as np
import ml_dtypes

import concourse.bass as bass  # noqa: F401
import concourse.mybir as mybir
import concourse.tile as tile
from concourse import bacc
from concourse.bass import IndirectOffsetOnAxis
from concourse.bass_utils import run_bass_kernel_spmd
from concourse.masks import make_identity

# ---- problem constants (hardcoded per the task spec) ----
L, H, NH, DH, FF, V, K = 12, 768, 12, 64, 3072, 30522, 9
B, S = 32, 256
CORES = 8
BC = B // CORES          # 4 examples per core
T = BC * S               # 1024 tokens per core
P = 128
NT = T // P              # 8 token tiles
HC = H // P              # 6 hidden chunks
FC = FF // P             # 24 ff chunks
C_SHIFT = 2.35           # per-step CRF shift (cancels exactly in num-denom)

F32 = mybir.dt.float32
BF16 = mybir.dt.bfloat16
FP8 = mybir.dt.float8e4
I32 = mybir.dt.int32
AX = mybir.AxisListType
OP = mybir.AluOpType
AF = mybir.ActivationFunctionType
DR = mybir.MatmulPerfMode.DoubleRow

BF = ml_dtypes.bfloat16

# fp8 quantization scales (exact powers of two)
WS = 1024.0              # weight scale into fp8e4
XS = 32.0                # activation scale into fp8e4
DQ_WX = 2.0 ** -15       # dequant for w*x products
DQ_W = 2.0 ** -10        # dequant when only the weight was scaled


def _bf(x):
    return np.ascontiguousarray(np.asarray(x, dtype=np.float32)).astype(BF)


def _f8(x, scale):
    return np.ascontiguousarray(np.clip(
        np.asarray(x, dtype=np.float32) * scale, -240.0, 240.0)
    ).astype(ml_dtypes.float8_e4m3)


def _f32(x):
    return np.ascontiguousarray(np.asarray(x, dtype=np.float32))


# ---------------------------------------------------------------------------
# device program
# ---------------------------------------------------------------------------

def _layernorm(nc, tmp, out_bf, xf, gb, s1=None, sq_act=True):
    """LN over the free dim of xf [P, H] f32 -> out_bf (bf16).

    s1, if given, is a [P, 1] tile already holding sum(xf) (computed for free
    via accum_out on the op that produced xf).  sq_act picks the engine for
    the sum-of-squares pass: ACT (Square is in every table set) when ACT has
    headroom, DVE when ACT is the busier engine (attention phase).
    """
    if s1 is None:
        s1 = tmp.tile([P, 1], F32, tag="s1", name="s1")
        nc.vector.tensor_reduce(out=s1[:], in_=xf[:], axis=AX.X, op=OP.add)
    sq = tmp.tile([P, H], F32, tag="sq", name="sq")
    s2 = tmp.tile([P, 1], F32, tag="s2", name="s2")
    if sq_act:
        nc.scalar.activation(sq[:], xf[:], AF.Square, accum_out=s2[:])
    else:
        nc.vector.scalar_tensor_tensor(out=sq[:], in0=xf[:], scalar=1.0,
                                       in1=xf[:], op0=OP.mult, op1=OP.mult,
                                       accum_out=s2[:])
    m = tmp.tile([P, 1], F32, tag="m", name="m")
    nc.vector.tensor_scalar(out=m[:], in0=s1[:], scalar1=1.0 / H, scalar2=None,
                            op0=OP.mult)
    msq = tmp.tile([P, 1], F32, tag="msq", name="msq")
    nc.vector.tensor_tensor(out=msq[:], in0=m[:], in1=m[:], op=OP.mult)
    var = tmp.tile([P, 1], F32, tag="var", name="var")
    nc.vector.tensor_scalar(out=var[:], in0=s2[:], scalar1=1.0 / H,
                            scalar2=msq[:, :1], op0=OP.mult, op1=OP.subtract)
    # eps=1e-12 is below f32 resolution for var~O(1); bias=0.0 is identical
    sd = tmp.tile([P, 1], F32, tag="sd", name="sd")
    nc.scalar.activation(sd[:], var[:], AF.Sqrt, bias=0.0)
    rs = tmp.tile([P, 1], F32, tag="rs", name="rs")
    nc.vector.reciprocal_approx_fast(rs[:], sd[:])
    if gb is None:
        # out = rs*x - m*rs, one half on DVE, one half on ACT (in parallel)
        nmrs = tmp.tile([P, 1], F32, tag="nmrs", name="nmrs")
        nc.vector.tensor_scalar(out=nmrs[:], in0=m[:], scalar1=-1.0,
                                scalar2=rs[:, :1], op0=OP.mult, op1=OP.mult)
        HH = H // 2
        nc.vector.tensor_scalar(out=out_bf[:, 0:HH], in0=xf[:, 0:HH],
                                scalar1=m[:, :1], scalar2=rs[:, :1],
                                op0=OP.subtract, op1=OP.mult)
        nc.scalar.activation(out_bf[:, HH:H], xf[:, HH:H], AF.Identity,
                             bias=nmrs[:, :1], scale=rs[:, :1])
    else:
        G, Bb = gb
        t2 = tmp.tile([P, H], F32, tag="t2", name="t2")
        nc.vector.tensor_scalar(out=t2[:], in0=xf[:], scalar1=m[:, :1],
                                scalar2=rs[:, :1], op0=OP.subtract, op1=OP.mult)
        t3 = tmp.tile([P, H], F32, tag="t3", name="t3")
        nc.vector.tensor_tensor(out=t3[:], in0=t2[:], in1=G[:], op=OP.mult)
        nc.vector.tensor_tensor(out=out_bf, in0=t3[:], in1=Bb[:], op=OP.add)


def _transpose_tiles(nc, ps, xT, x_sb, ident, tiles):
    """x_sb [P, NT, H] token-major -> xT [P, HC, T] feature-major, per tile."""
    for g in tiles:
        for cg in range(2):          # chunk groups of 3
            pt_ = ps.tile([P, 1024], BF16, tag="ps", name="ptp", space="PSUM")
            for ci in range(3):
                c = cg * 3 + ci
                nc.tensor.matmul(
                    pt_[:, ci * P:(ci + 1) * P],
                    lhsT=x_sb[:, g, c * P:(c + 1) * P], rhs=ident[:],
                    start=True, stop=True, is_transpose=True)
            nc.scalar.activation(
                xT[:, cg * 3:cg * 3 + 3, g * P:(g + 1) * P],
                pt_[:, :384], AF.Identity, scale=XS)


def _bcast_row(nc, ps, tmp, dst, row_dram, ones_bf):
    """dst [P, H] bf16 = broadcast of a [1, H] bf16 dram row across partitions."""
    row = tmp.tile([1, H], BF16, tag="brow", name="brow")
    nc.sync.dma_start(row[:], row_dram)
    for fh in range(2):
        pb = ps.tile([P, 512], F32, tag="ps", name="pbc", space="PSUM")
        nc.tensor.matmul(pb[:, :384], lhsT=ones_bf[:, :P],
                         rhs=row[:, fh * 384:(fh + 1) * 384],
                         start=True, stop=True)
        nc.scalar.activation(dst[:, fh * 384:(fh + 1) * 384], pb[:, :384],
                             AF.Identity)


def build_nc(n_layers=L, mask_ones=True, zero_bias=True, unit_ln=True,
             debug=False):
    nc = bacc.Bacc("TRN2", target_bir_lowering=False, debug=False)

    dd = {}

    def din(name, shape, dtype):
        dd[name] = nc.dram_tensor(name, list(shape), dtype, kind="ExternalInput")
        return dd[name]

    def dout(name, shape, dtype):
        dd[name] = nc.dram_tensor(name, list(shape), dtype, kind="ExternalOutput")
        return dd[name]

    din("word", [V, H], BF16)
    din("ids", [NT, P, 1], I32)
    din("pt", [P, 2, H], BF16)
    din("wq", [L, P, 3, 2, H], FP8)
    din("wk", [L, P, 3, 2, H], FP8)
    din("wv", [L, P, 3, 2, H], FP8)
    din("wo", [L, P, 3, 2, H], FP8)
    din("w1", [L, FC, P, 3, 2, P], FP8)   # [l, j, ki, c2, ko, m]
    din("w2", [L, FC // 2, P, 2, H], FP8)  # [l, c2, ki, ko, n]
    din("clsw", [P, 3, 2, 16], FP8)      # K padded to 16
    din("clsb", [K, 1], F32)             # already shifted by -C_SHIFT
    din("mexp", [K, K], F32)             # exp(crf_trans)
    din("mexpt", [K, K], F32)            # exp(crf_trans).T
    din("mrep", [K, BC, 16, K], F32)     # exp(crf_trans) replicated 64x
    din("expstart", [K, 1], F32)         # exp(crf_start)
    din("expend", [K, 1], F32)           # exp(crf_end)
    din("oh9w", [K, T], F32)             # one-hot(labels) * emission weight
    if not mask_ones:
        din("maskrow", [1, T], I32)
        din("maskcols", [P, 2, BC], I32)
    if not zero_bias:
        din("bq", [L, P, HC], F32)
        din("bk", [L, P, HC], F32)
        din("b1", [L, P, FC], F32)
        din("bvrow", [L, 1, H], BF16)
        din("borow", [L, 1, H], BF16)
        din("b2row", [L, 1, H], BF16)
    if not unit_ln:
        din("lng", [L, 2, 1, H], BF16)
        din("lnb", [L, 2, 1, H], BF16)
        din("elng", [2, 1, H], BF16)

    dout("out", [1, 4], F32)
    if debug:
        dout("dbg_x0", [P, NT, H], BF16)
        dout("dbg_x", [P, NT, H], BF16)
        dout("dbg_em", [K, T], F32)

    _build_body(nc, dd, n_layers, mask_ones, zero_bias, unit_ln, debug)
    nc.compile()
    return nc


def _build_body(nc, dd, n_layers, mask_ones, zero_bias, unit_ln, debug):
    (word, ids, pt, wq_d, wk_d, wv_d, wo_d, w1_d, w2_d, clsw_d, clsb_d,
     mexp_d, expstart_d, expend_d, oh9w_d, out_d) = (
        dd["word"], dd["ids"], dd["pt"], dd["wq"], dd["wk"], dd["wv"],
        dd["wo"], dd["w1"], dd["w2"], dd["clsw"], dd["clsb"], dd["mexp"],
        dd["expstart"], dd["expend"], dd["oh9w"], dd["out"])
    if not mask_ones:
        maskrow_d = dd["maskrow"]
        maskcols_d = dd["maskcols"]
    if not zero_bias:
        bq_d, bk_d, b1_d = dd["bq"], dd["bk"], dd["b1"]
        bvrow_d, borow_d, b2row_d = dd["bvrow"], dd["borow"], dd["b2row"]
    if not unit_ln:
        lng_d, lnb_d, elng_d = dd["lng"], dd["lnb"], dd["elng"]
    with tile.TileContext(nc) as tc, contextlib.ExitStack() as octx:
        cst = octx.enter_context(tc.tile_pool(name="cst", bufs=1))
        act = octx.enter_context(tc.tile_pool(name="act", bufs=1))
        ps = octx.enter_context(tc.tile_pool(name="ps", bufs=8, space="PSUM"))

        # ---- persistent activation buffers ----
        x_sb = act.tile([P, NT, H], BF16, tag="x_sb", name="x_sb")
        xT = act.tile([P, HC, T], FP8, tag="xT", name="xT")
        qT = act.tile([P, HC, T], BF16, tag="qT", name="qT")
        kT = act.tile([P, HC, T], BF16, tag="kT", name="kT")
        vS = act.tile([P, NT, NH, DH], BF16, tag="vS", name="vS")
        cT = act.tile([P, HC, T], FP8, tag="cT", name="cT")
        hT = act.tile([P, FC, T // 2], FP8, tag="hT", name="hT")
        emT = act.tile([K, T], F32, tag="emT", name="emT")
        expEm = act.tile([K, T], F32, tag="expEm", name="expEm")

        # ---- constants ----
        ident = cst.tile([P, P], BF16, tag="ident", name="ident")
        make_identity(nc, ident[:])
        ones_bf = cst.tile([1, P], BF16, tag="ones_bf", name="ones_bf")
        nc.vector.memset(ones_bf[:], 1.0)
        # ones64: [128, 64] all-ones lhsT; sum over keys of exp(scores) into
        # one 64-partition half of the normalizer PSUM tile per head
        ones64 = cst.tile([P, DH], BF16, tag="ones64", name="ones64")
        nc.vector.memset(ones64[:], 1.0)
        onescol_f = cst.tile([P, 1], F32, tag="onescol_f", name="onescol_f")
        nc.vector.memset(onescol_f[:], 1.0)

        pt_sb = cst.tile([P, 2, H], BF16, tag="pt_sb", name="pt_sb")
        nc.sync.dma_start(pt_sb[:], pt[:])
        if not mask_ones:
            mcol = cst.tile([P, 2, BC], F32, tag="mcol", name="mcol")
        if not unit_ln:
            elnG = cst.tile([P, H], BF16, tag="elnG", name="elnG")
            elnB = cst.tile([P, H], BF16, tag="elnB", name="elnB")

        with contextlib.ExitStack() as ictx:
            wts = ictx.enter_context(tc.tile_pool(name="wts", bufs=1))
            tmp = ictx.enter_context(tc.tile_pool(name="tmp", bufs=3))

            if not unit_ln:
                _bcast_row(nc, ps, tmp, elnG, elng_d[0], ones_bf)
                _bcast_row(nc, ps, tmp, elnB, elng_d[1], ones_bf)

            # =========== embeddings ===========
            for g in range(NT):
                idx = tmp.tile([P, 1], I32, tag="idx", name="idx")
                nc.sync.dma_start(idx[:], ids[g])
                emb = tmp.tile([P, H], BF16, tag="emb", name="emb")
                nc.gpsimd.indirect_dma_start(
                    out=emb[:], out_offset=None, in_=word[:],
                    in_offset=IndirectOffsetOnAxis(ap=idx[:, :1], axis=0),
                )
                xf = tmp.tile([P, H], F32, tag="xf", name="xf")
                s1e = tmp.tile([P, 1], F32, tag="s1e", name="s1e")
                nc.vector.scalar_tensor_tensor(
                    out=xf[:], in0=emb[:], scalar=0.0,
                    in1=pt_sb[:, g % 2, :], op0=OP.add, op1=OP.add,
                    accum_out=s1e[:])
                _layernorm(nc, tmp, x_sb[:, g, :], xf,
                           None if unit_ln else (elnG, elnB), s1=s1e)
            if debug:
                nc.sync.dma_start(dd["dbg_x0"][:], x_sb[:])
            # prime xT tiles 0-3 for layer 0's QK proj t2=0
            _transpose_tiles(nc, ps, xT, x_sb, ident, range(4))

            if not mask_ones:
                mi = tmp.tile([P, 2, BC], I32, tag="mi", name="mi")
                nc.sync.dma_start(mi[:], maskcols_d[:])
                nc.vector.tensor_scalar(out=mcol[:], in0=mi[:], scalar1=1.0,
                                        scalar2=10000.0, op0=OP.subtract,
                                        op1=OP.mult)

            # =========== encoder layers ===========
            for l in range(n_layers):
                wq = wts.tile([P, 3, 2, H], FP8, tag="wq", name="wq")
                wk = wts.tile([P, 3, 2, H], FP8, tag="wk", name="wk")
                wv = wts.tile([P, 3, 2, H], FP8, tag="wv", name="wv")
                wo = wts.tile([P, 3, 2, H], FP8, tag="wo", name="wo")
                nc.sync.dma_start(wq[:], wq_d[l])
                nc.sync.dma_start(wk[:], wk_d[l])
                nc.sync.dma_start(wv[:], wv_d[l])
                nc.sync.dma_start(wo[:], wo_d[l])
                # preload the whole layer's W1 once (18.4KB/partition): the
                # 24 transfers stream during attention, FFN1 reads them from
                # SBUF in both token-half passes — halves W1 HBM traffic and
                # removes the just-in-time DMA stalls inside the FFN1 loop
                w1js = []
                for j in range(FC):
                    w1j = wts.tile([P, 3, 2, P], FP8, tag="w1j",
                                   name=f"w1j{j}", bufs=FC)
                    nc.sync.dma_start(w1j[:], w1_d[l, j])
                    w1js.append(w1j)

                if not zero_bias:
                    bq_sb = wts.tile([P, HC], F32, tag="bq", name="bq")
                    bk_sb = wts.tile([P, HC], F32, tag="bk", name="bk")
                    b1_sb = wts.tile([P, FC], F32, tag="b1", name="b1")
                    nc.sync.dma_start(bq_sb[:], bq_d[l])
                    nc.sync.dma_start(bk_sb[:], bk_d[l])
                    nc.sync.dma_start(b1_sb[:], b1_d[l])
                    bvrow = wts.tile([1, H], BF16, tag="bvrow", name="bvrow")
                    borow = wts.tile([1, H], BF16, tag="borow", name="borow")
                    b2row = wts.tile([1, H], BF16, tag="b2row", name="b2row")
                    nc.sync.dma_start(bvrow[:], bvrow_d[l])
                    nc.sync.dma_start(borow[:], borow_d[l])
                    nc.sync.dma_start(b2row[:], b2row_d[l])
                if not unit_ln:
                    G1 = wts.tile([P, H], BF16, tag="G1", name="G1")
                    B1t = wts.tile([P, H], BF16, tag="B1t", name="B1t")
                    G2 = wts.tile([P, H], BF16, tag="G2", name="G2")
                    B2t = wts.tile([P, H], BF16, tag="B2t", name="B2t")
                    _bcast_row(nc, ps, tmp, G1, lng_d[l, 0], ones_bf)
                    _bcast_row(nc, ps, tmp, B1t, lnb_d[l, 0], ones_bf)
                    _bcast_row(nc, ps, tmp, G2, lng_d[l, 1], ones_bf)
                    _bcast_row(nc, ps, tmp, B2t, lnb_d[l, 1], ones_bf)

                # ---- qT/kT projections, token-half pipelined.  xT tiles 0-3
                #      were already transposed in the previous layer's FFN
                #      tail (or right after the embeddings for layer 0), so
                #      QK proj t2=0 can start while the previous layer's
                #      second-half LN2 chain is still draining; tiles 4-7 are
                #      transposed here in between. ----
                def _v_proj(gs):
                    for g in gs:
                        for fh in range(2):
                            pm = ps.tile([P, 512], F32, tag="ps", name="pv",
                                         space="PSUM")
                            for c2 in range(3):
                                nc.tensor.matmul(
                                    pm[:, :384],
                                    lhsT=xT[:, 2 * c2:2 * c2 + 2,
                                            g * P:(g + 1) * P],
                                    rhs=wv[:, c2, :, fh * 384:(fh + 1) * 384],
                                    start=(c2 == 0),
                                    stop=(c2 == 2 and zero_bias),
                                    perf_mode=DR)
                            if not zero_bias:
                                nc.tensor.matmul(
                                    pm[:, :384], lhsT=ones_bf[:, :P],
                                    rhs=bvrow[:, fh * 384:(fh + 1) * 384],
                                    start=False, stop=True)
                            nc.vector.tensor_scalar(
                                out=vS[:, g, 6 * fh:6 * fh + 6, :],
                                in0=pm[:, :384], scalar1=DQ_WX, scalar2=None,
                                op0=OP.mult)

                for t2 in range(2):
                    if t2 == 1:
                        # V proj of tiles 0-3 also only needs xT 0-3 + wv:
                        # more PE work that is independent of the previous
                        # layer's LN2-th1 chain, emitted before the
                        # transposes that must wait for it
                        _v_proj(range(4))
                        _transpose_tiles(nc, ps, xT, x_sb, ident, range(4, 8))
                    for wmat, bname, dst in ((wq, "bq", qT), (wk, "bk", kT)):
                        for f in range(HC):
                            pm = ps.tile([P, 512], F32, tag="ps", name="pqk",
                                         space="PSUM")
                            for c2 in range(3):
                                nc.tensor.matmul(
                                    pm[:],
                                    lhsT=wmat[:, c2, :, f * P:(f + 1) * P],
                                    rhs=xT[:, 2 * c2:2 * c2 + 2,
                                           t2 * 512:(t2 + 1) * 512],
                                    start=(c2 == 0), stop=(c2 == 2),
                                    perf_mode=DR)
                            if zero_bias:
                                nc.vector.tensor_scalar(
                                    out=dst[:, f, t2 * 512:(t2 + 1) * 512],
                                    in0=pm[:], scalar1=DQ_WX, scalar2=None,
                                    op0=OP.mult)
                            else:
                                bias = (bq_sb if bname == "bq"
                                        else bk_sb)[:, f:f + 1]
                                nc.scalar.activation(
                                    dst[:, f, t2 * 512:(t2 + 1) * 512], pm[:],
                                    AF.Identity, bias=bias, scale=DQ_WX)

                # ---- V projection for tiles 4-7 (0-3 done above) ----
                _v_proj(range(4, NT))

                # ---- output proj + residual + LN1 (interleaved
                #      into the attention loop, per example) ----
                def _oproj_ln1(g):
                    xf = tmp.tile([P, H], F32, tag="xf", name="xf")
                    s1a = tmp.tile([P, 1], F32, tag="s1a", name="s1a")
                    s1b = tmp.tile([P, 1], F32, tag="s1b", name="s1b")
                    for fh in range(2):
                        pm = ps.tile([P, 512], F32, tag="ps", name="po",
                                     space="PSUM")
                        for c2 in range(3):
                            nc.tensor.matmul(
                                pm[:, :384],
                                lhsT=cT[:, 2 * c2:2 * c2 + 2,
                                        g * P:(g + 1) * P],
                                rhs=wo[:, c2, :, fh * 384:(fh + 1) * 384],
                                start=(c2 == 0),
                                stop=(c2 == 2 and zero_bias),
                                perf_mode=DR)
                        if not zero_bias:
                            nc.tensor.matmul(
                                pm[:, :384], lhsT=ones_bf[:, :P],
                                rhs=borow[:, fh * 384:(fh + 1) * 384],
                                start=False, stop=True)
                        nc.vector.scalar_tensor_tensor(
                            out=xf[:, fh * 384:(fh + 1) * 384],
                            in0=pm[:, :384], scalar=DQ_WX,
                            in1=x_sb[:, g, fh * 384:(fh + 1) * 384],
                            op0=OP.mult, op1=OP.add,
                            accum_out=(s1a[:] if fh == 0 else s1b[:]))
                    s1g = tmp.tile([P, 1], F32, tag="s1g", name="s1g",
                                   bufs=4)
                    nc.vector.tensor_tensor(out=s1g[:], in0=s1a[:],
                                            in1=s1b[:], op=OP.add)
                    _layernorm(nc, tmp, x_sb[:, g, :], xf,
                               None if unit_ln else (G1, B1t), s1=s1g)

                # ---- attention.  Per head-pair: QK^T (2 heads in separate
                #      PE row-groups), fused exp on ACT, then per head both
                #      the AxV matmul and a ones-matmul normalizer sum (z)
                #      on PE.  The normalize tail (fast reciprocal of z +
                #      multiply) is deferred one pair so it overlaps the
                #      next pair's matmul/exp front.  PSUM: 4 banks/pair ->
                #      two pairs in flight. ----
                def _attn_tail(st):
                    e, ch, prbz, pcx = st
                    rb2 = tmp.tile([P, S], F32, tag="rb", name="rb")
                    nc.vector.reciprocal_approx_fast(rb2[:], prbz[:, :S])
                    for hh in range(2):
                        r0 = hh * DH
                        nc.vector.tensor_tensor(
                            out=cT[r0:r0 + DH, ch, e * S:(e + 1) * S],
                            in0=pcx[:DH, hh * S:(hh + 1) * S],
                            in1=rb2[r0:r0 + DH, :], op=OP.mult)

                # QK^T contracts only 64 partitions; heads A/B live in
                # PE row-groups {0,1}/{2,3} (lhsT base 0/64), so
                # alternating their matmuls runs them concurrently.
                # The QK matmuls of pair i+1 are EMITTED before the AV/z
                # matmuls of pair i: PE executes in order, so this gives it
                # work to do while pair i's exp runs on ACT.
                def _emit_qk(e, ch):
                    pscs = [ps.tile([P, 512], F32, tag="ps", name="psc",
                                    space="PSUM") for _ in range(2)]
                    for kt in range(2):
                        for hh in range(2):
                            r0 = hh * DH
                            nc.tensor.matmul(
                                pscs[hh][:, kt * S:(kt + 1) * S],
                                lhsT=kT[r0:r0 + DH, ch,
                                        e * S + kt * P:
                                        e * S + (kt + 1) * P],
                                rhs=qT[r0:r0 + DH, ch, e * S:(e + 1) * S],
                                start=True, stop=True)
                    return pscs

                pairs = [(e, ch) for e in range(BC) for ch in range(NH // 2)]
                prev_st = None
                pscs = _emit_qk(*pairs[0])
                for i, (e, ch) in enumerate(pairs):
                    next_pscs = (_emit_qk(*pairs[i + 1])
                                 if i + 1 < len(pairs) else None)
                    ET2 = tmp.tile([P, 2, 2, S], BF16, tag="ET", name="ET")
                    pcx = ps.tile([P, 512], F32, tag="ps", name="pcx",
                                  space="PSUM")
                    prbz = ps.tile([P, 512], F32, tag="ps", name="prb",
                                   space="PSUM")
                    for hh in range(2):
                        r0 = hh * DH
                        psc = pscs[hh]
                        if mask_ones:
                            # one fused exp over both key tiles
                            nc.scalar.activation(
                                ET2[:, hh].rearrange("p k s -> p (k s)"),
                                psc[:], AF.Exp, bias=0.0, scale=0.125)
                        else:
                            for kt in range(2):
                                nc.scalar.activation(
                                    ET2[:, hh, kt, :],
                                    psc[:, kt * S:(kt + 1) * S],
                                    AF.Exp, bias=mcol[:, kt, e:e + 1],
                                    scale=0.125)
                        h = 2 * ch + hh
                        for kt in range(2):
                            nc.tensor.matmul(
                                pcx[:DH, hh * S:(hh + 1) * S],
                                lhsT=vS[:, 2 * e + kt, h, :],
                                rhs=ET2[:, hh, kt, :],
                                start=(kt == 0), stop=(kt == 1))
                        for kt in range(2):
                            nc.tensor.matmul(
                                prbz[r0:r0 + DH, :S],
                                lhsT=ones64[:],
                                rhs=ET2[:, hh, kt, :],
                                start=(kt == 0), stop=(kt == 1))
                    if prev_st is not None:
                        _attn_tail(prev_st)
                    prev_st = (e, ch, prbz, pcx)
                    pscs = next_pscs
                _attn_tail(prev_st)
                for g in range(NT):
                    _oproj_ln1(g)

                # ---- FFN (two token-half passes) ----
                for th in range(2):
                    tiles = list(range(4 * th, 4 * th + 4))
                    _transpose_tiles(nc, ps, xT, x_sb, ident, tiles)
                    for j in range(FC):
                        if th == 1 and j == FC - 1:
                            # post-LN2 re-transpose of tiles 0-3 for the next
                            # layer (or classifier), emitted here so its
                            # PSUM->xT copies drain on ACT during the FFN2
                            # window instead of behind the LN2 chain
                            _transpose_tiles(nc, ps, xT, x_sb, ident, range(4))
                        pm = ps.tile([P, 512], F32, tag="ps", name="ph",
                                     space="PSUM")
                        for c2 in range(3):
                            nc.tensor.matmul(
                                pm[:], lhsT=w1js[j][:, c2],
                                rhs=xT[:, 2 * c2:2 * c2 + 2,
                                       th * 512:(th + 1) * 512],
                                start=(c2 == 0), stop=(c2 == 2),
                                perf_mode=DR)
                        bias = 0.0 if zero_bias else b1_sb[:, j:j + 1]
                        nc.scalar.activation(hT[:, j, :], pm[:], AF.Gelu,
                                             bias=bias, scale=DQ_WX)
                    # FFN2: f-half outer so W2 streams once per (th, fh)
                    xfs = [tmp.tile([P, H], F32, tag="xff", name="xff", bufs=4)
                           for _ in range(4)]
                    s1as = [tmp.tile([P, 1], F32, tag="s1fa", name="s1fa",
                                     bufs=4) for _ in range(4)]
                    s1bs = [tmp.tile([P, 1], F32, tag="s1fb", name="s1fb",
                                     bufs=4) for _ in range(4)]
                    for fh in range(2):
                        pms = [ps.tile([P, 512], F32, tag="ps", name="pf2",
                                       space="PSUM") for _ in range(4)]
                        for c2 in range(FC // 2):
                            w2c = wts.tile([P, 2, 384], FP8, tag="w2c",
                                           name="w2c", bufs=8)
                            nc.sync.dma_start(
                                w2c[:],
                                w2_d[l, c2, :, :, fh * 384:(fh + 1) * 384])
                            for gi in range(4):
                                nc.tensor.matmul(
                                    pms[gi][:, :384],
                                    lhsT=hT[:, 2 * c2:2 * c2 + 2,
                                            gi * P:(gi + 1) * P],
                                    rhs=w2c[:],
                                    start=(c2 == 0),
                                    stop=(c2 == FC // 2 - 1 and zero_bias),
                                    perf_mode=DR)
                        if not zero_bias:
                            for gi in range(4):
                                nc.tensor.matmul(
                                    pms[gi][:, :384], lhsT=ones_bf[:, :P],
                                    rhs=b2row[:, fh * 384:(fh + 1) * 384],
                                    start=False, stop=True)
                        for gi in range(4):
                            g = tiles[gi]
                            nc.vector.scalar_tensor_tensor(
                                out=xfs[gi][:, fh * 384:(fh + 1) * 384],
                                in0=pms[gi][:, :384], scalar=DQ_W,
                                in1=x_sb[:, g, fh * 384:(fh + 1) * 384],
                                op0=OP.mult, op1=OP.add,
                                accum_out=(s1as[gi][:] if fh == 0
                                           else s1bs[gi][:]))
                    for gi in range(4):
                        s1g = tmp.tile([P, 1], F32, tag="s1g", name="s1g",
                                       bufs=4)
                        nc.vector.tensor_tensor(out=s1g[:], in0=s1as[gi][:],
                                                in1=s1bs[gi][:], op=OP.add)
                        _layernorm(nc, tmp, x_sb[:, tiles[gi], :], xfs[gi],
                                   None if unit_ln else (G2, B2t), s1=s1g)

            if debug:
                nc.sync.dma_start(dd["dbg_x"][:], x_sb[:])

            # =========== classifier ===========
            clsw = cst.tile([P, 3, 2, 16], FP8, tag="clsw", name="clsw")
            nc.sync.dma_start(clsw[:], clsw_d[:])
            clsb = cst.tile([K, 1], F32, tag="clsb", name="clsb")
            nc.sync.dma_start(clsb[:], clsb_d[:])
            # tiles 0-3 already re-transposed in the last layer's FFN tail
            _transpose_tiles(nc, ps, xT, x_sb, ident, range(4, NT))
            for t2 in range(2):
                pm = ps.tile([P, 512], F32, tag="ps", name="pcls", space="PSUM")
                for c2 in range(3):
                    nc.tensor.matmul(
                        pm[:K, :], lhsT=clsw[:, c2, :, 0:K],
                        rhs=xT[:, 2 * c2:2 * c2 + 2, t2 * 512:(t2 + 1) * 512],
                        start=(c2 == 0), stop=(c2 == 2), perf_mode=DR)
                nc.scalar.activation(emT[:, t2 * 512:(t2 + 1) * 512],
                                     pm[:K, :], AF.Identity, bias=clsb[:, :1],
                                     scale=DQ_WX)
            nc.scalar.activation(expEm[:], emT[:], AF.Exp)
            if debug:
                nc.sync.dma_start(dd["dbg_em"][:], emT[:])

        # =========== CRF (weights/tmp pools closed; SBUF freed) ===========
        with contextlib.ExitStack() as cctx:
            crf = cctx.enter_context(tc.tile_pool(name="crf", bufs=1))
            ctmp = cctx.enter_context(tc.tile_pool(name="ctmp", bufs=4))

            def ct(name, shape, dtype=F32):
                return crf.tile(shape, dtype, tag=name, name=name)

            Mexp = ct("Mexp", [K, K])
            nc.sync.dma_start(Mexp[:], mexp_d[:])
            expStart = ct("expStart", [K, 1])
            expEnd = ct("expEnd", [K, 1])
            nc.sync.dma_start(expStart[:], expstart_d[:])
            nc.sync.dma_start(expEnd[:], expend_d[:])
            oh9w = ct("oh9w", [K, T])
            nc.sync.dma_start(oh9w[:], oh9w_d[:])

            # gold-emission dot product: num_dev = sum(emT * oh9w)
            sink9 = ct("sink9", [K, T])
            accK = ct("accK", [K, 1])
            nc.vector.scalar_tensor_tensor(
                out=sink9[:], in0=emT[:], scalar=1.0, in1=oh9w[:],
                op0=OP.mult, op1=OP.mult, accum_out=accK[:])

            # ---- linear-space scan ----
            expEm4 = expEm[:].rearrange("k (b s) -> k b s", b=BC)
            if mask_ones:
                # Chunked scan: alpha_255 = D_255 G_15...G_0 (M^T alpha_0)
                # with B_t = M^T D_t and G_c = B_{16c+16}...B_{16c+1}
                # (G_15 ends at B_254).  The 16 chunk factors G_c^T are built
                # simultaneously, 15 batched rounds of one matmul + one
                # row-scale over all (example, chunk) blocks:
                #   Pt <- D_t (M @ Pt),  t descending within each chunk.
                # The sequential fold is then only 16 steps deep per example.
                CH, CL = 16, 16
                # em_rep[k, t, j] = expEm[k, t]  (j-broadcast via 9 copies,
                # split ACT/DVE; both engines' copies avoid table reloads)
                em_rep = ct("em_rep", [K, T, K])
                srcEm = expEm[:].rearrange("k (t o) -> k t o", o=1)
                for j in range(K):
                    if j % 2 == 0:
                        nc.vector.tensor_copy(em_rep[:, :, j:j + 1], srcEm)
                    else:
                        nc.scalar.copy(em_rep[:, :, j:j + 1], srcEm)
                emr = em_rep[:].rearrange("k (b c s) j -> k b c s j",
                                          b=BC, c=CH)
                mexptS = ct("mexptS", [K, K])
                nc.sync.dma_start(mexptS[:], dd["mexpt"][:])
                mrepS = ct("mrepS", [K, BC, CH, K])
                nc.sync.dma_start(mrepS[:], dd["mrep"][:])
                PtS = ct("PtS", [K, BC, CH, K])
                # init chunks 0..14 at t=16(c+1); chunk 15 at t=254
                nc.vector.tensor_tensor(
                    out=PtS[:, :, 0:CH - 1, :], in0=mrepS[:, :, 0:CH - 1, :],
                    in1=emr[:, :, 1:CH, 0, :], op=OP.mult)
                nc.vector.tensor_tensor(
                    out=PtS[:, :, CH - 1, :], in0=mrepS[:, :, CH - 1, :],
                    in1=emr[:, :, CH - 1, 14, :], op=OP.mult)
                for r in range(1, CL):
                    cmax = CH - 1 if r <= 2 else CH
                    for h in range(2):
                        pu = ps.tile([P, 512], F32, tag="ps", name="pu",
                                     space="PSUM")
                        nc.tensor.matmul(
                            pu[:K, :2 * cmax * K], lhsT=mexptS[:],
                            rhs=PtS[:, 2 * h:2 * h + 2, 0:cmax, :],
                            start=True, stop=True)
                        nc.vector.tensor_tensor(
                            out=PtS[:, 2 * h:2 * h + 2, 0:cmax, :],
                            in0=pu[:K, :2 * cmax * K].rearrange(
                                "k (b c j) -> k b c j", b=2, c=cmax),
                            in1=emr[:, 2 * h:2 * h + 2, 0:cmax, CL - r, :],
                            op=OP.mult)
                # fold: beta0 = M^T @ (expStart * em_0), then 16 steps/example
                a0 = ctmp.tile([K, BC], F32, tag="a0", name="a0")
                nc.vector.tensor_scalar(
                    out=a0[:], in0=expEm4[:, :, 0],
                    scalar1=expStart[:, :1], scalar2=None, op0=OP.mult)
                endem = ctmp.tile([K, BC], F32, tag="endem", name="endem")
                nc.vector.tensor_scalar(
                    out=endem[:], in0=expEm4[:, :, S - 1],
                    scalar1=expEnd[:, :1], scalar2=None, op0=OP.mult)
                pb0 = ps.tile([P, 512], F32, tag="ps", name="pb0",
                              space="PSUM")
                nc.tensor.matmul(pb0[:K, :BC], lhsT=Mexp[:], rhs=a0[:],
                                 start=True, stop=True)
                gams = []
                for b in range(BC):
                    g0 = ctmp.tile([K, 1], F32, tag=f"g{b}", name=f"g0_{b}")
                    nc.vector.tensor_copy(g0[:], pb0[:K, b:b + 1])
                    gams.append(g0)
                F_ = ctmp.tile([K, BC], F32, tag="F", name="F_")
                for c in range(CH):
                    for b in range(BC):
                        pg = ps.tile([P, 512], F32, tag="ps", name="pg",
                                     space="PSUM")
                        nc.tensor.matmul(pg[:K, :1], lhsT=PtS[:, b, c, :],
                                         rhs=gams[b][:], start=True,
                                         stop=True)
                        if c == CH - 1:
                            nc.vector.tensor_tensor(
                                out=F_[:, b:b + 1], in0=pg[:K, :1],
                                in1=endem[:, b:b + 1], op=OP.mult)
                        else:
                            gn = ctmp.tile([K, 1], F32, tag=f"g{b}",
                                           name=f"g{c}_{b}")
                            nc.vector.tensor_copy(gn[:], pg[:K, :1])
                            gams[b] = gn
            else:
                mrow_i = ct("mrow_i", [1, T], I32)
                nc.sync.dma_start(mrow_i[:], dd["maskrow"][:])
                mrow = ct("mrow", [1, T])
                nc.vector.tensor_copy(mrow[:], mrow_i[:])
                inv9 = ct("inv9", [K, T])
                mb9 = ct("mb9", [K, T])
                for i in range(2):
                    pb = ps.tile([P, 512], F32, tag="ps", name="pmb",
                                 space="PSUM")
                    nc.tensor.matmul(pb[:K, :], lhsT=onescol_f[:1, :K],
                                     rhs=mrow[:, i * 512:(i + 1) * 512],
                                     start=True, stop=True)
                    nc.scalar.activation(mb9[:, i * 512:(i + 1) * 512],
                                         pb[:K, :], AF.Identity)
                nc.vector.tensor_scalar(out=inv9[:], in0=mb9[:], scalar1=0.0,
                                        scalar2=None, op0=OP.is_equal)
                inv4 = inv9[:].rearrange("k (b s) -> k b s", b=BC)
                # two independent 2-example chains
                Ecurs = []
                for hf in range(2):
                    Ec = ctmp.tile([K, 2], F32, tag=f"E{hf}", name=f"E0_{hf}")
                    nc.vector.tensor_scalar(
                        out=Ec[:], in0=expEm4[:, 2 * hf:2 * hf + 2, 0],
                        scalar1=expStart[:, :1], scalar2=None, op0=OP.mult)
                    Ecurs.append(Ec)
                for t in range(1, S):
                    for hf in range(2):
                        psn = ps.tile([P, 512], F32, tag="ps", name="pcrf",
                                      space="PSUM")
                        nc.tensor.matmul(psn[:K, :2], lhsT=Mexp[:],
                                         rhs=Ecurs[hf][:],
                                         start=True, stop=True)
                        Enew = ctmp.tile([K, 2], F32, tag=f"E{hf}",
                                         name=f"E{t}_{hf}")
                        nc.vector.tensor_tensor(
                            out=Enew[:], in0=psn[:K, :2],
                            in1=expEm4[:, 2 * hf:2 * hf + 2, t], op=OP.mult)
                        nc.vector.copy_predicated(
                            Enew[:], inv4[:, 2 * hf:2 * hf + 2, t],
                            Ecurs[hf][:])
                        Ecurs[hf] = Enew

                F_ = ctmp.tile([K, BC], F32, tag="F", name="F_")
                for hf in range(2):
                    nc.vector.tensor_scalar(
                        out=F_[:, 2 * hf:2 * hf + 2], in0=Ecurs[hf][:],
                        scalar1=expEnd[:, :1], scalar2=None, op0=OP.mult)
            psd = ps.tile([P, 512], F32, tag="ps", name="psd", space="PSUM")
            nc.tensor.matmul(psd[:1, :BC], lhsT=onescol_f[:K, :], rhs=F_[:],
                             start=True, stop=True)
            denomv = ctmp.tile([1, BC], F32, tag="denomv", name="denomv")
            denom_tot = ct("denom_tot", [1, 1])
            nc.scalar.activation(denomv[:], psd[:1, :BC], AF.Ln,
                                 accum_out=denom_tot[:])

            psn2 = ps.tile([P, 512], F32, tag="ps", name="psn2", space="PSUM")
            nc.tensor.matmul(psn2[:1, :1], lhsT=onescol_f[:K, :],
                             rhs=accK[:], start=True, stop=True)
            num_tot = ct("num_tot", [1, 1])
            nc.vector.tensor_copy(num_tot[:], psn2[:1, :1])
            out_sb = ct("out_sb", [1, 4])
            nc.vector.memset(out_sb[:], 0.0)
            nc.vector.tensor_tensor(out=out_sb[:, 0:1], in0=denom_tot[:],
                                    in1=num_tot[:], op=OP.subtract)
            nc.vector.tensor_copy(out_sb[:, 1:2], num_tot[:])
            nc.vector.tensor_copy(out_sb[:, 2:3], denom_tot[:])
            nc.sync.dma_start(out_d[:], out_sb[:])


# ---------------------------------------------------------------------------
# host wrapper
# ---------------------------------------------------------------------------

_NC_CACHE = {}


def _get_nc(key):
    if key not in _NC_CACHE:
        _NC_CACHE[key] = build_nc(*key)
    return _NC_CACHE[key]


def prepare_maps(inputs, mask_ones, zero_bias, unit_ln):
    """Returns (in_maps, label_const): per-core device inputs and the
    host-computed label-only CRF numerator sum over the whole batch."""
    input_ids = np.asarray(inputs["input_ids"]).astype(np.int32)
    attention_mask = np.asarray(inputs["attention_mask"]).astype(np.int32)
    labels = np.asarray(inputs["labels"]).astype(np.int64)

    word = _bf(inputs["word_emb"])
    pt = _bf((_f32(inputs["pos_emb"][:S]) + _f32(inputs["type_emb"][0])[None, :])
             .reshape(2, P, H).transpose(1, 0, 2))
    wq = _f8(inputs["Wq"], WS).reshape(L, 3, 2, P, H).transpose(
        0, 3, 1, 2, 4).copy()
    wk = _f8(inputs["Wk"], WS).reshape(L, 3, 2, P, H).transpose(
        0, 3, 1, 2, 4).copy()
    wv = _f8(inputs["Wv"], WS).reshape(L, 3, 2, P, H).transpose(
        0, 3, 1, 2, 4).copy()
    wo = _f8(inputs["Wo"], WS).reshape(L, 3, 2, P, H).transpose(
        0, 3, 1, 2, 4).copy()
    w1 = (_f8(inputs["W1"], WS).reshape(L, 3, 2, P, FC, P)
          .transpose(0, 4, 3, 1, 2, 5).copy())
    w2 = (_f8(inputs["W2"], WS).reshape(L, FC // 2, 2, P, H)
          .transpose(0, 1, 3, 2, 4).copy())
    cwpad = np.zeros((H, 16), np.float32)
    cwpad[:, :K] = _f32(inputs["cls_W"])
    clsw = _f8(cwpad, WS).reshape(3, 2, P, 16).transpose(2, 0, 1, 3).copy()
    clsb = (_f32(inputs["cls_b"]) - np.float32(C_SHIFT)).reshape(K, 1)

    trans = _f32(inputs["crf_trans"]).reshape(K, K)
    startv = _f32(inputs["crf_start"]).reshape(K)
    endv = _f32(inputs["crf_end"]).reshape(K)

    shared = dict(
        word=word, pt=pt, wq=wq, wk=wk, wv=wv, wo=wo, w1=w1, w2=w2,
        clsw=clsw, clsb=clsb,
        mexp=np.exp(trans).astype(np.float32),
        mexpt=np.ascontiguousarray(np.exp(trans).T.astype(np.float32)),
        mrep=np.ascontiguousarray(np.broadcast_to(
            np.exp(trans).astype(np.float32)[:, None, None, :],
            (K, BC, 16, K))),
        expstart=np.exp(startv).astype(np.float32).reshape(K, 1),
        expend=np.exp(endv).astype(np.float32).reshape(K, 1),
    )
    if not zero_bias:
        shared.update(
            bq=_f32(inputs["bq"]).reshape(L, HC, P).transpose(0, 2, 1).copy(),
            bk=_f32(inputs["bk"]).reshape(L, HC, P).transpose(0, 2, 1).copy(),
            b1=_f32(inputs["b1"]).reshape(L, FC, P).transpose(0, 2, 1).copy(),
            bvrow=_bf(_f32(inputs["bv"]) / DQ_WX).reshape(L, 1, H),
            borow=_bf(_f32(inputs["bo"]) / DQ_WX).reshape(L, 1, H),
            b2row=_bf(_f32(inputs["b2"]) / DQ_W).reshape(L, 1, H),
        )
    if not unit_ln:
        shared.update(
            lng=np.stack([_bf(inputs["ln1_g"]), _bf(inputs["ln2_g"])],
                         axis=1).reshape(L, 2, 1, H),
            lnb=np.stack([_bf(inputs["ln1_b"]), _bf(inputs["ln2_b"])],
                         axis=1).reshape(L, 2, 1, H),
            elng=np.stack([_bf(inputs["emb_ln_g"]), _bf(inputs["emb_ln_b"])],
                          axis=0).reshape(2, 1, H),
        )

    # ---- host label-only numerator + per-core oh9w ----
    mf = attention_mask.astype(np.float32)               # [B, S]
    w9 = mf.copy()
    w9[:, 0] = 1.0                                       # t=0 emission always counted
    trans_gold = trans[labels[:, :-1], labels[:, 1:]]    # [B, S-1]
    last_idx = attention_mask.astype(np.int64).sum(axis=1) - 1
    label_num = (startv[labels[:, 0]]
                 + (trans_gold * mf[:, 1:]).sum(axis=1)
                 + endv[labels[np.arange(B), last_idx]])  # [B]
    label_const = float(np.float32(label_num.astype(np.float32).sum()))

    in_maps = []
    for c in range(CORES):
        ids_c = input_ids[BC * c:BC * (c + 1)].reshape(NT, P, 1).copy()
        lab_c = labels[BC * c:BC * (c + 1)]              # [BC, S]
        w9_c = w9[BC * c:BC * (c + 1)]                   # [BC, S]
        oh = np.zeros((K, BC, S), np.float32)
        oh[lab_c.reshape(-1), np.repeat(np.arange(BC), S),
           np.tile(np.arange(S), BC)] = w9_c.reshape(-1)
        msk_c = attention_mask[BC * c:BC * (c + 1)]
        m = dict(shared)
        m["ids"] = ids_c
        m["oh9w"] = oh.reshape(K, T).copy()
        if not mask_ones:
            m["maskrow"] = msk_c.reshape(1, T).copy()
            m["maskcols"] = (msk_c.reshape(BC, 2, P).transpose(2, 1, 0)
                             .astype(np.int32).copy())
        in_maps.append(m)
    return in_maps, label_const


def kernel(**inputs) -> np.ndarray:
    attention_mask = np.asarray(inputs["attention_mask"])
    assert np.asarray(inputs["input_ids"]).shape == (B, S)

    mask_ones = bool(np.all(attention_mask == 1))
    zero_bias = all(
        not np.any(np.asarray(inputs[k]))
        for k in ("bq", "bk", "bv", "bo", "b1", "b2"))
    unit_ln = (all(np.all(np.asarray(inputs[k]) == 1.0)
                   for k in ("emb_ln_g", "ln1_g", "ln2_g"))
               and all(not np.any(np.asarray(inputs[k]))
                       for k in ("emb_ln_b", "ln1_b", "ln2_b")))

    n_layers = int(os.environ.get("BERTCRF_LAYERS", L))
    debug = bool(int(os.environ.get("BERTCRF_DEBUG", "0")))
    nc = _get_nc((n_layers, mask_ones, zero_bias, unit_ln, debug))
    in_maps, label_const = prepare_maps(inputs, mask_ones, zero_bias, unit_ln)

    res = run_bass_kernel_spmd(nc, in_maps, core_ids=list(range(CORES)))
    total = np.float32(0.0)
    for c in range(CORES):
        total += np.float32(res.results[c]["out"][0, 0])
    return np.float32(total - np.float32(label_const))


if __name__ == "__main__":
    import jax
    jax.config.update("jax_platforms", "cpu")
    import reference
    inp = reference.setup_inputs()
    outv = kernel(**{k: np.asarray(v) for k, v in inp.items()})
    print("kernel:", outv)


# revision 45
# speedup vs baseline: 1.2023x; 1.0210x over previous
"""BertCRF forward (BERT-base encoder + CRF NLL) on 8 Trainium2 NeuronCores.

Strategy: data-parallel over the batch (32 examples -> 4 per core), params
replicated.  Each core runs the full 12-layer encoder on its 1024 tokens with
fp8 (DoubleRow) matmuls for the dense projections, bf16 attention, fp32
layernorm, and a max-free softmax whose normalizer is computed by an extra
ones-matmul on the PE and inverted with a fast DVE reciprocal.  The CRF
numerator's label-only terms (start/transition/end) are computed on the host;
the device computes the gold-emission dot product and the exact linear-space
forward scan with a fixed per-step shift (cancels exactly between numerator
and denominator).  The host shards inputs, pre-arranges weight layouts, and
sums the 8 per-core partial NLLs plus the host-side label constant.
"""

import contextlib
import os

import numpy as np
import ml_dtypes

import concourse.bass as bass  # noqa: F401
import concourse.mybir as mybir
import concourse.tile as tile
from concourse import bacc
from concourse.bass import IndirectOffsetOnAxis
from concourse.bass_utils import run_bass_kernel_spmd
from concourse.masks import make_identity

# ---- problem constants (hardcoded per the task spec) ----
L, H, NH, DH, FF, V, K = 12, 768, 12, 64, 3072, 30522, 9
B, S = 32, 256
CORES = 8
BC = B // CORES          # 4 examples per core
T = BC * S               # 1024 tokens per core
P = 128
NT = T // P              # 8 token tiles
HC = H // P              # 6 hidden chunks
FC = FF // P             # 24 ff chunks
C_SHIFT = 2.35           # per-step CRF shift (cancels exactly in num-denom)

F32 = mybir.dt.float32
BF16 = mybir.dt.bfloat16
FP8 = mybir.dt.float8e4
I32 = mybir.dt.int32
AX = mybir.AxisListType
OP = mybir.AluOpType
AF = mybir.ActivationFunctionType
DR = mybir.MatmulPerfMode.DoubleRow

BF = ml_dtypes.bfloat16

# fp8 quantization scales (exact powers of two)
WS = 1024.0              # weight scale into fp8e4
XS = 32.0                # activation scale into fp8e4
DQ_WX = 2.0 ** -15       # dequant for w*x products
DQ_W = 2.0 ** -10        # dequant when only the weight was scaled


def _bf(x):
    return np.ascontiguousarray(np.asarray(x, dtype=np.float32)).astype(BF)


def _f8(x, scale):
    return np.ascontiguousarray(np.clip(
        np.asarray(x, dtype=np.float32) * scale, -240.0, 240.0)
    ).astype(ml_dtypes.float8_e4m3)


def _f32(x):
    return np.ascontiguousarray(np.asarray(x, dtype=np.float32))


# ---------------------------------------------------------------------------
# device program
# ---------------------------------------------------------------------------

def _layernorm(nc, tmp, out_bf, xf, gb, s1=None, sq_act=True):
    """LN over the free dim of xf [P, H] f32 -> out_bf (bf16).

    s1, if given, is a [P, 1] tile already holding sum(xf) (computed for free
    via accum_out on the op that produced xf).  sq_act picks the engine for
    the sum-of-squares pass: ACT (Square is in every table set) when ACT has
    headroom, DVE when ACT is the busier engine (attention phase).
    """
    if s1 is None:
        s1 = tmp.tile([P, 1], F32, tag="s1", name="s1")
        nc.vector.tensor_reduce(out=s1[:], in_=xf[:], axis=AX.X, op=OP.add)
    sq = tmp.tile([P, H], F32, tag="sq", name="sq")
    s2 = tmp.tile([P, 1], F32, tag="s2", name="s2")
    if sq_act:
        nc.scalar.activation(sq[:], xf[:], AF.Square, accum_out=s2[:])
    else:
        nc.vector.scalar_tensor_tensor(out=sq[:], in0=xf[:], scalar=1.0,
                                       in1=xf[:], op0=OP.mult, op1=OP.mult,
                                       accum_out=s2[:])
    m = tmp.tile([P, 1], F32, tag="m", name="m")
    nc.vector.tensor_scalar(out=m[:], in0=s1[:], scalar1=1.0 / H, scalar2=None,
                            op0=OP.mult)
    msq = tmp.tile([P, 1], F32, tag="msq", name="msq")
    nc.vector.tensor_tensor(out=msq[:], in0=m[:], in1=m[:], op=OP.mult)
    var = tmp.tile([P, 1], F32, tag="var", name="var")
    nc.vector.tensor_scalar(out=var[:], in0=s2[:], scalar1=1.0 / H,
                            scalar2=msq[:, :1], op0=OP.mult, op1=OP.subtract)
    # eps=1e-12 is below f32 resolution for var~O(1); bias=0.0 is identical
    sd = tmp.tile([P, 1], F32, tag="sd", name="sd")
    nc.scalar.activation(sd[:], var[:], AF.Sqrt, bias=0.0)
    rs = tmp.tile([P, 1], F32, tag="rs", name="rs")
    nc.vector.reciprocal_approx_fast(rs[:], sd[:])
    if gb is None:
        # out = rs*x - m*rs, one half on DVE, one half on ACT (in parallel)
        nmrs = tmp.tile([P, 1], F32, tag="nmrs", name="nmrs")
        nc.vector.tensor_scalar(out=nmrs[:], in0=m[:], scalar1=-1.0,
                                scalar2=rs[:, :1], op0=OP.mult, op1=OP.mult)
        HH = H // 2
        nc.vector.tensor_scalar(out=out_bf[:, 0:HH], in0=xf[:, 0:HH],
                                scalar1=m[:, :1], scalar2=rs[:, :1],
                                op0=OP.subtract, op1=OP.mult)
        nc.scalar.activation(out_bf[:, HH:H], xf[:, HH:H], AF.Identity,
                             bias=nmrs[:, :1], scale=rs[:, :1])
    else:
        G, Bb = gb
        t2 = tmp.tile([P, H], F32, tag="t2", name="t2")
        nc.vector.tensor_scalar(out=t2[:], in0=xf[:], scalar1=m[:, :1],
                                scalar2=rs[:, :1], op0=OP.subtract, op1=OP.mult)
        t3 = tmp.tile([P, H], F32, tag="t3", name="t3")
        nc.vector.tensor_tensor(out=t3[:], in0=t2[:], in1=G[:], op=OP.mult)
        nc.vector.tensor_tensor(out=out_bf, in0=t3[:], in1=Bb[:], op=OP.add)


def _transpose_tiles(nc, ps, xT, x_sb, ident, tiles):
    """x_sb [P, NT, H] token-major -> xT [P, HC, T] feature-major, per tile."""
    for g in tiles:
        for cg in range(2):          # chunk groups of 3
            pt_ = ps.tile([P, 1024], BF16, tag="ps", name="ptp", space="PSUM")
            for ci in range(3):
                c = cg * 3 + ci
                nc.tensor.matmul(
                    pt_[:, ci * P:(ci + 1) * P],
                    lhsT=x_sb[:, g, c * P:(c + 1) * P], rhs=ident[:],
                    start=True, stop=True, is_transpose=True)
            nc.scalar.activation(
                xT[:, cg * 3:cg * 3 + 3, g * P:(g + 1) * P],
                pt_[:, :384], AF.Identity, scale=XS)


def _bcast_row(nc, ps, tmp, dst, row_dram, ones_bf):
    """dst [P, H] bf16 = broadcast of a [1, H] bf16 dram row across partitions."""
    row = tmp.tile([1, H], BF16, tag="brow", name="brow")
    nc.sync.dma_start(row[:], row_dram)
    for fh in range(2):
        pb = ps.tile([P, 512], F32, tag="ps", name="pbc", space="PSUM")
        nc.tensor.matmul(pb[:, :384], lhsT=ones_bf[:, :P],
                         rhs=row[:, fh * 384:(fh + 1) * 384],
                         start=True, stop=True)
        nc.scalar.activation(dst[:, fh * 384:(fh + 1) * 384], pb[:, :384],
                             AF.Identity)


def build_nc(n_layers=L, mask_ones=True, zero_bias=True, unit_ln=True,
             debug=False):
    nc = bacc.Bacc("TRN2", target_bir_lowering=False, debug=False)

    dd = {}

    def din(name, shape, dtype):
        dd[name] = nc.dram_tensor(name, list(shape), dtype, kind="ExternalInput")
        return dd[name]

    def dout(name, shape, dtype):
        dd[name] = nc.dram_tensor(name, list(shape), dtype, kind="ExternalOutput")
        return dd[name]

    din("word", [V, H], BF16)
    din("ids", [NT, P, 1], I32)
    din("pt", [P, 2, H], BF16)
    din("wq", [L, P, 3, 2, H], FP8)
    din("wk", [L, P, 3, 2, H], FP8)
    din("wv", [L, P, 3, 2, H], FP8)
    din("wo", [L, P, 3, 2, H], FP8)
    din("w1", [L, FC, P, 3, 2, P], FP8)   # [l, j, ki, c2, ko, m]
    din("w2", [L, P, FC // 2, 2, H], FP8)  # [l, ki, c2, ko, n]
    din("clsw", [P, 3, 2, 16], FP8)      # K padded to 16
    din("clsb", [K, 1], F32)             # already shifted by -C_SHIFT
    din("mexp", [K, K], F32)             # exp(crf_trans)
    din("mexpt", [K, K], F32)            # exp(crf_trans).T
    din("mrep", [K, BC, 16, K], F32)     # exp(crf_trans) replicated 64x
    din("expstart", [K, 1], F32)         # exp(crf_start)
    din("expend", [K, 1], F32)           # exp(crf_end)
    din("oh9w", [K, T], F32)             # one-hot(labels) * emission weight
    if not mask_ones:
        din("maskrow", [1, T], I32)
        din("maskcols", [P, 2, BC], I32)
    if not zero_bias:
        din("bq", [L, P, HC], F32)
        din("bk", [L, P, HC], F32)
        din("b1", [L, P, FC], F32)
        din("bvrow", [L, 1, H], BF16)
        din("borow", [L, 1, H], BF16)
        din("b2row", [L, 1, H], BF16)
    if not unit_ln:
        din("lng", [L, 2, 1, H], BF16)
        din("lnb", [L, 2, 1, H], BF16)
        din("elng", [2, 1, H], BF16)

    dout("out", [1, 4], F32)
    if debug:
        dout("dbg_x0", [P, NT, H], BF16)
        dout("dbg_x", [P, NT, H], BF16)
        dout("dbg_em", [K, T], F32)

    _build_body(nc, dd, n_layers, mask_ones, zero_bias, unit_ln, debug)
    nc.compile()
    return nc


def _build_body(nc, dd, n_layers, mask_ones, zero_bias, unit_ln, debug):
    (word, ids, pt, wq_d, wk_d, wv_d, wo_d, w1_d, w2_d, clsw_d, clsb_d,
     mexp_d, expstart_d, expend_d, oh9w_d, out_d) = (
        dd["word"], dd["ids"], dd["pt"], dd["wq"], dd["wk"], dd["wv"],
        dd["wo"], dd["w1"], dd["w2"], dd["clsw"], dd["clsb"], dd["mexp"],
        dd["expstart"], dd["expend"], dd["oh9w"], dd["out"])
    if not mask_ones:
        maskrow_d = dd["maskrow"]
        maskcols_d = dd["maskcols"]
    if not zero_bias:
        bq_d, bk_d, b1_d = dd["bq"], dd["bk"], dd["b1"]
        bvrow_d, borow_d, b2row_d = dd["bvrow"], dd["borow"], dd["b2row"]
    if not unit_ln:
        lng_d, lnb_d, elng_d = dd["lng"], dd["lnb"], dd["elng"]
    with tile.TileContext(nc) as tc, contextlib.ExitStack() as octx:
        cst = octx.enter_context(tc.tile_pool(name="cst", bufs=1))
        act = octx.enter_context(tc.tile_pool(name="act", bufs=1))
        ps = octx.enter_context(tc.tile_pool(name="ps", bufs=8, space="PSUM"))

        # ---- persistent activation buffers ----
        x_sb = act.tile([P, NT, H], BF16, tag="x_sb", name="x_sb")
        xT = act.tile([P, HC, T], FP8, tag="xT", name="xT")
        qT = act.tile([P, HC, T], BF16, tag="qT", name="qT")
        kT = act.tile([P, HC, T], BF16, tag="kT", name="kT")
        vS = act.tile([P, NT, NH, DH], BF16, tag="vS", name="vS")
        cT = act.tile([P, HC, T], FP8, tag="cT", name="cT")
        hT = act.tile([P, FC, T // 2], FP8, tag="hT", name="hT")
        emT = act.tile([K, T], F32, tag="emT", name="emT")
        expEm = act.tile([K, T], F32, tag="expEm", name="expEm")

        # ---- constants ----
        ident = cst.tile([P, P], BF16, tag="ident", name="ident")
        make_identity(nc, ident[:])
        ones_bf = cst.tile([1, P], BF16, tag="ones_bf", name="ones_bf")
        nc.vector.memset(ones_bf[:], 1.0)
        # ones64: [128, 64] all-ones lhsT; sum over keys of exp(scores) into
        # one 64-partition half of the normalizer PSUM tile per head
        ones64 = cst.tile([P, DH], BF16, tag="ones64", name="ones64")
        nc.vector.memset(ones64[:], 1.0)
        onescol_f = cst.tile([P, 1], F32, tag="onescol_f", name="onescol_f")
        nc.vector.memset(onescol_f[:], 1.0)

        pt_sb = cst.tile([P, 2, H], BF16, tag="pt_sb", name="pt_sb")
        nc.sync.dma_start(pt_sb[:], pt[:])
        if not mask_ones:
            mcol = cst.tile([P, 2, BC], F32, tag="mcol", name="mcol")
        if not unit_ln:
            elnG = cst.tile([P, H], BF16, tag="elnG", name="elnG")
            elnB = cst.tile([P, H], BF16, tag="elnB", name="elnB")

        with contextlib.ExitStack() as ictx:
            wts = ictx.enter_context(tc.tile_pool(name="wts", bufs=1))
            tmp = ictx.enter_context(tc.tile_pool(name="tmp", bufs=3))

            if not unit_ln:
                _bcast_row(nc, ps, tmp, elnG, elng_d[0], ones_bf)
                _bcast_row(nc, ps, tmp, elnB, elng_d[1], ones_bf)

            # =========== embeddings ===========
            for g in range(NT):
                idx = tmp.tile([P, 1], I32, tag="idx", name="idx")
                nc.sync.dma_start(idx[:], ids[g])
                emb = tmp.tile([P, H], BF16, tag="emb", name="emb")
                nc.gpsimd.indirect_dma_start(
                    out=emb[:], out_offset=None, in_=word[:],
                    in_offset=IndirectOffsetOnAxis(ap=idx[:, :1], axis=0),
                )
                xf = tmp.tile([P, H], F32, tag="xf", name="xf")
                s1e = tmp.tile([P, 1], F32, tag="s1e", name="s1e")
                nc.vector.scalar_tensor_tensor(
                    out=xf[:], in0=emb[:], scalar=0.0,
                    in1=pt_sb[:, g % 2, :], op0=OP.add, op1=OP.add,
                    accum_out=s1e[:])
                _layernorm(nc, tmp, x_sb[:, g, :], xf,
                           None if unit_ln else (elnG, elnB), s1=s1e)
            if debug:
                nc.sync.dma_start(dd["dbg_x0"][:], x_sb[:])
            # prime xT tiles 0-3 for layer 0's QK proj t2=0
            _transpose_tiles(nc, ps, xT, x_sb, ident, range(4))

            if not mask_ones:
                mi = tmp.tile([P, 2, BC], I32, tag="mi", name="mi")
                nc.sync.dma_start(mi[:], maskcols_d[:])
                nc.vector.tensor_scalar(out=mcol[:], in0=mi[:], scalar1=1.0,
                                        scalar2=10000.0, op0=OP.subtract,
                                        op1=OP.mult)

            # =========== encoder layers ===========
            for l in range(n_layers):
                wq = wts.tile([P, 3, 2, H], FP8, tag="wq", name="wq")
                wk = wts.tile([P, 3, 2, H], FP8, tag="wk", name="wk")
                wv = wts.tile([P, 3, 2, H], FP8, tag="wv", name="wv")
                wo = wts.tile([P, 3, 2, H], FP8, tag="wo", name="wo")
                nc.sync.dma_start(wq[:], wq_d[l])
                nc.sync.dma_start(wk[:], wk_d[l])
                nc.sync.dma_start(wv[:], wv_d[l])
                nc.sync.dma_start(wo[:], wo_d[l])
                # preload the whole layer's W1 once (18.4KB/partition): the
                # 24 transfers stream during attention, FFN1 reads them from
                # SBUF in both token-half passes — halves W1 HBM traffic and
                # removes the just-in-time DMA stalls inside the FFN1 loop
                w1js = []
                for j in range(FC):
                    w1j = wts.tile([P, 3, 2, P], FP8, tag="w1j",
                                   name=f"w1j{j}", bufs=FC)
                    nc.sync.dma_start(w1j[:], w1_d[l, j])
                    w1js.append(w1j)
                # W2 likewise preloaded whole (18.4KB/partition, one DMA):
                # FFN2 reads it from SBUF in both token-half passes
                w2full = wts.tile([P, FC // 2, 2, H], FP8, tag="w2f",
                                  name="w2f")
                nc.sync.dma_start(w2full[:], w2_d[l])

                if not zero_bias:
                    bq_sb = wts.tile([P, HC], F32, tag="bq", name="bq")
                    bk_sb = wts.tile([P, HC], F32, tag="bk", name="bk")
                    b1_sb = wts.tile([P, FC], F32, tag="b1", name="b1")
                    nc.sync.dma_start(bq_sb[:], bq_d[l])
                    nc.sync.dma_start(bk_sb[:], bk_d[l])
                    nc.sync.dma_start(b1_sb[:], b1_d[l])
                    bvrow = wts.tile([1, H], BF16, tag="bvrow", name="bvrow")
                    borow = wts.tile([1, H], BF16, tag="borow", name="borow")
                    b2row = wts.tile([1, H], BF16, tag="b2row", name="b2row")
                    nc.sync.dma_start(bvrow[:], bvrow_d[l])
                    nc.sync.dma_start(borow[:], borow_d[l])
                    nc.sync.dma_start(b2row[:], b2row_d[l])
                if not unit_ln:
                    G1 = wts.tile([P, H], BF16, tag="G1", name="G1")
                    B1t = wts.tile([P, H], BF16, tag="B1t", name="B1t")
                    G2 = wts.tile([P, H], BF16, tag="G2", name="G2")
                    B2t = wts.tile([P, H], BF16, tag="B2t", name="B2t")
                    _bcast_row(nc, ps, tmp, G1, lng_d[l, 0], ones_bf)
                    _bcast_row(nc, ps, tmp, B1t, lnb_d[l, 0], ones_bf)
                    _bcast_row(nc, ps, tmp, G2, lng_d[l, 1], ones_bf)
                    _bcast_row(nc, ps, tmp, B2t, lnb_d[l, 1], ones_bf)

                # ---- qT/kT projections, token-half pipelined.  xT tiles 0-3
                #      were already transposed in the previous layer's FFN
                #      tail (or right after the embeddings for layer 0), so
                #      QK proj t2=0 can start while the previous layer's
                #      second-half LN2 chain is still draining; tiles 4-7 are
                #      transposed here in between. ----
                def _v_proj(gs):
                    for g in gs:
                        for fh in range(2):
                            pm = ps.tile([P, 512], F32, tag="ps", name="pv",
                                         space="PSUM")
                            for c2 in range(3):
                                nc.tensor.matmul(
                                    pm[:, :384],
                                    lhsT=xT[:, 2 * c2:2 * c2 + 2,
                                            g * P:(g + 1) * P],
                                    rhs=wv[:, c2, :, fh * 384:(fh + 1) * 384],
                                    start=(c2 == 0),
                                    stop=(c2 == 2 and zero_bias),
                                    perf_mode=DR)
                            if not zero_bias:
                                nc.tensor.matmul(
                                    pm[:, :384], lhsT=ones_bf[:, :P],
                                    rhs=bvrow[:, fh * 384:(fh + 1) * 384],
                                    start=False, stop=True)
                            nc.vector.tensor_scalar(
                                out=vS[:, g, 6 * fh:6 * fh + 6, :],
                                in0=pm[:, :384], scalar1=DQ_WX, scalar2=None,
                                op0=OP.mult)

                for t2 in range(2):
                    if t2 == 1:
                        # V proj of tiles 0-3 also only needs xT 0-3 + wv:
                        # more PE work that is independent of the previous
                        # layer's LN2-th1 chain, emitted before the
                        # transposes that must wait for it
                        _v_proj(range(4))
                        _transpose_tiles(nc, ps, xT, x_sb, ident, range(4, 8))
                    for wmat, bname, dst in ((wq, "bq", qT), (wk, "bk", kT)):
                        for f in range(HC):
                            pm = ps.tile([P, 512], F32, tag="ps", name="pqk",
                                         space="PSUM")
                            for c2 in range(3):
                                nc.tensor.matmul(
                                    pm[:],
                                    lhsT=wmat[:, c2, :, f * P:(f + 1) * P],
                                    rhs=xT[:, 2 * c2:2 * c2 + 2,
                                           t2 * 512:(t2 + 1) * 512],
                                    start=(c2 == 0), stop=(c2 == 2),
                                    perf_mode=DR)
                            if zero_bias:
                                nc.vector.tensor_scalar(
                                    out=dst[:, f, t2 * 512:(t2 + 1) * 512],
                                    in0=pm[:], scalar1=DQ_WX, scalar2=None,
                                    op0=OP.mult)
                            else:
                                bias = (bq_sb if bname == "bq"
                                        else bk_sb)[:, f:f + 1]
                                nc.scalar.activation(
                                    dst[:, f, t2 * 512:(t2 + 1) * 512], pm[:],
                                    AF.Identity, bias=bias, scale=DQ_WX)

                # ---- V projection for tiles 4-7 (0-3 done above) ----
                _v_proj(range(4, NT))

                # ---- output proj + residual + LN1 (interleaved
                #      into the attention loop, per example) ----
                def _oproj_ln1(g):
                    xf = tmp.tile([P, H], F32, tag="xf", name="xf")
                    s1a = tmp.tile([P, 1], F32, tag="s1a", name="s1a")
                    s1b = tmp.tile([P, 1], F32, tag="s1b", name="s1b")
                    for fh in range(2):
                        pm = ps.tile([P, 512], F32, tag="ps", name="po",
                                     space="PSUM")
                        for c2 in range(3):
                            nc.tensor.matmul(
                                pm[:, :384],
                                lhsT=cT[:, 2 * c2:2 * c2 + 2,
                                        g * P:(g + 1) * P],
                                rhs=wo[:, c2, :, fh * 384:(fh + 1) * 384],
                                start=(c2 == 0),
                                stop=(c2 == 2 and zero_bias),
                                perf_mode=DR)
                        if not zero_bias:
                            nc.tensor.matmul(
                                pm[:, :384], lhsT=ones_bf[:, :P],
                                rhs=borow[:, fh * 384:(fh + 1) * 384],
                                start=False, stop=True)
                        nc.vector.scalar_tensor_tensor(
                            out=xf[:, fh * 384:(fh + 1) * 384],
                            in0=pm[:, :384], scalar=DQ_WX,
                            in1=x_sb[:, g, fh * 384:(fh + 1) * 384],
                            op0=OP.mult, op1=OP.add,
                            accum_out=(s1a[:] if fh == 0 else s1b[:]))
                    s1g = tmp.tile([P, 1], F32, tag="s1g", name="s1g",
                                   bufs=4)
                    nc.vector.tensor_tensor(out=s1g[:], in0=s1a[:],
                                            in1=s1b[:], op=OP.add)
                    _layernorm(nc, tmp, x_sb[:, g, :], xf,
                               None if unit_ln else (G1, B1t), s1=s1g)

                # ---- attention.  Per head-pair: QK^T (2 heads in separate
                #      PE row-groups), fused exp on ACT, then per head both
                #      the AxV matmul and a ones-matmul normalizer sum (z)
                #      on PE.  The normalize tail (fast reciprocal of z +
                #      multiply) is deferred one pair so it overlaps the
                #      next pair's matmul/exp front.  PSUM: 4 banks/pair ->
                #      two pairs in flight. ----
                def _attn_tail(st):
                    e, ch, prbz, pcx = st
                    rb2 = tmp.tile([P, S], F32, tag="rb", name="rb")
                    nc.vector.reciprocal_approx_fast(rb2[:], prbz[:, :S])
                    for hh in range(2):
                        r0 = hh * DH
                        nc.vector.tensor_tensor(
                            out=cT[r0:r0 + DH, ch, e * S:(e + 1) * S],
                            in0=pcx[:DH, hh * S:(hh + 1) * S],
                            in1=rb2[r0:r0 + DH, :], op=OP.mult)

                # QK^T contracts only 64 partitions; heads A/B live in
                # PE row-groups {0,1}/{2,3} (lhsT base 0/64), so
                # alternating their matmuls runs them concurrently.
                # The QK matmuls of pair i+1 are EMITTED before the AV/z
                # matmuls of pair i: PE executes in order, so this gives it
                # work to do while pair i's exp runs on ACT.
                def _emit_qk(e, ch):
                    pscs = [ps.tile([P, 512], F32, tag="ps", name="psc",
                                    space="PSUM") for _ in range(2)]
                    for kt in range(2):
                        for hh in range(2):
                            r0 = hh * DH
                            nc.tensor.matmul(
                                pscs[hh][:, kt * S:(kt + 1) * S],
                                lhsT=kT[r0:r0 + DH, ch,
                                        e * S + kt * P:
                                        e * S + (kt + 1) * P],
                                rhs=qT[r0:r0 + DH, ch, e * S:(e + 1) * S],
                                start=True, stop=True)
                    return pscs

                pairs = [(e, ch) for e in range(BC) for ch in range(NH // 2)]
                prev_st = None
                pscs = _emit_qk(*pairs[0])
                for i, (e, ch) in enumerate(pairs):
                    next_pscs = (_emit_qk(*pairs[i + 1])
                                 if i + 1 < len(pairs) else None)
                    ET2 = tmp.tile([P, 2, 2, S], BF16, tag="ET", name="ET")
                    pcx = ps.tile([P, 512], F32, tag="ps", name="pcx",
                                  space="PSUM")
                    prbz = ps.tile([P, 512], F32, tag="ps", name="prb",
                                   space="PSUM")
                    for hh in range(2):
                        r0 = hh * DH
                        psc = pscs[hh]
                        if mask_ones:
                            # one fused exp over both key tiles
                            nc.scalar.activation(
                                ET2[:, hh].rearrange("p k s -> p (k s)"),
                                psc[:], AF.Exp, bias=0.0, scale=0.125)
                        else:
                            for kt in range(2):
                                nc.scalar.activation(
                                    ET2[:, hh, kt, :],
                                    psc[:, kt * S:(kt + 1) * S],
                                    AF.Exp, bias=mcol[:, kt, e:e + 1],
                                    scale=0.125)
                        h = 2 * ch + hh
                        for kt in range(2):
                            nc.tensor.matmul(
                                pcx[:DH, hh * S:(hh + 1) * S],
                                lhsT=vS[:, 2 * e + kt, h, :],
                                rhs=ET2[:, hh, kt, :],
                                start=(kt == 0), stop=(kt == 1))
                        for kt in range(2):
                            nc.tensor.matmul(
                                prbz[r0:r0 + DH, :S],
                                lhsT=ones64[:],
                                rhs=ET2[:, hh, kt, :],
                                start=(kt == 0), stop=(kt == 1))
                    if prev_st is not None:
                        _attn_tail(prev_st)
                    prev_st = (e, ch, prbz, pcx)
                    pscs = next_pscs
                _attn_tail(prev_st)
                for g in range(NT):
                    _oproj_ln1(g)

                # ---- FFN (two token-half passes) ----
                for th in range(2):
                    tiles = list(range(4 * th, 4 * th + 4))
                    _transpose_tiles(nc, ps, xT, x_sb, ident, tiles)
                    for j in range(FC):
                        if th == 1 and j == FC - 1:
                            # post-LN2 re-transpose of tiles 0-3 for the next
                            # layer (or classifier), emitted here so its
                            # PSUM->xT copies drain on ACT during the FFN2
                            # window instead of behind the LN2 chain
                            _transpose_tiles(nc, ps, xT, x_sb, ident, range(4))
                        pm = ps.tile([P, 512], F32, tag="ps", name="ph",
                                     space="PSUM")
                        for c2 in range(3):
                            nc.tensor.matmul(
                                pm[:], lhsT=w1js[j][:, c2],
                                rhs=xT[:, 2 * c2:2 * c2 + 2,
                                       th * 512:(th + 1) * 512],
                                start=(c2 == 0), stop=(c2 == 2),
                                perf_mode=DR)
                        bias = 0.0 if zero_bias else b1_sb[:, j:j + 1]
                        nc.scalar.activation(hT[:, j, :], pm[:], AF.Gelu,
                                             bias=bias, scale=DQ_WX)
                    # FFN2: f-half outer so W2 streams once per (th, fh)
                    xfs = [tmp.tile([P, H], F32, tag="xff", name="xff", bufs=4)
                           for _ in range(4)]
                    s1as = [tmp.tile([P, 1], F32, tag="s1fa", name="s1fa",
                                     bufs=4) for _ in range(4)]
                    s1bs = [tmp.tile([P, 1], F32, tag="s1fb", name="s1fb",
                                     bufs=4) for _ in range(4)]
                    for fh in range(2):
                        pms = [ps.tile([P, 512], F32, tag="ps", name="pf2",
                                       space="PSUM") for _ in range(4)]
                        for c2 in range(FC // 2):
                            for gi in range(4):
                                nc.tensor.matmul(
                                    pms[gi][:, :384],
                                    lhsT=hT[:, 2 * c2:2 * c2 + 2,
                                            gi * P:(gi + 1) * P],
                                    rhs=w2full[:, c2, :,
                                               fh * 384:(fh + 1) * 384],
                                    start=(c2 == 0),
                                    stop=(c2 == FC // 2 - 1 and zero_bias),
                                    perf_mode=DR)
                        if not zero_bias:
                            for gi in range(4):
                                nc.tensor.matmul(
                                    pms[gi][:, :384], lhsT=ones_bf[:, :P],
                                    rhs=b2row[:, fh * 384:(fh + 1) * 384],
                                    start=False, stop=True)
                        for gi in range(4):
                            g = tiles[gi]
                            nc.vector.scalar_tensor_tensor(
                                out=xfs[gi][:, fh * 384:(fh + 1) * 384],
                                in0=pms[gi][:, :384], scalar=DQ_W,
                                in1=x_sb[:, g, fh * 384:(fh + 1) * 384],
                                op0=OP.mult, op1=OP.add,
                                accum_out=(s1as[gi][:] if fh == 0
                                           else s1bs[gi][:]))
                    for gi in range(4):
                        s1g = tmp.tile([P, 1], F32, tag="s1g", name="s1g",
                                       bufs=4)
                        nc.vector.tensor_tensor(out=s1g[:], in0=s1as[gi][:],
                                                in1=s1bs[gi][:], op=OP.add)
                        _layernorm(nc, tmp, x_sb[:, tiles[gi], :], xfs[gi],
                                   None if unit_ln else (G2, B2t), s1=s1g)

            if debug:
                nc.sync.dma_start(dd["dbg_x"][:], x_sb[:])

            # =========== classifier ===========
            clsw = cst.tile([P, 3, 2, 16], FP8, tag="clsw", name="clsw")
            nc.sync.dma_start(clsw[:], clsw_d[:])
            clsb = cst.tile([K, 1], F32, tag="clsb", name="clsb")
            nc.sync.dma_start(clsb[:], clsb_d[:])
            # tiles 0-3 already re-transposed in the last layer's FFN tail
            _transpose_tiles(nc, ps, xT, x_sb, ident, range(4, NT))
            for t2 in range(2):
                pm = ps.tile([P, 512], F32, tag="ps", name="pcls", space="PSUM")
                for c2 in range(3):
                    nc.tensor.matmul(
                        pm[:K, :], lhsT=clsw[:, c2, :, 0:K],
                        rhs=xT[:, 2 * c2:2 * c2 + 2, t2 * 512:(t2 + 1) * 512],
                        start=(c2 == 0), stop=(c2 == 2), perf_mode=DR)
                nc.scalar.activation(emT[:, t2 * 512:(t2 + 1) * 512],
                                     pm[:K, :], AF.Identity, bias=clsb[:, :1],
                                     scale=DQ_WX)
            nc.scalar.activation(expEm[:], emT[:], AF.Exp)
            if debug:
                nc.sync.dma_start(dd["dbg_em"][:], emT[:])

        # =========== CRF (weights/tmp pools closed; SBUF freed) ===========
        with contextlib.ExitStack() as cctx:
            crf = cctx.enter_context(tc.tile_pool(name="crf", bufs=1))
            ctmp = cctx.enter_context(tc.tile_pool(name="ctmp", bufs=4))

            def ct(name, shape, dtype=F32):
                return crf.tile(shape, dtype, tag=name, name=name)

            Mexp = ct("Mexp", [K, K])
            nc.sync.dma_start(Mexp[:], mexp_d[:])
            expStart = ct("expStart", [K, 1])
            expEnd = ct("expEnd", [K, 1])
            nc.sync.dma_start(expStart[:], expstart_d[:])
            nc.sync.dma_start(expEnd[:], expend_d[:])
            oh9w = ct("oh9w", [K, T])
            nc.sync.dma_start(oh9w[:], oh9w_d[:])

            # gold-emission dot product: num_dev = sum(emT * oh9w)
            sink9 = ct("sink9", [K, T])
            accK = ct("accK", [K, 1])
            nc.vector.scalar_tensor_tensor(
                out=sink9[:], in0=emT[:], scalar=1.0, in1=oh9w[:],
                op0=OP.mult, op1=OP.mult, accum_out=accK[:])

            # ---- linear-space scan ----
            expEm4 = expEm[:].rearrange("k (b s) -> k b s", b=BC)
            if mask_ones:
                # Chunked scan: alpha_255 = D_255 G_15...G_0 (M^T alpha_0)
                # with B_t = M^T D_t and G_c = B_{16c+16}...B_{16c+1}
                # (G_15 ends at B_254).  The 16 chunk factors G_c^T are built
                # simultaneously, 15 batched rounds of one matmul + one
                # row-scale over all (example, chunk) blocks:
                #   Pt <- D_t (M @ Pt),  t descending within each chunk.
                # The sequential fold is then only 16 steps deep per example.
                CH, CL = 16, 16
                # em_rep[k, t, j] = expEm[k, t]  (j-broadcast via 9 copies,
                # split ACT/DVE; both engines' copies avoid table reloads)
                em_rep = ct("em_rep", [K, T, K])
                srcEm = expEm[:].rearrange("k (t o) -> k t o", o=1)
                for j in range(K):
                    if j % 2 == 0:
                        nc.vector.tensor_copy(em_rep[:, :, j:j + 1], srcEm)
                    else:
                        nc.scalar.copy(em_rep[:, :, j:j + 1], srcEm)
                emr = em_rep[:].rearrange("k (b c s) j -> k b c s j",
                                          b=BC, c=CH)
                mexptS = ct("mexptS", [K, K])
                nc.sync.dma_start(mexptS[:], dd["mexpt"][:])
                mrepS = ct("mrepS", [K, BC, CH, K])
                nc.sync.dma_start(mrepS[:], dd["mrep"][:])
                PtS = ct("PtS", [K, BC, CH, K])
                # init chunks 0..14 at t=16(c+1); chunk 15 at t=254
                nc.vector.tensor_tensor(
                    out=PtS[:, :, 0:CH - 1, :], in0=mrepS[:, :, 0:CH - 1, :],
                    in1=emr[:, :, 1:CH, 0, :], op=OP.mult)
                nc.vector.tensor_tensor(
                    out=PtS[:, :, CH - 1, :], in0=mrepS[:, :, CH - 1, :],
                    in1=emr[:, :, CH - 1, 14, :], op=OP.mult)
                for r in range(1, CL):
                    cmax = CH - 1 if r <= 2 else CH
                    for h in range(2):
                        pu = ps.tile([P, 512], F32, tag="ps", name="pu",
                                     space="PSUM")
                        nc.tensor.matmul(
                            pu[:K, :2 * cmax * K], lhsT=mexptS[:],
                            rhs=PtS[:, 2 * h:2 * h + 2, 0:cmax, :],
                            start=True, stop=True)
                        nc.vector.tensor_tensor(
                            out=PtS[:, 2 * h:2 * h + 2, 0:cmax, :],
                            in0=pu[:K, :2 * cmax * K].rearrange(
                                "k (b c j) -> k b c j", b=2, c=cmax),
                            in1=emr[:, 2 * h:2 * h + 2, 0:cmax, CL - r, :],
                            op=OP.mult)
                # fold: beta0 = M^T @ (expStart * em_0), then 16 steps/example
                a0 = ctmp.tile([K, BC], F32, tag="a0", name="a0")
                nc.vector.tensor_scalar(
                    out=a0[:], in0=expEm4[:, :, 0],
                    scalar1=expStart[:, :1], scalar2=None, op0=OP.mult)
                endem = ctmp.tile([K, BC], F32, tag="endem", name="endem")
                nc.vector.tensor_scalar(
                    out=endem[:], in0=expEm4[:, :, S - 1],
                    scalar1=expEnd[:, :1], scalar2=None, op0=OP.mult)
                pb0 = ps.tile([P, 512], F32, tag="ps", name="pb0",
                              space="PSUM")
                nc.tensor.matmul(pb0[:K, :BC], lhsT=Mexp[:], rhs=a0[:],
                                 start=True, stop=True)
                gams = []
                for b in range(BC):
                    g0 = ctmp.tile([K, 1], F32, tag=f"g{b}", name=f"g0_{b}")
                    nc.vector.tensor_copy(g0[:], pb0[:K, b:b + 1])
                    gams.append(g0)
                F_ = ctmp.tile([K, BC], F32, tag="F", name="F_")
                for c in range(CH):
                    for b in range(BC):
                        pg = ps.tile([P, 512], F32, tag="ps", name="pg",
                                     space="PSUM")
                        nc.tensor.matmul(pg[:K, :1], lhsT=PtS[:, b, c, :],
                                         rhs=gams[b][:], start=True,
                                         stop=True)
                        if c == CH - 1:
                            nc.vector.tensor_tensor(
                                out=F_[:, b:b + 1], in0=pg[:K, :1],
                                in1=endem[:, b:b + 1], op=OP.mult)
                        else:
                            gn = ctmp.tile([K, 1], F32, tag=f"g{b}",
                                           name=f"g{c}_{b}")
                            nc.vector.tensor_copy(gn[:], pg[:K, :1])
                            gams[b] = gn
            else:
                mrow_i = ct("mrow_i", [1, T], I32)
                nc.sync.dma_start(mrow_i[:], dd["maskrow"][:])
                mrow = ct("mrow", [1, T])
                nc.vector.tensor_copy(mrow[:], mrow_i[:])
                inv9 = ct("inv9", [K, T])
                mb9 = ct("mb9", [K, T])
                for i in range(2):
                    pb = ps.tile([P, 512], F32, tag="ps", name="pmb",
                                 space="PSUM")
                    nc.tensor.matmul(pb[:K, :], lhsT=onescol_f[:1, :K],
                                     rhs=mrow[:, i * 512:(i + 1) * 512],
                                     start=True, stop=True)
                    nc.scalar.activation(mb9[:, i * 512:(i + 1) * 512],
                                         pb[:K, :], AF.Identity)
                nc.vector.tensor_scalar(out=inv9[:], in0=mb9[:], scalar1=0.0,
                                        scalar2=None, op0=OP.is_equal)
                inv4 = inv9[:].rearrange("k (b s) -> k b s", b=BC)
                # two independent 2-example chains
                Ecurs = []
                for hf in range(2):
                    Ec = ctmp.tile([K, 2], F32, tag=f"E{hf}", name=f"E0_{hf}")
                    nc.vector.tensor_scalar(
                        out=Ec[:], in0=expEm4[:, 2 * hf:2 * hf + 2, 0],
                        scalar1=expStart[:, :1], scalar2=None, op0=OP.mult)
                    Ecurs.append(Ec)
                for t in range(1, S):
                    for hf in range(2):
                        psn = ps.tile([P, 512], F32, tag="ps", name="pcrf",
                                      space="PSUM")
                        nc.tensor.matmul(psn[:K, :2], lhsT=Mexp[:],
                                         rhs=Ecurs[hf][:],
                                         start=True, stop=True)
                        Enew = ctmp.tile([K, 2], F32, tag=f"E{hf}",
                                         name=f"E{t}_{hf}")
                        nc.vector.tensor_tensor(
                            out=Enew[:], in0=psn[:K, :2],
                            in1=expEm4[:, 2 * hf:2 * hf + 2, t], op=OP.mult)
                        nc.vector.copy_predicated(
                            Enew[:], inv4[:, 2 * hf:2 * hf + 2, t],
                            Ecurs[hf][:])
                        Ecurs[hf] = Enew

                F_ = ctmp.tile([K, BC], F32, tag="F", name="F_")
                for hf in range(2):
                    nc.vector.tensor_scalar(
                        out=F_[:, 2 * hf:2 * hf + 2], in0=Ecurs[hf][:],
                        scalar1=expEnd[:, :1], scalar2=None, op0=OP.mult)
            psd = ps.tile([P, 512], F32, tag="ps", name="psd", space="PSUM")
            nc.tensor.matmul(psd[:1, :BC], lhsT=onescol_f[:K, :], rhs=F_[:],
                             start=True, stop=True)
            denomv = ctmp.tile([1, BC], F32, tag="denomv", name="denomv")
            denom_tot = ct("denom_tot", [1, 1])
            nc.scalar.activation(denomv[:], psd[:1, :BC], AF.Ln,
                                 accum_out=denom_tot[:])

            psn2 = ps.tile([P, 512], F32, tag="ps", name="psn2", space="PSUM")
            nc.tensor.matmul(psn2[:1, :1], lhsT=onescol_f[:K, :],
                             rhs=accK[:], start=True, stop=True)
            num_tot = ct("num_tot", [1, 1])
            nc.vector.tensor_copy(num_tot[:], psn2[:1, :1])
            out_sb = ct("out_sb", [1, 4])
            nc.vector.memset(out_sb[:], 0.0)
            nc.vector.tensor_tensor(out=out_sb[:, 0:1], in0=denom_tot[:],
                                    in1=num_tot[:], op=OP.subtract)
            nc.vector.tensor_copy(out_sb[:, 1:2], num_tot[:])
            nc.vector.tensor_copy(out_sb[:, 2:3], denom_tot[:])
            nc.sync.dma_start(out_d[:], out_sb[:])


# ---------------------------------------------------------------------------
# host wrapper
# ---------------------------------------------------------------------------

_NC_CACHE = {}


def _get_nc(key):
    if key not in _NC_CACHE:
        _NC_CACHE[key] = build_nc(*key)
    return _NC_CACHE[key]


def prepare_maps(inputs, mask_ones, zero_bias, unit_ln):
    """Returns (in_maps, label_const): per-core device inputs and the
    host-computed label-only CRF numerator sum over the whole batch."""
    input_ids = np.asarray(inputs["input_ids"]).astype(np.int32)
    attention_mask = np.asarray(inputs["attention_mask"]).astype(np.int32)
    labels = np.asarray(inputs["labels"]).astype(np.int64)

    word = _bf(inputs["word_emb"])
    pt = _bf((_f32(inputs["pos_emb"][:S]) + _f32(inputs["type_emb"][0])[None, :])
             .reshape(2, P, H).transpose(1, 0, 2))
    wq = _f8(inputs["Wq"], WS).reshape(L, 3, 2, P, H).transpose(
        0, 3, 1, 2, 4).copy()
    wk = _f8(inputs["Wk"], WS).reshape(L, 3, 2, P, H).transpose(
        0, 3, 1, 2, 4).copy()
    wv = _f8(inputs["Wv"], WS).reshape(L, 3, 2, P, H).transpose(
        0, 3, 1, 2, 4).copy()
    wo = _f8(inputs["Wo"], WS).reshape(L, 3, 2, P, H).transpose(
        0, 3, 1, 2, 4).copy()
    w1 = (_f8(inputs["W1"], WS).reshape(L, 3, 2, P, FC, P)
          .transpose(0, 4, 3, 1, 2, 5).copy())
    w2 = (_f8(inputs["W2"], WS).reshape(L, FC // 2, 2, P, H)
          .transpose(0, 3, 1, 2, 4).copy())
    cwpad = np.zeros((H, 16), np.float32)
    cwpad[:, :K] = _f32(inputs["cls_W"])
    clsw = _f8(cwpad, WS).reshape(3, 2, P, 16).transpose(2, 0, 1, 3).copy()
    clsb = (_f32(inputs["cls_b"]) - np.float32(C_SHIFT)).reshape(K, 1)

    trans = _f32(inputs["crf_trans"]).reshape(K, K)
    startv = _f32(inputs["crf_start"]).reshape(K)
    endv = _f32(inputs["crf_end"]).reshape(K)

    shared = dict(
        word=word, pt=pt, wq=wq, wk=wk, wv=wv, wo=wo, w1=w1, w2=w2,
        clsw=clsw, clsb=clsb,
        mexp=np.exp(trans).astype(np.float32),
        mexpt=np.ascontiguousarray(np.exp(trans).T.astype(np.float32)),
        mrep=np.ascontiguousarray(np.broadcast_to(
            np.exp(trans).astype(np.float32)[:, None, None, :],
            (K, BC, 16, K))),
        expstart=np.exp(startv).astype(np.float32).reshape(K, 1),
        expend=np.exp(endv).astype(np.float32).reshape(K, 1),
    )
    if not zero_bias:
        shared.update(
            bq=_f32(inputs["bq"]).reshape(L, HC, P).transpose(0, 2, 1).copy(),
            bk=_f32(inputs["bk"]).reshape(L, HC, P).transpose(0, 2, 1).copy(),
            b1=_f32(inputs["b1"]).reshape(L, FC, P).transpose(0, 2, 1).copy(),
            bvrow=_bf(_f32(inputs["bv"]) / DQ_WX).reshape(L, 1, H),
            borow=_bf(_f32(inputs["bo"]) / DQ_WX).reshape(L, 1, H),
            b2row=_bf(_f32(inputs["b2"]) / DQ_W).reshape(L, 1, H),
        )
    if not unit_ln:
        shared.update(
            lng=np.stack([_bf(inputs["ln1_g"]), _bf(inputs["ln2_g"])],
                         axis=1).reshape(L, 2, 1, H),
            lnb=np.stack([_bf(inputs["ln1_b"]), _bf(inputs["ln2_b"])],
                         axis=1).reshape(L, 2, 1, H),
            elng=np.stack([_bf(inputs["emb_ln_g"]), _bf(inputs["emb_ln_b"])],
                          axis=0).reshape(2, 1, H),
        )

    # ---- host label-only numerator + per-core oh9w ----
    mf = attention_mask.astype(np.float32)               # [B, S]
    w9 = mf.copy()
    w9[:, 0] = 1.0                                       # t=0 emission always counted
    trans_gold = trans[labels[:, :-1], labels[:, 1:]]    # [B, S-1]
    last_idx = attention_mask.astype(np.int64).sum(axis=1) - 1
    label_num = (startv[labels[:, 0]]
                 + (trans_gold * mf[:, 1:]).sum(axis=1)
                 + endv[labels[np.arange(B), last_idx]])  # [B]
    label_const = float(np.float32(label_num.astype(np.float32).sum()))

    in_maps = []
    for c in range(CORES):
        ids_c = input_ids[BC * c:BC * (c + 1)].reshape(NT, P, 1).copy()
        lab_c = labels[BC * c:BC * (c + 1)]              # [BC, S]
        w9_c = w9[BC * c:BC * (c + 1)]                   # [BC, S]
        oh = np.zeros((K, BC, S), np.float32)
        oh[lab_c.reshape(-1), np.repeat(np.arange(BC), S),
           np.tile(np.arange(S), BC)] = w9_c.reshape(-1)
        msk_c = attention_mask[BC * c:BC * (c + 1)]
        m = dict(shared)
        m["ids"] = ids_c
        m["oh9w"] = oh.reshape(K, T).copy()
        if not mask_ones:
            m["maskrow"] = msk_c.reshape(1, T).copy()
            m["maskcols"] = (msk_c.reshape(BC, 2, P).transpose(2, 1, 0)
                             .astype(np.int32).copy())
        in_maps.append(m)
    return in_maps, label_const


def kernel(**inputs) -> np.ndarray:
    attention_mask = np.asarray(inputs["attention_mask"])
    assert np.asarray(inputs["input_ids"]).shape == (B, S)

    mask_ones = bool(np.all(attention_mask == 1))
    zero_bias = all(
        not np.any(np.asarray(inputs[k]))
        for k in ("bq", "bk", "bv", "bo", "b1", "b2"))
    unit_ln = (all(np.all(np.asarray(inputs[k]) == 1.0)
                   for k in ("emb_ln_g", "ln1_g", "ln2_g"))
               and all(not np.any(np.asarray(inputs[k]))
                       for k in ("emb_ln_b", "ln1_b", "ln2_b")))

    n_layers = int(os.environ.get("BERTCRF_LAYERS", L))
    debug = bool(int(os.environ.get("BERTCRF_DEBUG", "0")))
    nc = _get_nc((n_layers, mask_ones, zero_bias, unit_ln, debug))
    in_maps, label_const = prepare_maps(inputs, mask_ones, zero_bias, unit_ln)

    res = run_bass_kernel_spmd(nc, in_maps, core_ids=list(range(CORES)))
    total = np.float32(0.0)
    for c in range(CORES):
        total += np.float32(res.results[c]["out"][0, 0])
    return np.float32(total - np.float32(label_const))


if __name__ == "__main__":
    import jax
    jax.config.update("jax_platforms", "cpu")
    import reference
    inp = reference.setup_inputs()
    outv = kernel(**{k: np.asarray(v) for k, v in inp.items()})
    print("kernel:", outv)


# revision 47
# speedup vs baseline: 1.2027x; 1.0003x over previous
"""BertCRF forward (BERT-base encoder + CRF NLL) on 8 Trainium2 NeuronCores.

Strategy: data-parallel over the batch (32 examples -> 4 per core), params
replicated.  Each core runs the full 12-layer encoder on its 1024 tokens with
fp8 (DoubleRow) matmuls for the dense projections, bf16 attention, fp32
layernorm, and a max-free softmax whose normalizer is computed by an extra
ones-matmul on the PE and inverted with a fast DVE reciprocal.  The CRF
numerator's label-only terms (start/transition/end) are computed on the host;
the device computes the gold-emission dot product and the exact linear-space
forward scan with a fixed per-step shift (cancels exactly between numerator
and denominator).  The host shards inputs, pre-arranges weight layouts, and
sums the 8 per-core partial NLLs plus the host-side label constant.
"""

import contextlib
import os

import numpy as np
import ml_dtypes

import concourse.bass as bass  # noqa: F401
import concourse.mybir as mybir
import concourse.tile as tile
from concourse import bacc
from concourse.bass import IndirectOffsetOnAxis
from concourse.bass_utils import run_bass_kernel_spmd
from concourse.masks import make_identity

# ---- problem constants (hardcoded per the task spec) ----
L, H, NH, DH, FF, V, K = 12, 768, 12, 64, 3072, 30522, 9
B, S = 32, 256
CORES = 8
BC = B // CORES          # 4 examples per core
T = BC * S               # 1024 tokens per core
P = 128
NT = T // P              # 8 token tiles
HC = H // P              # 6 hidden chunks
FC = FF // P             # 24 ff chunks
C_SHIFT = 2.35           # per-step CRF shift (cancels exactly in num-denom)

F32 = mybir.dt.float32
BF16 = mybir.dt.bfloat16
FP8 = mybir.dt.float8e4
I32 = mybir.dt.int32
AX = mybir.AxisListType
OP = mybir.AluOpType
AF = mybir.ActivationFunctionType
DR = mybir.MatmulPerfMode.DoubleRow

BF = ml_dtypes.bfloat16

# fp8 quantization scales (exact powers of two)
WS = 1024.0              # weight scale into fp8e4
XS = 32.0                # activation scale into fp8e4
DQ_WX = 2.0 ** -15       # dequant for w*x products
DQ_W = 2.0 ** -10        # dequant when only the weight was scaled


def _bf(x):
    return np.ascontiguousarray(np.asarray(x, dtype=np.float32)).astype(BF)


def _f8(x, scale):
    return np.ascontiguousarray(np.clip(
        np.asarray(x, dtype=np.float32) * scale, -240.0, 240.0)
    ).astype(ml_dtypes.float8_e4m3)


def _f32(x):
    return np.ascontiguousarray(np.asarray(x, dtype=np.float32))


# ---------------------------------------------------------------------------
# device program
# ---------------------------------------------------------------------------

def _layernorm(nc, tmp, out_bf, xf, gb, s1=None, sq_act=True):
    """LN over the free dim of xf [P, H] f32 -> out_bf (bf16).

    s1, if given, is a [P, 1] tile already holding sum(xf) (computed for free
    via accum_out on the op that produced xf).  sq_act picks the engine for
    the sum-of-squares pass: ACT (Square is in every table set) when ACT has
    headroom, DVE when ACT is the busier engine (attention phase).
    """
    if s1 is None:
        s1 = tmp.tile([P, 1], F32, tag="s1", name="s1")
        nc.vector.tensor_reduce(out=s1[:], in_=xf[:], axis=AX.X, op=OP.add)
    sq = tmp.tile([P, H], F32, tag="sq", name="sq")
    s2 = tmp.tile([P, 1], F32, tag="s2", name="s2")
    if sq_act:
        nc.scalar.activation(sq[:], xf[:], AF.Square, accum_out=s2[:])
    else:
        nc.vector.scalar_tensor_tensor(out=sq[:], in0=xf[:], scalar=1.0,
                                       in1=xf[:], op0=OP.mult, op1=OP.mult,
                                       accum_out=s2[:])
    m = tmp.tile([P, 1], F32, tag="m", name="m")
    nc.vector.tensor_scalar(out=m[:], in0=s1[:], scalar1=1.0 / H, scalar2=None,
                            op0=OP.mult)
    msq = tmp.tile([P, 1], F32, tag="msq", name="msq")
    nc.vector.tensor_tensor(out=msq[:], in0=m[:], in1=m[:], op=OP.mult)
    var = tmp.tile([P, 1], F32, tag="var", name="var")
    nc.vector.tensor_scalar(out=var[:], in0=s2[:], scalar1=1.0 / H,
                            scalar2=msq[:, :1], op0=OP.mult, op1=OP.subtract)
    # eps=1e-12 is below f32 resolution for var~O(1); bias=0.0 is identical
    sd = tmp.tile([P, 1], F32, tag="sd", name="sd")
    nc.scalar.activation(sd[:], var[:], AF.Sqrt, bias=0.0)
    rs = tmp.tile([P, 1], F32, tag="rs", name="rs")
    nc.vector.reciprocal_approx_fast(rs[:], sd[:])
    if gb is None:
        # out = rs*x - m*rs, one half on DVE, one half on ACT (in parallel)
        nmrs = tmp.tile([P, 1], F32, tag="nmrs", name="nmrs")
        nc.vector.tensor_scalar(out=nmrs[:], in0=m[:], scalar1=-1.0,
                                scalar2=rs[:, :1], op0=OP.mult, op1=OP.mult)
        HH = H // 2
        nc.vector.tensor_scalar(out=out_bf[:, 0:HH], in0=xf[:, 0:HH],
                                scalar1=m[:, :1], scalar2=rs[:, :1],
                                op0=OP.subtract, op1=OP.mult)
        nc.scalar.activation(out_bf[:, HH:H], xf[:, HH:H], AF.Identity,
                             bias=nmrs[:, :1], scale=rs[:, :1])
    else:
        G, Bb = gb
        t2 = tmp.tile([P, H], F32, tag="t2", name="t2")
        nc.vector.tensor_scalar(out=t2[:], in0=xf[:], scalar1=m[:, :1],
                                scalar2=rs[:, :1], op0=OP.subtract, op1=OP.mult)
        t3 = tmp.tile([P, H], F32, tag="t3", name="t3")
        nc.vector.tensor_tensor(out=t3[:], in0=t2[:], in1=G[:], op=OP.mult)
        nc.vector.tensor_tensor(out=out_bf, in0=t3[:], in1=Bb[:], op=OP.add)


def _transpose_tiles(nc, ps, xT, x_sb, ident, tiles):
    """x_sb [P, NT, H] token-major -> xT [P, HC, T] feature-major, per tile."""
    for g in tiles:
        for cg in range(2):          # chunk groups of 3
            pt_ = ps.tile([P, 1024], BF16, tag="ps", name="ptp", space="PSUM")
            for ci in range(3):
                c = cg * 3 + ci
                nc.tensor.matmul(
                    pt_[:, ci * P:(ci + 1) * P],
                    lhsT=x_sb[:, g, c * P:(c + 1) * P], rhs=ident[:],
                    start=True, stop=True, is_transpose=True)
            nc.scalar.activation(
                xT[:, cg * 3:cg * 3 + 3, g * P:(g + 1) * P],
                pt_[:, :384], AF.Identity, scale=XS)


def _bcast_row(nc, ps, tmp, dst, row_dram, ones_bf):
    """dst [P, H] bf16 = broadcast of a [1, H] bf16 dram row across partitions."""
    row = tmp.tile([1, H], BF16, tag="brow", name="brow")
    nc.sync.dma_start(row[:], row_dram)
    for fh in range(2):
        pb = ps.tile([P, 512], F32, tag="ps", name="pbc", space="PSUM")
        nc.tensor.matmul(pb[:, :384], lhsT=ones_bf[:, :P],
                         rhs=row[:, fh * 384:(fh + 1) * 384],
                         start=True, stop=True)
        nc.scalar.activation(dst[:, fh * 384:(fh + 1) * 384], pb[:, :384],
                             AF.Identity)


def build_nc(n_layers=L, mask_ones=True, zero_bias=True, unit_ln=True,
             debug=False):
    nc = bacc.Bacc("TRN2", target_bir_lowering=False, debug=False)

    dd = {}

    def din(name, shape, dtype):
        dd[name] = nc.dram_tensor(name, list(shape), dtype, kind="ExternalInput")
        return dd[name]

    def dout(name, shape, dtype):
        dd[name] = nc.dram_tensor(name, list(shape), dtype, kind="ExternalOutput")
        return dd[name]

    din("word", [V, H], BF16)
    din("ids", [NT, P, 1], I32)
    din("pt", [P, 2, H], BF16)
    din("wq", [L, P, 3, 2, H], FP8)
    din("wk", [L, P, 3, 2, H], FP8)
    din("wv", [L, P, 3, 2, H], FP8)
    din("wo", [L, P, 3, 2, H], FP8)
    din("w1", [L, FC, P, 3, 2, P], FP8)   # [l, j, ki, c2, ko, m]
    din("w2", [L, P, FC // 2, 2, H], FP8)  # [l, ki, c2, ko, n]
    din("clsw", [P, 3, 2, 16], FP8)      # K padded to 16
    din("clsb", [K, 1], F32)             # already shifted by -C_SHIFT
    din("mexp", [K, K], F32)             # exp(crf_trans)
    din("mexpt", [K, K], F32)            # exp(crf_trans).T
    din("mrep", [K, BC, 16, K], F32)     # exp(crf_trans) replicated 64x
    din("expstart", [K, 1], F32)         # exp(crf_start)
    din("expend", [K, 1], F32)           # exp(crf_end)
    din("oh9w", [K, T], F32)             # one-hot(labels) * emission weight
    if not mask_ones:
        din("maskrow", [1, T], I32)
        din("maskcols", [P, 2, BC], I32)
    if not zero_bias:
        din("bq", [L, P, HC], F32)
        din("bk", [L, P, HC], F32)
        din("b1", [L, P, FC], F32)
        din("bvrow", [L, 1, H], BF16)
        din("borow", [L, 1, H], BF16)
        din("b2row", [L, 1, H], BF16)
    if not unit_ln:
        din("lng", [L, 2, 1, H], BF16)
        din("lnb", [L, 2, 1, H], BF16)
        din("elng", [2, 1, H], BF16)

    dout("out", [1, 4], F32)
    if debug:
        dout("dbg_x0", [P, NT, H], BF16)
        dout("dbg_x", [P, NT, H], BF16)
        dout("dbg_em", [K, T], F32)

    _build_body(nc, dd, n_layers, mask_ones, zero_bias, unit_ln, debug)
    nc.compile()
    return nc


def _build_body(nc, dd, n_layers, mask_ones, zero_bias, unit_ln, debug):
    (word, ids, pt, wq_d, wk_d, wv_d, wo_d, w1_d, w2_d, clsw_d, clsb_d,
     mexp_d, expstart_d, expend_d, oh9w_d, out_d) = (
        dd["word"], dd["ids"], dd["pt"], dd["wq"], dd["wk"], dd["wv"],
        dd["wo"], dd["w1"], dd["w2"], dd["clsw"], dd["clsb"], dd["mexp"],
        dd["expstart"], dd["expend"], dd["oh9w"], dd["out"])
    if not mask_ones:
        maskrow_d = dd["maskrow"]
        maskcols_d = dd["maskcols"]
    if not zero_bias:
        bq_d, bk_d, b1_d = dd["bq"], dd["bk"], dd["b1"]
        bvrow_d, borow_d, b2row_d = dd["bvrow"], dd["borow"], dd["b2row"]
    if not unit_ln:
        lng_d, lnb_d, elng_d = dd["lng"], dd["lnb"], dd["elng"]
    with tile.TileContext(nc) as tc, contextlib.ExitStack() as octx:
        cst = octx.enter_context(tc.tile_pool(name="cst", bufs=1))
        act = octx.enter_context(tc.tile_pool(name="act", bufs=1))
        ps = octx.enter_context(tc.tile_pool(name="ps", bufs=8, space="PSUM"))

        # ---- persistent activation buffers ----
        x_sb = act.tile([P, NT, H], BF16, tag="x_sb", name="x_sb")
        xT = act.tile([P, HC, T], FP8, tag="xT", name="xT")
        qT = act.tile([P, HC, T], BF16, tag="qT", name="qT")
        kT = act.tile([P, HC, T], BF16, tag="kT", name="kT")
        vS = act.tile([P, NT, NH, DH], BF16, tag="vS", name="vS")
        cT = act.tile([P, HC, T], FP8, tag="cT", name="cT")
        hT = act.tile([P, FC, T // 2], FP8, tag="hT", name="hT")
        emT = act.tile([K, T], F32, tag="emT", name="emT")
        expEm = act.tile([K, T], F32, tag="expEm", name="expEm")

        # ---- constants ----
        ident = cst.tile([P, P], BF16, tag="ident", name="ident")
        make_identity(nc, ident[:])
        ones_bf = cst.tile([1, P], BF16, tag="ones_bf", name="ones_bf")
        nc.vector.memset(ones_bf[:], 1.0)
        # ones64: [128, 64] all-ones lhsT; sum over keys of exp(scores) into
        # one 64-partition half of the normalizer PSUM tile per head
        ones64 = cst.tile([P, DH], BF16, tag="ones64", name="ones64")
        nc.vector.memset(ones64[:], 1.0)
        onescol_f = cst.tile([P, 1], F32, tag="onescol_f", name="onescol_f")
        nc.vector.memset(onescol_f[:], 1.0)

        pt_sb = cst.tile([P, 2, H], BF16, tag="pt_sb", name="pt_sb")
        nc.sync.dma_start(pt_sb[:], pt[:])
        if not mask_ones:
            mcol = cst.tile([P, 2, BC], F32, tag="mcol", name="mcol")
        if not unit_ln:
            elnG = cst.tile([P, H], BF16, tag="elnG", name="elnG")
            elnB = cst.tile([P, H], BF16, tag="elnB", name="elnB")

        with contextlib.ExitStack() as ictx:
            wts = ictx.enter_context(tc.tile_pool(name="wts", bufs=1))
            tmp = ictx.enter_context(tc.tile_pool(name="tmp", bufs=3))

            if not unit_ln:
                _bcast_row(nc, ps, tmp, elnG, elng_d[0], ones_bf)
                _bcast_row(nc, ps, tmp, elnB, elng_d[1], ones_bf)

            # =========== embeddings ===========
            for g in range(NT):
                idx = tmp.tile([P, 1], I32, tag="idx", name="idx")
                nc.sync.dma_start(idx[:], ids[g])
                emb = tmp.tile([P, H], BF16, tag="emb", name="emb")
                nc.gpsimd.indirect_dma_start(
                    out=emb[:], out_offset=None, in_=word[:],
                    in_offset=IndirectOffsetOnAxis(ap=idx[:, :1], axis=0),
                )
                xf = tmp.tile([P, H], F32, tag="xf", name="xf")
                s1e = tmp.tile([P, 1], F32, tag="s1e", name="s1e")
                nc.vector.scalar_tensor_tensor(
                    out=xf[:], in0=emb[:], scalar=0.0,
                    in1=pt_sb[:, g % 2, :], op0=OP.add, op1=OP.add,
                    accum_out=s1e[:])
                _layernorm(nc, tmp, x_sb[:, g, :], xf,
                           None if unit_ln else (elnG, elnB), s1=s1e)
            if debug:
                nc.sync.dma_start(dd["dbg_x0"][:], x_sb[:])
            # prime xT tiles 0-3 for layer 0's QK proj t2=0
            _transpose_tiles(nc, ps, xT, x_sb, ident, range(4))

            if not mask_ones:
                mi = tmp.tile([P, 2, BC], I32, tag="mi", name="mi")
                nc.sync.dma_start(mi[:], maskcols_d[:])
                nc.vector.tensor_scalar(out=mcol[:], in0=mi[:], scalar1=1.0,
                                        scalar2=10000.0, op0=OP.subtract,
                                        op1=OP.mult)

            # =========== encoder layers ===========
            # QKV/O weights are double-buffered and prefetched one layer
            # ahead (layer 0's during the embeddings ramp): V-proj(0-3) and
            # QK-t2=0 at each layer top need wv/wq immediately, and with
            # just-in-time loads those transfers queued behind the big W1/W2
            # preloads.  wv is issued first — it is the first one consumed.
            def _load_qkvo(l):
                wq = wts.tile([P, 3, 2, H], FP8, tag="wq", name=f"wq{l % 2}",
                              bufs=2)
                wk = wts.tile([P, 3, 2, H], FP8, tag="wk", name=f"wk{l % 2}",
                              bufs=2)
                wv = wts.tile([P, 3, 2, H], FP8, tag="wv", name=f"wv{l % 2}",
                              bufs=2)
                wo = wts.tile([P, 3, 2, H], FP8, tag="wo", name=f"wo{l % 2}",
                              bufs=2)
                nc.sync.dma_start(wv[:], wv_d[l])
                nc.sync.dma_start(wq[:], wq_d[l])
                nc.sync.dma_start(wk[:], wk_d[l])
                nc.sync.dma_start(wo[:], wo_d[l])
                return wq, wk, wv, wo

            w_next = _load_qkvo(0)
            for l in range(n_layers):
                wq, wk, wv, wo = w_next
                # preload the whole layer's W1 once (18.4KB/partition): the
                # 24 transfers stream during attention, FFN1 reads them from
                # SBUF in both token-half passes — halves W1 HBM traffic and
                # removes the just-in-time DMA stalls inside the FFN1 loop
                w1js = []
                for j in range(FC):
                    w1j = wts.tile([P, 3, 2, P], FP8, tag="w1j",
                                   name=f"w1j{j}", bufs=FC)
                    nc.sync.dma_start(w1j[:], w1_d[l, j])
                    w1js.append(w1j)
                # W2 likewise preloaded whole (18.4KB/partition, one DMA):
                # FFN2 reads it from SBUF in both token-half passes
                w2full = wts.tile([P, FC // 2, 2, H], FP8, tag="w2f",
                                  name="w2f")
                nc.sync.dma_start(w2full[:], w2_d[l])

                if not zero_bias:
                    bq_sb = wts.tile([P, HC], F32, tag="bq", name="bq")
                    bk_sb = wts.tile([P, HC], F32, tag="bk", name="bk")
                    b1_sb = wts.tile([P, FC], F32, tag="b1", name="b1")
                    nc.sync.dma_start(bq_sb[:], bq_d[l])
                    nc.sync.dma_start(bk_sb[:], bk_d[l])
                    nc.sync.dma_start(b1_sb[:], b1_d[l])
                    bvrow = wts.tile([1, H], BF16, tag="bvrow", name="bvrow")
                    borow = wts.tile([1, H], BF16, tag="borow", name="borow")
                    b2row = wts.tile([1, H], BF16, tag="b2row", name="b2row")
                    nc.sync.dma_start(bvrow[:], bvrow_d[l])
                    nc.sync.dma_start(borow[:], borow_d[l])
                    nc.sync.dma_start(b2row[:], b2row_d[l])
                if not unit_ln:
                    G1 = wts.tile([P, H], BF16, tag="G1", name="G1")
                    B1t = wts.tile([P, H], BF16, tag="B1t", name="B1t")
                    G2 = wts.tile([P, H], BF16, tag="G2", name="G2")
                    B2t = wts.tile([P, H], BF16, tag="B2t", name="B2t")
                    _bcast_row(nc, ps, tmp, G1, lng_d[l, 0], ones_bf)
                    _bcast_row(nc, ps, tmp, B1t, lnb_d[l, 0], ones_bf)
                    _bcast_row(nc, ps, tmp, G2, lng_d[l, 1], ones_bf)
                    _bcast_row(nc, ps, tmp, B2t, lnb_d[l, 1], ones_bf)

                # ---- qT/kT projections, token-half pipelined.  xT tiles 0-3
                #      were already transposed in the previous layer's FFN
                #      tail (or right after the embeddings for layer 0), so
                #      QK proj t2=0 can start while the previous layer's
                #      second-half LN2 chain is still draining; tiles 4-7 are
                #      transposed here in between. ----
                def _v_proj(gs):
                    for g in gs:
                        for fh in range(2):
                            pm = ps.tile([P, 512], F32, tag="ps", name="pv",
                                         space="PSUM")
                            for c2 in range(3):
                                nc.tensor.matmul(
                                    pm[:, :384],
                                    lhsT=xT[:, 2 * c2:2 * c2 + 2,
                                            g * P:(g + 1) * P],
                                    rhs=wv[:, c2, :, fh * 384:(fh + 1) * 384],
                                    start=(c2 == 0),
                                    stop=(c2 == 2 and zero_bias),
                                    perf_mode=DR)
                            if not zero_bias:
                                nc.tensor.matmul(
                                    pm[:, :384], lhsT=ones_bf[:, :P],
                                    rhs=bvrow[:, fh * 384:(fh + 1) * 384],
                                    start=False, stop=True)
                            nc.vector.tensor_scalar(
                                out=vS[:, g, 6 * fh:6 * fh + 6, :],
                                in0=pm[:, :384], scalar1=DQ_WX, scalar2=None,
                                op0=OP.mult)

                for t2 in range(2):
                    if t2 == 1:
                        # V proj of tiles 0-3 also only needs xT 0-3 + wv:
                        # more PE work that is independent of the previous
                        # layer's LN2-th1 chain, emitted before the
                        # transposes that must wait for it
                        _v_proj(range(4))
                        _transpose_tiles(nc, ps, xT, x_sb, ident, range(4, 8))
                    for wmat, bname, dst in ((wq, "bq", qT), (wk, "bk", kT)):
                        for f in range(HC):
                            pm = ps.tile([P, 512], F32, tag="ps", name="pqk",
                                         space="PSUM")
                            for c2 in range(3):
                                nc.tensor.matmul(
                                    pm[:],
                                    lhsT=wmat[:, c2, :, f * P:(f + 1) * P],
                                    rhs=xT[:, 2 * c2:2 * c2 + 2,
                                           t2 * 512:(t2 + 1) * 512],
                                    start=(c2 == 0), stop=(c2 == 2),
                                    perf_mode=DR)
                            if zero_bias:
                                nc.vector.tensor_scalar(
                                    out=dst[:, f, t2 * 512:(t2 + 1) * 512],
                                    in0=pm[:], scalar1=DQ_WX, scalar2=None,
                                    op0=OP.mult)
                            else:
                                bias = (bq_sb if bname == "bq"
                                        else bk_sb)[:, f:f + 1]
                                nc.scalar.activation(
                                    dst[:, f, t2 * 512:(t2 + 1) * 512], pm[:],
                                    AF.Identity, bias=bias, scale=DQ_WX)

                # ---- V projection for tiles 4-7 (0-3 done above) ----
                _v_proj(range(4, NT))

                # ---- output proj + residual + LN1 (interleaved
                #      into the attention loop, per example) ----
                def _oproj_ln1(g):
                    xf = tmp.tile([P, H], F32, tag="xf", name="xf")
                    s1a = tmp.tile([P, 1], F32, tag="s1a", name="s1a")
                    s1b = tmp.tile([P, 1], F32, tag="s1b", name="s1b")
                    for fh in range(2):
                        pm = ps.tile([P, 512], F32, tag="ps", name="po",
                                     space="PSUM")
                        for c2 in range(3):
                            nc.tensor.matmul(
                                pm[:, :384],
                                lhsT=cT[:, 2 * c2:2 * c2 + 2,
                                        g * P:(g + 1) * P],
                                rhs=wo[:, c2, :, fh * 384:(fh + 1) * 384],
                                start=(c2 == 0),
                                stop=(c2 == 2 and zero_bias),
                                perf_mode=DR)
                        if not zero_bias:
                            nc.tensor.matmul(
                                pm[:, :384], lhsT=ones_bf[:, :P],
                                rhs=borow[:, fh * 384:(fh + 1) * 384],
                                start=False, stop=True)
                        nc.vector.scalar_tensor_tensor(
                            out=xf[:, fh * 384:(fh + 1) * 384],
                            in0=pm[:, :384], scalar=DQ_WX,
                            in1=x_sb[:, g, fh * 384:(fh + 1) * 384],
                            op0=OP.mult, op1=OP.add,
                            accum_out=(s1a[:] if fh == 0 else s1b[:]))
                    s1g = tmp.tile([P, 1], F32, tag="s1g", name="s1g",
                                   bufs=4)
                    nc.vector.tensor_tensor(out=s1g[:], in0=s1a[:],
                                            in1=s1b[:], op=OP.add)
                    _layernorm(nc, tmp, x_sb[:, g, :], xf,
                               None if unit_ln else (G1, B1t), s1=s1g)

                # ---- attention.  Per head-pair: QK^T (2 heads in separate
                #      PE row-groups), fused exp on ACT, then per head both
                #      the AxV matmul and a ones-matmul normalizer sum (z)
                #      on PE.  The normalize tail (fast reciprocal of z +
                #      multiply) is deferred one pair so it overlaps the
                #      next pair's matmul/exp front.  PSUM: 4 banks/pair ->
                #      two pairs in flight. ----
                def _attn_tail(st):
                    e, ch, prbz, pcx = st
                    rb2 = tmp.tile([P, S], F32, tag="rb", name="rb")
                    nc.vector.reciprocal_approx_fast(rb2[:], prbz[:, :S])
                    for hh in range(2):
                        r0 = hh * DH
                        nc.vector.tensor_tensor(
                            out=cT[r0:r0 + DH, ch, e * S:(e + 1) * S],
                            in0=pcx[:DH, hh * S:(hh + 1) * S],
                            in1=rb2[r0:r0 + DH, :], op=OP.mult)

                # QK^T contracts only 64 partitions; heads A/B live in
                # PE row-groups {0,1}/{2,3} (lhsT base 0/64), so
                # alternating their matmuls runs them concurrently.
                # The QK matmuls of pair i+1 are EMITTED before the AV/z
                # matmuls of pair i: PE executes in order, so this gives it
                # work to do while pair i's exp runs on ACT.
                def _emit_qk(e, ch):
                    pscs = [ps.tile([P, 512], F32, tag="ps", name="psc",
                                    space="PSUM") for _ in range(2)]
                    for kt in range(2):
                        for hh in range(2):
                            r0 = hh * DH
                            nc.tensor.matmul(
                                pscs[hh][:, kt * S:(kt + 1) * S],
                                lhsT=kT[r0:r0 + DH, ch,
                                        e * S + kt * P:
                                        e * S + (kt + 1) * P],
                                rhs=qT[r0:r0 + DH, ch, e * S:(e + 1) * S],
                                start=True, stop=True)
                    return pscs

                pairs = [(e, ch) for e in range(BC) for ch in range(NH // 2)]
                prev_st = None
                pscs = _emit_qk(*pairs[0])
                for i, (e, ch) in enumerate(pairs):
                    next_pscs = (_emit_qk(*pairs[i + 1])
                                 if i + 1 < len(pairs) else None)
                    ET2 = tmp.tile([P, 2, 2, S], BF16, tag="ET", name="ET")
                    pcx = ps.tile([P, 512], F32, tag="ps", name="pcx",
                                  space="PSUM")
                    prbz = ps.tile([P, 512], F32, tag="ps", name="prb",
                                   space="PSUM")
                    for hh in range(2):
                        r0 = hh * DH
                        psc = pscs[hh]
                        if mask_ones:
                            # one fused exp over both key tiles
                            nc.scalar.activation(
                                ET2[:, hh].rearrange("p k s -> p (k s)"),
                                psc[:], AF.Exp, bias=0.0, scale=0.125)
                        else:
                            for kt in range(2):
                                nc.scalar.activation(
                                    ET2[:, hh, kt, :],
                                    psc[:, kt * S:(kt + 1) * S],
                                    AF.Exp, bias=mcol[:, kt, e:e + 1],
                                    scale=0.125)
                        h = 2 * ch + hh
                        for kt in range(2):
                            nc.tensor.matmul(
                                pcx[:DH, hh * S:(hh + 1) * S],
                                lhsT=vS[:, 2 * e + kt, h, :],
                                rhs=ET2[:, hh, kt, :],
                                start=(kt == 0), stop=(kt == 1))
                        for kt in range(2):
                            nc.tensor.matmul(
                                prbz[r0:r0 + DH, :S],
                                lhsT=ones64[:],
                                rhs=ET2[:, hh, kt, :],
                                start=(kt == 0), stop=(kt == 1))
                    if prev_st is not None:
                        _attn_tail(prev_st)
                    prev_st = (e, ch, prbz, pcx)
                    pscs = next_pscs
                _attn_tail(prev_st)
                # prefetch the next layer's QKV/O weights: transfers stream
                # during the oproj + FFN phases
                if l + 1 < n_layers:
                    w_next = _load_qkvo(l + 1)
                for g in range(NT):
                    _oproj_ln1(g)

                # ---- FFN (two token-half passes) ----
                for th in range(2):
                    tiles = list(range(4 * th, 4 * th + 4))
                    _transpose_tiles(nc, ps, xT, x_sb, ident, tiles)
                    for j in range(FC):
                        if th == 1 and j == FC - 1:
                            # post-LN2 re-transpose of tiles 0-3 for the next
                            # layer (or classifier), emitted here so its
                            # PSUM->xT copies drain on ACT during the FFN2
                            # window instead of behind the LN2 chain
                            _transpose_tiles(nc, ps, xT, x_sb, ident, range(4))
                        pm = ps.tile([P, 512], F32, tag="ps", name="ph",
                                     space="PSUM")
                        for c2 in range(3):
                            nc.tensor.matmul(
                                pm[:], lhsT=w1js[j][:, c2],
                                rhs=xT[:, 2 * c2:2 * c2 + 2,
                                       th * 512:(th + 1) * 512],
                                start=(c2 == 0), stop=(c2 == 2),
                                perf_mode=DR)
                        bias = 0.0 if zero_bias else b1_sb[:, j:j + 1]
                        nc.scalar.activation(hT[:, j, :], pm[:], AF.Gelu,
                                             bias=bias, scale=DQ_WX)
                    # FFN2: f-half outer so W2 streams once per (th, fh)
                    xfs = [tmp.tile([P, H], F32, tag="xff", name="xff", bufs=4)
                           for _ in range(4)]
                    s1as = [tmp.tile([P, 1], F32, tag="s1fa", name="s1fa",
                                     bufs=4) for _ in range(4)]
                    s1bs = [tmp.tile([P, 1], F32, tag="s1fb", name="s1fb",
                                     bufs=4) for _ in range(4)]
                    for fh in range(2):
                        pms = [ps.tile([P, 512], F32, tag="ps", name="pf2",
                                       space="PSUM") for _ in range(4)]
                        for c2 in range(FC // 2):
                            for gi in range(4):
                                nc.tensor.matmul(
                                    pms[gi][:, :384],
                                    lhsT=hT[:, 2 * c2:2 * c2 + 2,
                                            gi * P:(gi + 1) * P],
                                    rhs=w2full[:, c2, :,
                                               fh * 384:(fh + 1) * 384],
                                    start=(c2 == 0),
                                    stop=(c2 == FC // 2 - 1 and zero_bias),
                                    perf_mode=DR)
                        if not zero_bias:
                            for gi in range(4):
                                nc.tensor.matmul(
                                    pms[gi][:, :384], lhsT=ones_bf[:, :P],
                                    rhs=b2row[:, fh * 384:(fh + 1) * 384],
                                    start=False, stop=True)
                        for gi in range(4):
                            g = tiles[gi]
                            nc.vector.scalar_tensor_tensor(
                                out=xfs[gi][:, fh * 384:(fh + 1) * 384],
                                in0=pms[gi][:, :384], scalar=DQ_W,
                                in1=x_sb[:, g, fh * 384:(fh + 1) * 384],
                                op0=OP.mult, op1=OP.add,
                                accum_out=(s1as[gi][:] if fh == 0
                                           else s1bs[gi][:]))
                    for gi in range(4):
                        s1g = tmp.tile([P, 1], F32, tag="s1g", name="s1g",
                                       bufs=4)
                        nc.vector.tensor_tensor(out=s1g[:], in0=s1as[gi][:],
                                                in1=s1bs[gi][:], op=OP.add)
                        _layernorm(nc, tmp, x_sb[:, tiles[gi], :], xfs[gi],
                                   None if unit_ln else (G2, B2t), s1=s1g)

            if debug:
                nc.sync.dma_start(dd["dbg_x"][:], x_sb[:])

            # =========== classifier ===========
            clsw = cst.tile([P, 3, 2, 16], FP8, tag="clsw", name="clsw")
            nc.sync.dma_start(clsw[:], clsw_d[:])
            clsb = cst.tile([K, 1], F32, tag="clsb", name="clsb")
            nc.sync.dma_start(clsb[:], clsb_d[:])
            # tiles 0-3 already re-transposed in the last layer's FFN tail
            _transpose_tiles(nc, ps, xT, x_sb, ident, range(4, NT))
            for t2 in range(2):
                pm = ps.tile([P, 512], F32, tag="ps", name="pcls", space="PSUM")
                for c2 in range(3):
                    nc.tensor.matmul(
                        pm[:K, :], lhsT=clsw[:, c2, :, 0:K],
                        rhs=xT[:, 2 * c2:2 * c2 + 2, t2 * 512:(t2 + 1) * 512],
                        start=(c2 == 0), stop=(c2 == 2), perf_mode=DR)
                nc.scalar.activation(emT[:, t2 * 512:(t2 + 1) * 512],
                                     pm[:K, :], AF.Identity, bias=clsb[:, :1],
                                     scale=DQ_WX)
            nc.scalar.activation(expEm[:], emT[:], AF.Exp)
            if debug:
                nc.sync.dma_start(dd["dbg_em"][:], emT[:])

        # =========== CRF (weights/tmp pools closed; SBUF freed) ===========
        with contextlib.ExitStack() as cctx:
            crf = cctx.enter_context(tc.tile_pool(name="crf", bufs=1))
            ctmp = cctx.enter_context(tc.tile_pool(name="ctmp", bufs=4))

            def ct(name, shape, dtype=F32):
                return crf.tile(shape, dtype, tag=name, name=name)

            Mexp = ct("Mexp", [K, K])
            nc.sync.dma_start(Mexp[:], mexp_d[:])
            expStart = ct("expStart", [K, 1])
            expEnd = ct("expEnd", [K, 1])
            nc.sync.dma_start(expStart[:], expstart_d[:])
            nc.sync.dma_start(expEnd[:], expend_d[:])
            oh9w = ct("oh9w", [K, T])
            nc.sync.dma_start(oh9w[:], oh9w_d[:])

            # gold-emission dot product: num_dev = sum(emT * oh9w)
            sink9 = ct("sink9", [K, T])
            accK = ct("accK", [K, 1])
            nc.vector.scalar_tensor_tensor(
                out=sink9[:], in0=emT[:], scalar=1.0, in1=oh9w[:],
                op0=OP.mult, op1=OP.mult, accum_out=accK[:])

            # ---- linear-space scan ----
            expEm4 = expEm[:].rearrange("k (b s) -> k b s", b=BC)
            if mask_ones:
                # Chunked scan: alpha_255 = D_255 G_15...G_0 (M^T alpha_0)
                # with B_t = M^T D_t and G_c = B_{16c+16}...B_{16c+1}
                # (G_15 ends at B_254).  The 16 chunk factors G_c^T are built
                # simultaneously, 15 batched rounds of one matmul + one
                # row-scale over all (example, chunk) blocks:
                #   Pt <- D_t (M @ Pt),  t descending within each chunk.
                # The sequential fold is then only 16 steps deep per example.
                CH, CL = 16, 16
                # em_rep[k, t, j] = expEm[k, t]  (j-broadcast via 9 copies,
                # split ACT/DVE; both engines' copies avoid table reloads)
                em_rep = ct("em_rep", [K, T, K])
                srcEm = expEm[:].rearrange("k (t o) -> k t o", o=1)
                for j in range(K):
                    if j % 2 == 0:
                        nc.vector.tensor_copy(em_rep[:, :, j:j + 1], srcEm)
                    else:
                        nc.scalar.copy(em_rep[:, :, j:j + 1], srcEm)
                emr = em_rep[:].rearrange("k (b c s) j -> k b c s j",
                                          b=BC, c=CH)
                mexptS = ct("mexptS", [K, K])
                nc.sync.dma_start(mexptS[:], dd["mexpt"][:])
                mrepS = ct("mrepS", [K, BC, CH, K])
                nc.sync.dma_start(mrepS[:], dd["mrep"][:])
                PtS = ct("PtS", [K, BC, CH, K])
                # init chunks 0..14 at t=16(c+1); chunk 15 at t=254
                nc.vector.tensor_tensor(
                    out=PtS[:, :, 0:CH - 1, :], in0=mrepS[:, :, 0:CH - 1, :],
                    in1=emr[:, :, 1:CH, 0, :], op=OP.mult)
                nc.vector.tensor_tensor(
                    out=PtS[:, :, CH - 1, :], in0=mrepS[:, :, CH - 1, :],
                    in1=emr[:, :, CH - 1, 14, :], op=OP.mult)
                for r in range(1, CL):
                    cmax = CH - 1 if r <= 2 else CH
                    for h in range(2):
                        pu = ps.tile([P, 512], F32, tag="ps", name="pu",
                                     space="PSUM")
                        nc.tensor.matmul(
                            pu[:K, :2 * cmax * K], lhsT=mexptS[:],
                            rhs=PtS[:, 2 * h:2 * h + 2, 0:cmax, :],
                            start=True, stop=True)
                        nc.vector.tensor_tensor(
                            out=PtS[:, 2 * h:2 * h + 2, 0:cmax, :],
                            in0=pu[:K, :2 * cmax * K].rearrange(
                                "k (b c j) -> k b c j", b=2, c=cmax),
                            in1=emr[:, 2 * h:2 * h + 2, 0:cmax, CL - r, :],
                            op=OP.mult)
                # fold: beta0 = M^T @ (expStart * em_0), then 16 steps/example
                a0 = ctmp.tile([K, BC], F32, tag="a0", name="a0")
                nc.vector.tensor_scalar(
                    out=a0[:], in0=expEm4[:, :, 0],
                    scalar1=expStart[:, :1], scalar2=None, op0=OP.mult)
                endem = ctmp.tile([K, BC], F32, tag="endem", name="endem")
                nc.vector.tensor_scalar(
                    out=endem[:], in0=expEm4[:, :, S - 1],
                    scalar1=expEnd[:, :1], scalar2=None, op0=OP.mult)
                pb0 = ps.tile([P, 512], F32, tag="ps", name="pb0",
                              space="PSUM")
                nc.tensor.matmul(pb0[:K, :BC], lhsT=Mexp[:], rhs=a0[:],
                                 start=True, stop=True)
                gams = []
                for b in range(BC):
                    g0 = ctmp.tile([K, 1], F32, tag=f"g{b}", name=f"g0_{b}")
                    nc.vector.tensor_copy(g0[:], pb0[:K, b:b + 1])
                    gams.append(g0)
                F_ = ctmp.tile([K, BC], F32, tag="F", name="F_")
                for c in range(CH):
                    for b in range(BC):
                        pg = ps.tile([P, 512], F32, tag="ps", name="pg",
                                     space="PSUM")
                        nc.tensor.matmul(pg[:K, :1], lhsT=PtS[:, b, c, :],
                                         rhs=gams[b][:], start=True,
                                         stop=True)
                        if c == CH - 1:
                            nc.vector.tensor_tensor(
                                out=F_[:, b:b + 1], in0=pg[:K, :1],
                                in1=endem[:, b:b + 1], op=OP.mult)
                        else:
                            gn = ctmp.tile([K, 1], F32, tag=f"g{b}",
                                           name=f"g{c}_{b}")
                            nc.vector.tensor_copy(gn[:], pg[:K, :1])
                            gams[b] = gn
            else:
                mrow_i = ct("mrow_i", [1, T], I32)
                nc.sync.dma_start(mrow_i[:], dd["maskrow"][:])
                mrow = ct("mrow", [1, T])
                nc.vector.tensor_copy(mrow[:], mrow_i[:])
                inv9 = ct("inv9", [K, T])
                mb9 = ct("mb9", [K, T])
                for i in range(2):
                    pb = ps.tile([P, 512], F32, tag="ps", name="pmb",
                                 space="PSUM")
                    nc.tensor.matmul(pb[:K, :], lhsT=onescol_f[:1, :K],
                                     rhs=mrow[:, i * 512:(i + 1) * 512],
                                     start=True, stop=True)
                    nc.scalar.activation(mb9[:, i * 512:(i + 1) * 512],
                                         pb[:K, :], AF.Identity)
                nc.vector.tensor_scalar(out=inv9[:], in0=mb9[:], scalar1=0.0,
                                        scalar2=None, op0=OP.is_equal)
                inv4 = inv9[:].rearrange("k (b s) -> k b s", b=BC)
                # two independent 2-example chains
                Ecurs = []
                for hf in range(2):
                    Ec = ctmp.tile([K, 2], F32, tag=f"E{hf}", name=f"E0_{hf}")
                    nc.vector.tensor_scalar(
                        out=Ec[:], in0=expEm4[:, 2 * hf:2 * hf + 2, 0],
                        scalar1=expStart[:, :1], scalar2=None, op0=OP.mult)
                    Ecurs.append(Ec)
                for t in range(1, S):
                    for hf in range(2):
                        psn = ps.tile([P, 512], F32, tag="ps", name="pcrf",
                                      space="PSUM")
                        nc.tensor.matmul(psn[:K, :2], lhsT=Mexp[:],
                                         rhs=Ecurs[hf][:],
                                         start=True, stop=True)
                        Enew = ctmp.tile([K, 2], F32, tag=f"E{hf}",
                                         name=f"E{t}_{hf}")
                        nc.vector.tensor_tensor(
                            out=Enew[:], in0=psn[:K, :2],
                            in1=expEm4[:, 2 * hf:2 * hf + 2, t], op=OP.mult)
                        nc.vector.copy_predicated(
                            Enew[:], inv4[:, 2 * hf:2 * hf + 2, t],
                            Ecurs[hf][:])
                        Ecurs[hf] = Enew

                F_ = ctmp.tile([K, BC], F32, tag="F", name="F_")
                for hf in range(2):
                    nc.vector.tensor_scalar(
                        out=F_[:, 2 * hf:2 * hf + 2], in0=Ecurs[hf][:],
                        scalar1=expEnd[:, :1], scalar2=None, op0=OP.mult)
            psd = ps.tile([P, 512], F32, tag="ps", name="psd", space="PSUM")
            nc.tensor.matmul(psd[:1, :BC], lhsT=onescol_f[:K, :], rhs=F_[:],
                             start=True, stop=True)
            denomv = ctmp.tile([1, BC], F32, tag="denomv", name="denomv")
            denom_tot = ct("denom_tot", [1, 1])
            nc.scalar.activation(denomv[:], psd[:1, :BC], AF.Ln,
                                 accum_out=denom_tot[:])

            psn2 = ps.tile([P, 512], F32, tag="ps", name="psn2", space="PSUM")
            nc.tensor.matmul(psn2[:1, :1], lhsT=onescol_f[:K, :],
                             rhs=accK[:], start=True, stop=True)
            num_tot = ct("num_tot", [1, 1])
            nc.vector.tensor_copy(num_tot[:], psn2[:1, :1])
            out_sb = ct("out_sb", [1, 4])
            nc.vector.memset(out_sb[:], 0.0)
            nc.vector.tensor_tensor(out=out_sb[:, 0:1], in0=denom_tot[:],
                                    in1=num_tot[:], op=OP.subtract)
            nc.vector.tensor_copy(out_sb[:, 1:2], num_tot[:])
            nc.vector.tensor_copy(out_sb[:, 2:3], denom_tot[:])
            nc.sync.dma_start(out_d[:], out_sb[:])


# ---------------------------------------------------------------------------
# host wrapper
# ---------------------------------------------------------------------------

_NC_CACHE = {}


def _get_nc(key):
    if key not in _NC_CACHE:
        _NC_CACHE[key] = build_nc(*key)
    return _NC_CACHE[key]


def prepare_maps(inputs, mask_ones, zero_bias, unit_ln):
    """Returns (in_maps, label_const): per-core device inputs and the
    host-computed label-only CRF numerator sum over the whole batch."""
    input_ids = np.asarray(inputs["input_ids"]).astype(np.int32)
    attention_mask = np.asarray(inputs["attention_mask"]).astype(np.int32)
    labels = np.asarray(inputs["labels"]).astype(np.int64)

    word = _bf(inputs["word_emb"])
    pt = _bf((_f32(inputs["pos_emb"][:S]) + _f32(inputs["type_emb"][0])[None, :])
             .reshape(2, P, H).transpose(1, 0, 2))
    wq = _f8(inputs["Wq"], WS).reshape(L, 3, 2, P, H).transpose(
        0, 3, 1, 2, 4).copy()
    wk = _f8(inputs["Wk"], WS).reshape(L, 3, 2, P, H).transpose(
        0, 3, 1, 2, 4).copy()
    wv = _f8(inputs["Wv"], WS).reshape(L, 3, 2, P, H).transpose(
        0, 3, 1, 2, 4).copy()
    wo = _f8(inputs["Wo"], WS).reshape(L, 3, 2, P, H).transpose(
        0, 3, 1, 2, 4).copy()
    w1 = (_f8(inputs["W1"], WS).reshape(L, 3, 2, P, FC, P)
          .transpose(0, 4, 3, 1, 2, 5).copy())
    w2 = (_f8(inputs["W2"], WS).reshape(L, FC // 2, 2, P, H)
          .transpose(0, 3, 1, 2, 4).copy())
    cwpad = np.zeros((H, 16), np.float32)
    cwpad[:, :K] = _f32(inputs["cls_W"])
    clsw = _f8(cwpad, WS).reshape(3, 2, P, 16).transpose(2, 0, 1, 3).copy()
    clsb = (_f32(inputs["cls_b"]) - np.float32(C_SHIFT)).reshape(K, 1)

    trans = _f32(inputs["crf_trans"]).reshape(K, K)
    startv = _f32(inputs["crf_start"]).reshape(K)
    endv = _f32(inputs["crf_end"]).reshape(K)

    shared = dict(
        word=word, pt=pt, wq=wq, wk=wk, wv=wv, wo=wo, w1=w1, w2=w2,
        clsw=clsw, clsb=clsb,
        mexp=np.exp(trans).astype(np.float32),
        mexpt=np.ascontiguousarray(np.exp(trans).T.astype(np.float32)),
        mrep=np.ascontiguousarray(np.broadcast_to(
            np.exp(trans).astype(np.float32)[:, None, None, :],
            (K, BC, 16, K))),
        expstart=np.exp(startv).astype(np.float32).reshape(K, 1),
        expend=np.exp(endv).astype(np.float32).reshape(K, 1),
    )
    if not zero_bias:
        shared.update(
            bq=_f32(inputs["bq"]).reshape(L, HC, P).transpose(0, 2, 1).copy(),
            bk=_f32(inputs["bk"]).reshape(L, HC, P).transpose(0, 2, 1).copy(),
            b1=_f32(inputs["b1"]).reshape(L, FC, P).transpose(0, 2, 1).copy(),
            bvrow=_bf(_f32(inputs["bv"]) / DQ_WX).reshape(L, 1, H),
            borow=_bf(_f32(inputs["bo"]) / DQ_WX).reshape(L, 1, H),
            b2row=_bf(_f32(inputs["b2"]) / DQ_W).reshape(L, 1, H),
        )
    if not unit_ln:
        shared.update(
            lng=np.stack([_bf(inputs["ln1_g"]), _bf(inputs["ln2_g"])],
                         axis=1).reshape(L, 2, 1, H),
            lnb=np.stack([_bf(inputs["ln1_b"]), _bf(inputs["ln2_b"])],
                         axis=1).reshape(L, 2, 1, H),
            elng=np.stack([_bf(inputs["emb_ln_g"]), _bf(inputs["emb_ln_b"])],
                          axis=0).reshape(2, 1, H),
        )

    # ---- host label-only numerator + per-core oh9w ----
    mf = attention_mask.astype(np.float32)               # [B, S]
    w9 = mf.copy()
    w9[:, 0] = 1.0                                       # t=0 emission always counted
    trans_gold = trans[labels[:, :-1], labels[:, 1:]]    # [B, S-1]
    last_idx = attention_mask.astype(np.int64).sum(axis=1) - 1
    label_num = (startv[labels[:, 0]]
                 + (trans_gold * mf[:, 1:]).sum(axis=1)
                 + endv[labels[np.arange(B), last_idx]])  # [B]
    label_const = float(np.float32(label_num.astype(np.float32).sum()))

    in_maps = []
    for c in range(CORES):
        ids_c = input_ids[BC * c:BC * (c + 1)].reshape(NT, P, 1).copy()
        lab_c = labels[BC * c:BC * (c + 1)]              # [BC, S]
        w9_c = w9[BC * c:BC * (c + 1)]                   # [BC, S]
        oh = np.zeros((K, BC, S), np.float32)
        oh[lab_c.reshape(-1), np.repeat(np.arange(BC), S),
           np.tile(np.arange(S), BC)] = w9_c.reshape(-1)
        msk_c = attention_mask[BC * c:BC * (c + 1)]
        m = dict(shared)
        m["ids"] = ids_c
        m["oh9w"] = oh.reshape(K, T).copy()
        if not mask_ones:
            m["maskrow"] = msk_c.reshape(1, T).copy()
            m["maskcols"] = (msk_c.reshape(BC, 2, P).transpose(2, 1, 0)
                             .astype(np.int32).copy())
        in_maps.append(m)
    return in_maps, label_const


def kernel(**inputs) -> np.ndarray:
    attention_mask = np.asarray(inputs["attention_mask"])
    assert np.asarray(inputs["input_ids"]).shape == (B, S)

    mask_ones = bool(np.all(attention_mask == 1))
    zero_bias = all(
        not np.any(np.asarray(inputs[k]))
        for k in ("bq", "bk", "bv", "bo", "b1", "b2"))
    unit_ln = (all(np.all(np.asarray(inputs[k]) == 1.0)
                   for k in ("emb_ln_g", "ln1_g", "ln2_g"))
               and all(not np.any(np.asarray(inputs[k]))
                       for k in ("emb_ln_b", "ln1_b", "ln2_b")))

    n_layers = int(os.environ.get("BERTCRF_LAYERS", L))
    debug = bool(int(os.environ.get("BERTCRF_DEBUG", "0")))
    nc = _get_nc((n_layers, mask_ones, zero_bias, unit_ln, debug))
    in_maps, label_const = prepare_maps(inputs, mask_ones, zero_bias, unit_ln)

    res = run_bass_kernel_spmd(nc, in_maps, core_ids=list(range(CORES)))
    total = np.float32(0.0)
    for c in range(CORES):
        total += np.float32(res.results[c]["out"][0, 0])
    return np.float32(total - np.float32(label_const))


if __name__ == "__main__":
    import jax
    jax.config.update("jax_platforms", "cpu")
    import reference
    inp = reference.setup_inputs()
    outv = kernel(**{k: np.asarray(v) for k, v in inp.items()})
    print("kernel:", outv)
